# revision 11
# baseline (speedup 1.0000x reference)
"""Trainium2 Bass kernel for nn_BaselineGCN (2-layer GCN + BN + mean-pool + MLP head).

Strategy (8 NeuronCores):
 - Nodes sharded contiguously across cores; each core owns the in-edges of its
   node shard (dst-sharding, per the graph-partitioning hint).
 - gcn_norm factorized: deg/dinv computed host-side (index/weight preprocessing);
   per-edge message = w_e * h'[src] with h' = dinv * (h @ W); per-dst scale by
   dinv[dst] folded into the ACT-engine epilogue.
 - The per-edge gather h'[src] runs on-device via SWDGE dma_gather (256B rows)
   from an AllGather'ed replica of h' in each core's DRAM, round-robined over 4
   SWDGE queues.
 - segment_sum becomes TensorE matmuls: per 128-edge chunk, a host-precomputed
   one-hot-times-weight fp8 matrix B[e, dst_local] is loaded as lhsT and PE
   accumulates B.T @ gathered_rows into the dst-block's PSUM tile.  The BN bias
   term (Cb) and the self-loop (+h'[dst]) are injected into the same PSUM
   accumulation via a K=1 outer-product matmul and a bf16 identity matmul, so
   the whole per-block epilogue is ONE ScalarE op: y = Relu(acc * dinv[dst]).
 - VectorE is kept almost empty: the v1 kernel was DVE+GPSIMD co-bottlenecked
   (DVE ~70% busy on epilogue/cast chains created backpressure that limited
   SWDGE descriptor-gen queue concurrency to ~2 of 4).
 - Graph mean-pool is a host-precomputed fp8 one-hot matmul accumulated inline
   with layer-1 epilogues; partials summed with an AllReduce; the tiny MLP
   head + log_softmax run on every core.
"""
import sys
import time

sys.path.insert(0, "/opt/trn_rl_repo")

import numpy as np

P = 128          # partitions / block size
NWIN = 4         # gather index windows (int16 range)
MAXCALL = 1024   # max indices per dma_gather (SWDGE ring capacity)
DMA_SCRATCH = 32768  # SWDGE ring carveout bytes (ring = this // 16 descs)
NQUEUES = 4      # SWDGE queues to round-robin
GBUFS = 16       # gather tile lookahead
BBUFS = 8        # DVE-built per-chunk B tile lookahead
PACCB = 4        # PSUM accumulator banks


# ---------------------------------------------------------------- host prep --

def _ceil(a, b):
    return -(-a // b)


class GCNStructure:
    """Graph partitioning + stream layout. Capacities are maxed across cores so
    the single SPMD program fits every core's data."""

    def __init__(self, src, dst, ew, batch, N, G, ncores):
        self.N, self.G, self.C = N, G, ncores
        NSH = N // ncores
        NB = _ceil(NSH, P)
        WS = _ceil(N // 2, NWIN // 2)  # window size in PAIRED rows
        assert N % ncores == 0
        assert WS <= 32767, "gather window exceeds int16"
        self.NSH, self.NB, self.WS = NSH, NB, WS
        self.LB = NSH - (NB - 1) * P  # rows in last block

        core = dst // NSH
        blk = (dst % NSH) // P
        # table permutation for split AllGathers: half H of every core's shard
        # is gathered into table-half H; row = H*WS + src_core*HWS + local//2
        HWS = NSH // 4  # pair-rows per (core, half)
        self.HWS = HWS
        sc = src // NSH
        sl = src % NSH
        H = sl // (NSH // 2)
        permrow = sc * HWS + (sl % (NSH // 2)) // 2
        # cell = (half, parity): row packs 2 nodes per 256B
        win = H * 2 + (src & 1)
        key = (core * NB + blk) * NWIN + win
        # secondary sort by permuted table row: ascending gather addresses
        # within each (core, block, cell) group improve DRAM read locality
        order = np.lexsort((permrow, key))
        self.permrow_s = permrow[order]
        self.src_s, self.dst_s, self.ew_s = src[order], dst[order], ew[order]
        counts = np.bincount(key, minlength=ncores * NB * NWIN).reshape(ncores, NB, NWIN)
        self.counts = counts
        cap = counts.max(axis=0)
        cap = _ceil(np.maximum(cap, 0), P) * P  # per (b, w), 0 stays 0
        self.cap = cap  # [NB, NWIN]

        # supergroups: consecutive blocks such that per-window call <= MAXCALL
        self.sgs = []
        cur = [0]
        for b in range(1, NB):
            trial = cur + [b]
            if all(cap[trial, w].sum() <= MAXCALL for w in range(NWIN)):
                cur = trial
            else:
                self.sgs.append(cur)
                cur = [b]
        self.sgs.append(cur)

        # layout: gather calls in (sg, w) order; chunks in (sg, b, w, j) order
        self.gcols = {}    # (sgi, w) -> columns in that call's tile
        self.icol = {}     # (sgi, w) -> start col (units of 16-idx) in idx stream
        self.coloff = {}   # (b, w) -> column offset inside its call tile
        sid = 0
        for sgi, sg in enumerate(self.sgs):
            for w in range(NWIN):
                cols = int(cap[sg, w].sum()) // P
                self.gcols[(sgi, w)] = cols
                self.icol[(sgi, w)] = sid
                off = 0
                for b in sg:
                    self.coloff[(b, w)] = off
                    off += int(cap[b, w]) // P
                sid += cols * 8  # n/16 = cols*128/16
        self.SID = max(sid, 8)
        self.CT = max(int(cap.sum()) // P, 1)
        self.GMAX = max(max(self.gcols.values(), default=1), 1)
        self.SGMAX = max(sum(int(cap[b].sum()) // P for b in sg) for sg in self.sgs)

        # per-core edge offsets into the sorted arrays, per (b, w)
        cum = np.zeros(ncores * NB * NWIN + 1, np.int64)
        np.cumsum(counts.reshape(-1), out=cum[1:])
        self.grp_start = cum  # index by (c*NB+b)*NWIN+w

        # batch / counts for pooling
        self.cnt = np.bincount(batch, minlength=G).astype(np.float32)
        self.inv_cnt = (1.0 / np.maximum(self.cnt, 1.0)).astype(np.float32)

    def core_streams(self, c, dinv):
        """Build per-core device streams: idx [128, SID] i16,
        dstloc/val [128, CT] f32, dinvb [128, NB] f32."""
        NB, WS, NSH = self.NB, self.WS, self.NSH
        idx_cols = np.zeros((128, self.SID), np.int16)
        dstloc = np.zeros((128, self.CT), np.float32)
        val = np.zeros((128, self.CT), np.float32)

        t = 0
        for sgi, sg in enumerate(self.sgs):
            # gather stream: (w, b) order
            for w in range(NWIN):
                col = self.icol[(sgi, w)]
                parts = []
                for b in sg:
                    g0 = self.grp_start[(c * NB + b) * NWIN + w]
                    g1 = self.grp_start[(c * NB + b) * NWIN + w + 1]
                    loc = self.permrow_s[g0:g1].astype(np.int16)
                    pad = int(self.cap[b, w]) - (g1 - g0)
                    parts.append(np.concatenate([loc, np.zeros(pad, np.int16)]))
                if parts:
                    flat = np.concatenate(parts)
                    if flat.size:
                        wrapped = np.tile(flat.reshape(-1, 16).T, (8, 1))
                        idx_cols[:, col:col + flat.size // 16] = wrapped
            # value/dst streams: (b, w, chunk) order
            for b in sg:
                for w in range(NWIN):
                    g0 = self.grp_start[(c * NB + b) * NWIN + w]
                    g1 = self.grp_start[(c * NB + b) * NWIN + w + 1]
                    n = g1 - g0
                    capbw = int(self.cap[b, w])
                    if capbw == 0:
                        continue
                    dl = np.zeros(capbw, np.float32)
                    vv = np.zeros(capbw, np.float32)
                    dl[:n] = (self.dst_s[g0:g1] - (c * NSH + b * P)).astype(np.float32)
                    vv[:n] = self.ew_s[g0:g1]
                    k = capbw // P
                    dstloc[:, t:t + k] = dl.reshape(k, P).T
                    val[:, t:t + k] = vv.reshape(k, P).T
                    t += k

        dinvb = np.zeros((128, NB), np.float32)
        sh_dinv = dinv[c * NSH:(c + 1) * NSH]
        for b in range(NB):
            nb = P if b < NB - 1 else self.LB
            dinvb[:nb, b] = sh_dinv[b * P:b * P + nb]
        return idx_cols, dstloc, val, dinvb

    def core_pool_oh(self, c, batch):
        """fp8 one-hot pooling matrix [128, NB*128]: [p, b*128+g] = (batch==g)."""
        import ml_dtypes
        FP8 = np.dtype(ml_dtypes.float8_e4m3)
        NB, NSH = self.NB, self.NSH
        oh = np.zeros((128, NB * 128), FP8)
        sh = batch[c * NSH:(c + 1) * NSH]
        for b in range(NB):
            nb = P if b < NB - 1 else self.LB
            g = sh[b * P:b * P + nb].astype(np.int64)
            oh[np.arange(nb), b * 128 + g] = 1.0
        return oh


def _host_prep(x, edge_index, batch, edge_attr, params, ncores):
    """All index-based preprocessing + BN folding. Returns (struct, in_maps)."""
    N, INDIM = x.shape
    G = int(params["cnt_G"])
    EPS = 1e-5

    src = np.asarray(edge_index[0], np.int64)
    dst = np.asarray(edge_index[1], np.int64)
    ew = np.asarray(edge_attr, np.float32)
    batch = np.asarray(batch, np.int64)

    deg = np.bincount(dst, weights=ew.astype(np.float64), minlength=N) + 1.0
    dinv = (1.0 / np.sqrt(deg)).astype(np.float32)

    st = GCNStructure(src, dst, ew, batch, N, G, ncores)

    def bnfold(g, be, m, v, bias):
        s = (g / np.sqrt(v + EPS)).astype(np.float32)
        cc = ((bias - m) * s + be).astype(np.float32)
        return s, cc

    S0, C0 = bnfold(params["g0"], params["be0"], params["m0"], params["v0"], params["b0"])
    S1, C1 = bnfold(params["g1"], params["be1"], params["m1"], params["v1"], params["b1"])
    Sf, Cf = bnfold(params["gf"], params["bef"], params["mf"], params["vf"], params["bf1"])

    # BN scale folded into the weight matrices (linear before the bias add)
    W0s = (np.asarray(params["W0"], np.float32) * S0[None, :]).astype(np.float32)
    W1s = (np.asarray(params["W1"], np.float32) * S1[None, :]).astype(np.float32)
    Wf1s = (np.asarray(params["Wf1"], np.float32) * Sf[None, :]).astype(np.float32)

    HID = params["W0"].shape[1]
    HHID = params["Wf1"].shape[1]
    NCLASS = params["Wf2"].shape[1]

    ident = np.eye(128, dtype=np.float32)
    iota = np.tile(np.arange(128, dtype=np.float32)[None, :], (128, 1))

    NSH, NB = st.NSH, st.NB

    import ml_dtypes
    FP8 = np.dtype(ml_dtypes.float8_e4m3)
    BF16 = np.dtype(ml_dtypes.bfloat16)

    xpadT = np.zeros((ncores, INDIM, NB * P), BF16)
    xv = np.asarray(x, np.float32)
    for c in range(ncores):
        xpadT[c, :, :NSH] = xv[c * NSH:(c + 1) * NSH].T.astype(BF16)

    in_maps = []
    for c in range(ncores):
        idx_cols, dstloc, val, dinvb = st.core_streams(c, dinv)
        pool_oh = st.core_pool_oh(c, batch)
        # invd_flat[0, b*128+j] = 1/dinv at node (c, b, j); 0 for pad rows.
        # (single partition: Ldweights requires lhsT at partition 0)
        invd_flat = np.zeros((1, NB * 128), np.float32)
        sh_dinv = dinv[c * NSH:(c + 1) * NSH]
        for b in range(NB):
            nb = P if b < NB - 1 else st.LB
            invd_flat[0, b * 128:b * 128 + nb] = 1.0 / sh_dinv[b * P:b * P + nb]
        # cRep[b, :] = C (bias row replicated so lhsT/rhs base partitions match)
        cRep0 = np.tile(C0[None, :], (128, 1)).astype(BF16)
        cRep1 = np.tile(C1[None, :], (128, 1)).astype(BF16)
        in_maps.append(dict(
            xshT=xpadT[c],
            idxs=idx_cols,
            dstloc=dstloc, bval=val,
            pool_oh=pool_oh, dinvb=dinvb,
            invd_flat=invd_flat.astype(BF16),
            cRep0=cRep0, cRep1=cRep1,
            w0=W0s.astype(BF16), w1=W1s, wf1=Wf1s,
            wf2=np.asarray(params["Wf2"], np.float32),
            cfb=np.tile(Cf[None, :], (128, 1)),
            bf2b=np.tile(np.asarray(params["bf2"], np.float32)[None, :], (128, 1)),
            invcnt=st.inv_cnt[:, None].copy(),
            ident=ident, identb=ident.astype(BF16), iota=iota,
        ))
    dims = dict(INDIM=INDIM, HID=HID, HHID=HHID, NCLASS=NCLASS)
    return st, in_maps, dims


# ------------------------------------------------------------- bass program --

def build_nc(st, dims, ncores, reps=1, fake_coll=False, no_gather=False, no_compute=False):
    from concourse import bass, mybir, bacc, tile

    INDIM, HID, HHID, NCLASS = dims["INDIM"], dims["HID"], dims["HHID"], dims["NCLASS"]
    N, G, NB, NSH, WS, LB = st.N, st.G, st.NB, st.NSH, st.WS, st.LB
    f32 = mybir.dt.float32
    bf16 = mybir.dt.bfloat16
    fp8 = mybir.dt.float8e4
    Alu = mybir.AluOpType
    Act = mybir.ActivationFunctionType

    nc = bacc.Bacc("TRN2", target_bir_lowering=False, debug=False,
                   enable_asserts=True, num_devices=ncores,
                   num_swdge_queues=NQUEUES,
                   dynamic_dma_scratch_size=DMA_SCRATCH)

    I = {}
    def inp(name, shape, dt=f32):
        I[name] = nc.dram_tensor(name, shape, dt, kind="ExternalInput")
        return I[name]

    inp("xshT", [INDIM, NB * P], bf16)
    inp("idxs", [128, st.SID], mybir.dt.int16)
    inp("dstloc", [128, st.CT]); inp("bval", [128, st.CT])
    inp("pool_oh", [128, NB * 128], mybir.dt.float8e4)
    inp("dinvb", [128, NB])
    inp("invd_flat", [1, NB * 128], bf16)
    inp("cRep0", [128, HID], bf16); inp("cRep1", [128, HID], bf16)
    inp("w0", [INDIM, HID], bf16); inp("w1", [HID, HID])
    inp("wf1", [HID, HHID]); inp("wf2", [HHID, NCLASS])
    inp("cfb", [128, HHID])
    inp("bf2b", [128, NCLASS])
    inp("invcnt", [128, 1])
    inp("ident", [128, 128]); inp("identb", [128, 128], bf16)
    inp("iota", [128, 128])
    out_d = nc.dram_tensor("out", [G, NCLASS], f32, kind="ExternalOutput")

    qctr = [0]
    def next_q():
        q = qctr[0] % NQUEUES
        qctr[0] += 1
        return q

    with tile.TileContext(nc) as tc:
        import contextlib
        with contextlib.ExitStack() as ctx:
            const = ctx.enter_context(tc.tile_pool(name="const", bufs=1))
            stream = ctx.enter_context(tc.tile_pool(name="stream", bufs=1))
            xio = ctx.enter_context(tc.tile_pool(name="xio", bufs=3))
            xts = ctx.enter_context(tc.tile_pool(name="xts", bufs=3))
            hpool = ctx.enter_context(tc.tile_pool(name="hpool", bufs=NB))
            ypool = ctx.enter_context(tc.tile_pool(name="ypool", bufs=NB))
            y1pool = ctx.enter_context(tc.tile_pool(name="y1pool", bufs=NB))
            gpool = ctx.enter_context(tc.tile_pool(name="gpool", bufs=GBUFS))
            bpool = ctx.enter_context(tc.tile_pool(name="bpool", bufs=BBUFS))
            tmp = ctx.enter_context(tc.tile_pool(name="tmp", bufs=6))
            ptrans = ctx.enter_context(tc.tile_pool(name="ptrans", bufs=1, space="PSUM"))
            phw = ctx.enter_context(tc.tile_pool(name="phw", bufs=2, space="PSUM"))
            pacc = ctx.enter_context(tc.tile_pool(name="pacc", bufs=PACCB, space="PSUM"))
            ppool = ctx.enter_context(tc.tile_pool(name="ppool", bufs=1, space="PSUM"))
            dram = ctx.enter_context(tc.tile_pool(name="dram", bufs=1, space="DRAM"))

            # ---- constants into SBUF
            C = {}
            cdts = dict(w0=bf16, invd_flat=bf16, cRep0=bf16, cRep1=bf16, identb=bf16,
                        pool_oh=fp8)
            for nm in ["w0", "w1", "wf1", "wf2", "cfb", "bf2b", "invcnt",
                       "ident", "identb", "iota", "invd_flat", "cRep0", "cRep1",
                       "pool_oh"]:
                shape = list(I[nm].shape)
                tile_ = const.tile(shape, cdts.get(nm, f32), tag=nm)
                nc.sync.dma_start(out=tile_[:], in_=I[nm][:])
                C[nm] = tile_
            idx_t = stream.tile([128, st.SID], mybir.dt.int16, tag="idx")
            nc.sync.dma_start(out=idx_t[:], in_=I["idxs"][:])
            dstloc_t = stream.tile([128, st.CT], f32, tag="dstloc")
            nc.sync.dma_start(out=dstloc_t[:], in_=I["dstloc"][:])
            bval_t = stream.tile([128, st.CT], f32, tag="bval")
            nc.sync.dma_start(out=bval_t[:], in_=I["bval"][:])
            dinv_t = stream.tile([128, NB], f32, tag="dnv")
            nc.sync.dma_start(out=dinv_t[:], in_=I["dinvb"][:])

            shspace = "Shared" if ncores > 4 else "Local"
            HSH = NSH // 2  # nodes per half-shard
            bounce = [[dram.tile([HSH, HID], bf16, tag=f"bnc{l}{h}",
                                 name=f"bounce{l}{h}") for h in range(2)]
                      for l in range(2)]
            ar_in = dram.tile([G, HID], f32, tag="arin")

            # ---- phase A: h0' = dinv * (x @ W0), shard -> AllGather table0
            for _rep in range(reps):
              # per-rep Shared tables: a Shared DRAM tile allows only one writer
              # split per half for split AllGathers (overlap with compute)
              table = [[dram.tile([WS, 2 * HID], bf16, tag=f"tab{l}{h}_{_rep}",
                                  name=f"table{l}{h}_{_rep}",
                                  addr_space=shspace) for h in range(2)]
                       for l in range(2)]
              ar_out = dram.tile([G, HID], f32, tag=f"arout_{_rep}", addr_space=shspace)

              def bounce_write(l, b, nb, tile_):
                  # write rows [b*P, b*P+nb) of the shard into half bounces
                  r0, r1 = b * P, b * P + nb
                  if r0 < HSH:
                      e = min(r1, HSH)
                      nc.sync.dma_start(out=bounce[l][0][r0:e, :], in_=tile_[:e - r0, :])
                  if r1 > HSH:
                      s = max(r0, HSH)
                      nc.sync.dma_start(out=bounce[l][1][s - HSH:r1 - HSH, :],
                                        in_=tile_[s - r0:nb, :])

              def allgather_half(l, h):
                  if fake_coll:
                      nc.sync.dma_start(out=table[l][h][0:HSH // 2, :],
                                        in_=bounce[l][h][:])
                  else:
                      nc.gpsimd.collective_compute(
                          "AllGather", Alu.bypass,
                          replica_groups=[list(range(ncores))],
                          ins=[bounce[l][h].opt()], outs=[table[l][h].opt()],
                      )

              # last block fully inside half 0: fire AG half 0 after it
              bsplit = (HSH - 1) // P  # block containing row HSH-1

              h_tiles = []
              for b in range(NB):
                  nb = P if b < NB - 1 else LB
                  xt = xio.tile([128, 128], bf16, tag="xt")
                  nc.sync.dma_start(out=xt[:INDIM, :], in_=I["xshT"][:, b * P:(b + 1) * P])
                  hp = phw.tile([128, HID], f32, tag="hp")
                  nc.tensor.matmul(hp[:], lhsT=xt[:INDIM, :], rhs=C["w0"][:],
                                   start=True, stop=True)
                  hb16 = hpool.tile([128, HID], bf16, tag="h")
                  nc.scalar.activation(out=hb16[:], in_=hp[:], func=Act.Copy,
                                       scale=dinv_t[:, b:b + 1])
                  bounce_write(0, b, nb, hb16)
                  h_tiles.append(hb16)
                  if b == bsplit:
                      allgather_half(0, 0)
              allgather_half(0, 1)

              # ---- GCN layers
              pp = ppool.tile([128, HID], f32, tag="pool")
              for l in range(2):
                  cRep = C["cRep0"] if l == 0 else C["cRep1"]
                  h1_tiles = []
                  t = 0
                  for sgi, sg in enumerate(st.sgs):
                      gt = {}
                      for w in range(NWIN):
                          cols = st.gcols[(sgi, w)]
                          if cols == 0:
                              continue
                          gbf = gpool.tile([128, st.GMAX, 2 * HID], bf16, tag="g")
                          ic = st.icol[(sgi, w)]
                          gt[w] = gbf
                          if no_gather:
                              continue
                          nc.gpsimd.dma_gather(
                              out_ap=gbf[:, :cols, :],
                              in_ap=table[l][w // 2][:, :],
                              idxs_ap=idx_t[:, ic:ic + cols * 8],
                              num_idxs=cols * P,
                              num_idxs_reg=cols * P,
                              elem_size=2 * HID,
                              queue_num=next_q(),
                          )
                      for b in sg:
                          nchunks = 0 if no_compute else int(st.cap[b].sum()) // P
                          acc = pacc.tile([128, HID], f32, tag="acc")
                          # inject Cb/dinv[dst] (outer product, K=1), then the
                          # self-loop h'[dst] (identity matmul), then the edges
                          nc.tensor.matmul(acc[:],
                                           lhsT=C["invd_flat"][0:1, b * 128:(b + 1) * 128],
                                           rhs=cRep[0:1, :],
                                           start=True, stop=False)
                          nc.tensor.matmul(acc[:], lhsT=C["identb"][:],
                                           rhs=h_tiles[b][:],
                                           start=False, stop=(nchunks == 0))
                          done = 0
                          for w in range(NWIN if not no_compute else 0):
                              kk = int(st.cap[b, w]) // P
                              for j in range(kk):
                                  Bt = bpool.tile([128, 128], bf16, tag="B")
                                  nc.vector.tensor_scalar(
                                      out=Bt[:], in0=C["iota"][:],
                                      scalar1=dstloc_t[:, t:t + 1],
                                      scalar2=bval_t[:, t:t + 1],
                                      op0=Alu.is_equal, op1=Alu.mult)
                                  nc.tensor.matmul(
                                      acc[:], lhsT=Bt[:],
                                      rhs=gt[w][:, st.coloff[(b, w)] + j,
                                                (w % 2) * HID:(w % 2 + 1) * HID],
                                      start=False, stop=(done == nchunks - 1))
                                  done += 1
                                  t += 1
                          nb = P if b < NB - 1 else LB
                          if l == 0:
                              # y0 (f32) -> transpose -> h1' = dinv*(y0 @ W1)
                              yb = ypool.tile([128, HID], f32, tag="y")
                              nc.scalar.activation(out=yb[:], in_=acc[:], func=Act.Relu,
                                                   scale=dinv_t[:, b:b + 1])
                              pt = ptrans.tile([128, 128], f32, tag="pt")
                              nc.tensor.transpose(pt[:HID, :], yb[:], C["ident"][:])
                              yTs = xts.tile([128, 128], f32, tag="xT")
                              nc.scalar.activation(out=yTs[:HID, :], in_=pt[:HID, :],
                                                   func=Act.Copy)
                              hp = phw.tile([128, HID], f32, tag="hp")
                              nc.tensor.matmul(hp[:], lhsT=yTs[:HID, :], rhs=C["w1"][:],
                                               start=True, stop=True)
                              h1b = hpool.tile([128, HID], bf16, tag="h")
                              nc.scalar.activation(out=h1b[:], in_=hp[:], func=Act.Copy,
                                                   scale=dinv_t[:, b:b + 1])
                              bounce_write(1, b, nb, h1b)
                              h1_tiles.append(h1b)
                          else:
                              # y1 (bf16) -> inline mean-pool accumulation
                              yb = y1pool.tile([128, HID], bf16, tag="y1")
                              nc.scalar.activation(out=yb[:], in_=acc[:], func=Act.Relu,
                                                   scale=dinv_t[:, b:b + 1])
                              nc.tensor.matmul(pp[:G, :],
                                               lhsT=C["pool_oh"][:nb, b * 128:b * 128 + G],
                                               rhs=yb[:nb, :],
                                               start=(b == 0), stop=(b == NB - 1))
                  if l == 0:
                      h_tiles = h1_tiles
                      allgather_half(1, 0)
                      allgather_half(1, 1)

              # ---- mean pool partials -> AllReduce -> head
              pooled = tmp.tile([128, HID], f32, tag="pl")
              nc.scalar.activation(out=pooled[:G, :], in_=pp[:G, :], func=Act.Copy)
              nc.sync.dma_start(out=ar_in[:], in_=pooled[:G, :])
              if fake_coll:
                  nc.sync.dma_start(out=ar_out[:], in_=ar_in[:])
              else:
                  nc.gpsimd.collective_compute(
                      "AllReduce", Alu.add,
                      replica_groups=[list(range(ncores))],
                      ins=[ar_in.opt()], outs=[ar_out.opt()],
                  )
              pooled2 = tmp.tile([128, HID], f32, tag="pl2")
              nc.sync.dma_start(out=pooled2[:G, :], in_=ar_out[:])
              nc.vector.tensor_scalar(out=pooled2[:G, :], in0=pooled2[:G, :],
                                      scalar1=C["invcnt"][:G, :], scalar2=None,
                                      op0=Alu.mult)

              # z = relu(Sf * (pooled @ Wf1) + Cf)
              pt = ptrans.tile([128, 128], f32, tag="pt")
              nc.tensor.transpose(pt[:HID, :G], pooled2[:G, :], C["ident"][:])
              pTs = xts.tile([128, 128], f32, tag="xT")
              nc.vector.tensor_copy(out=pTs[:HID, :G], in_=pt[:HID, :G])
              zp = phw.tile([128, HHID], f32, tag="hp")
              nc.tensor.matmul(zp[:G, :], lhsT=pTs[:HID, :G], rhs=C["wf1"][:],
                               start=True, stop=True)
              z = tmp.tile([128, HHID], f32, tag="z")
              nc.vector.tensor_tensor(out=z[:G, :], in0=zp[:G, :], in1=C["cfb"][:G, :], op=Alu.add)
              nc.vector.tensor_scalar(out=z[:G, :], in0=z[:G, :], scalar1=0.0,
                                      scalar2=None, op0=Alu.max)

              # logits = z @ Wf2 + bf2; out = log_softmax(logits)
              pt2 = ptrans.tile([128, 128], f32, tag="pt")
              nc.tensor.transpose(pt2[:HHID, :G], z[:G, :], C["ident"][:])
              zTs = xts.tile([128, 128], f32, tag="xT")
              nc.vector.tensor_copy(out=zTs[:HHID, :G], in_=pt2[:HHID, :G])
              lp = phw.tile([128, NCLASS], f32, tag="hp")
              nc.tensor.matmul(lp[:G, :], lhsT=zTs[:HHID, :G], rhs=C["wf2"][:],
                               start=True, stop=True)
              lg = tmp.tile([128, NCLASS], f32, tag="lg")
              nc.vector.tensor_tensor(out=lg[:G, :], in0=lp[:G, :], in1=C["bf2b"][:G, :], op=Alu.add)
              mx = tmp.tile([128, 1], f32, tag="mx")
              nc.vector.reduce_max(mx[:G, :], lg[:G, :], axis=mybir.AxisListType.X)
              nc.vector.tensor_scalar(out=lg[:G, :], in0=lg[:G, :], scalar1=mx[:G, :],
                                      scalar2=None, op0=Alu.subtract)
              ex = tmp.tile([128, NCLASS], f32, tag="ex")
              nc.scalar.activation(out=ex[:G, :], in_=lg[:G, :], func=Act.Exp)
              sm = tmp.tile([128, 1], f32, tag="sm")
              nc.vector.reduce_sum(sm[:G, :], ex[:G, :], axis=mybir.AxisListType.X)
              lsm = tmp.tile([128, 1], f32, tag="ls")
              nc.scalar.activation(out=lsm[:G, :], in_=sm[:G, :], func=Act.Ln)
              nc.vector.tensor_scalar(out=lg[:G, :], in0=lg[:G, :], scalar1=lsm[:G, :],
                                      scalar2=None, op0=Alu.subtract)
              nc.sync.dma_start(out=out_d[:], in_=lg[:G, :])

    nc.compile()
    return nc




# ------------------------------------------------------------ PJRT runner --

class SpmdRunner:
    """Run the compiled 8-core Bass module via PJRT (axon), mirroring
    concourse.bass2jax.run_bass_via_pjrt but keeping the jitted callable."""

    def __init__(self, nc, n_cores):
        import jax
        from jax.sharding import Mesh, PartitionSpec
        from jax.experimental.shard_map import shard_map
        from concourse import bass2jax, mybir as _mb
        from concourse.bass2jax import _bass_exec_p, install_neuronx_cc_hook
        install_neuronx_cc_hook()
        self.jax = jax
        self.nc = nc
        self.n_cores = n_cores
        partition_name = nc.partition_id_tensor.name if nc.partition_id_tensor else None
        in_names, out_names, out_avals, zero_outs = [], [], [], []
        for alloc in nc.m.functions[0].allocations:
            if not isinstance(alloc, _mb.MemoryLocationSet):
                continue
            name = alloc.memorylocations[0].name
            if alloc.kind == "ExternalInput":
                if name != partition_name:
                    in_names.append(name)
            elif alloc.kind == "ExternalOutput":
                shape = tuple(alloc.tensor_shape)
                dtype = _mb.dt.np(alloc.dtype)
                out_names.append(name)
                out_avals.append(jax.core.ShapedArray(shape, dtype))
                zero_outs.append(np.zeros(shape, dtype))
        self.in_names, self.out_names = in_names, out_names
        self.out_avals, self.zero_outs = out_avals, zero_outs
        n_params, n_outs = len(in_names), len(out_avals)
        self.n_params = n_params
        all_in_names = in_names + out_names + ([partition_name] if partition_name else [])

        def _body(*args):
            operands = list(args)
            if partition_name is not None:
                operands.append(bass2jax.partition_id_tensor())
            return tuple(_bass_exec_p.bind(
                *operands, out_avals=tuple(out_avals), in_names=tuple(all_in_names),
                out_names=tuple(out_names), lowering_input_output_aliases=(),
                sim_require_finite=True, sim_require_nnan=True, nc=nc))

        devices = jax.devices()[:n_cores]
        assert len(devices) == n_cores
        mesh = Mesh(np.asarray(devices), ("core",))
        self._sharding = jax.sharding.NamedSharding(mesh, PartitionSpec("core"))
        in_specs = (PartitionSpec("core"),) * (n_params + n_outs)
        out_specs = (PartitionSpec("core"),) * len(out_names)
        self._fn = jax.jit(
            shard_map(_body, mesh=mesh, in_specs=in_specs,
                      out_specs=out_specs, check_rep=False),
            keep_unused=True)

    def prepare(self, in_maps):
        per_core = [[np.asarray(m[name]) for name in self.in_names] for m in in_maps]
        concat_in = [np.concatenate([per_core[c][i] for c in range(self.n_cores)], axis=0)
                     for i in range(self.n_params)]
        concat_zeros = [np.zeros((self.n_cores * z.shape[0], *z.shape[1:]), z.dtype)
                        for z in self.zero_outs]
        return concat_in + concat_zeros

    def run(self, in_maps):
        out_arrs = self._fn(*self.prepare(in_maps))
        self.jax.block_until_ready(out_arrs)
        return self._split(out_arrs)

    def _split(self, out_arrs):
        return [{name: np.asarray(out_arrs[i]).reshape(self.n_cores, *self.out_avals[i].shape)[c]
                 for i, name in enumerate(self.out_names)}
                for c in range(self.n_cores)]

    def time(self, in_maps, iters=8):
        import time as _t
        args = self.prepare(in_maps)
        dargs = [self.jax.device_put(a, self._sharding) for a in args]
        out = self._fn(*dargs)
        self.jax.block_until_ready(out)
        results = self._split(out)
        times = []
        for _ in range(iters):
            t0 = _t.perf_counter()
            o = self._fn(*dargs)
            self.jax.block_until_ready(o)
            times.append(_t.perf_counter() - t0)
        return results, times


# ------------------------------------------------------------------- driver --

_CACHE = {}


def _get_runner(st, dims, ncores):
    nc = build_nc(st, dims, ncores)
    return SpmdRunner(nc, ncores)


def kernel(**inputs):
    x = np.asarray(inputs["x"], np.float32)
    edge_index = np.asarray(inputs["edge_index"])
    batch = np.asarray(inputs["batch"])
    edge_attr = np.asarray(inputs["edge_attr"], np.float32)
    G = 128
    params = {k: np.asarray(v) for k, v in inputs.items()
              if k not in ("x", "edge_index", "batch", "edge_attr", "pos")}
    params["cnt_G"] = G
    ncores = 8

    st, in_maps, dims = _host_prep(x, edge_index, batch, edge_attr, params, ncores)

    key = ("k", x.shape, edge_index.shape, st.SID, st.CT, st.GMAX,
           tuple(tuple(s) for s in st.sgs))
    if key not in _CACHE:
        _CACHE[key] = _get_runner(st, dims, ncores)
    runner = _CACHE[key]
    _LAST.update(st=st, dims=dims, ncores=ncores, in_maps=in_maps, runner=runner)
    results = runner.run(in_maps)
    return results[0]["out"]


_LAST = {}


def estimate_exec_ns(reps=16, iters=10):
    """Per-execution device time via wall-clock delta between a 1-rep NEFF and
    an in-NEFF `reps`-times-repeated body (cancels the axon dispatch floor).
    Median-based: the axon tunnel has heavy-tailed per-call jitter."""
    import time as _t
    import jax
    st, dims, ncores = _LAST["st"], _LAST["dims"], _LAST["ncores"]
    in_maps, r1 = _LAST["in_maps"], _LAST["runner"]
    rR = SpmdRunner(build_nc(st, dims, ncores, reps=reps), ncores)
    a1 = [jax.device_put(a, r1._sharding) for a in r1.prepare(in_maps)]
    aR = [jax.device_put(a, rR._sharding) for a in rR.prepare(in_maps)]
    jax.block_until_ready(r1._fn(*a1)); jax.block_until_ready(rR._fn(*aR))
    t1s, tRs = [], []
    for _ in range(iters):
        t0 = _t.perf_counter(); jax.block_until_ready(r1._fn(*a1)); t1s.append(_t.perf_counter() - t0)
        t0 = _t.perf_counter(); jax.block_until_ready(rR._fn(*aR)); tRs.append(_t.perf_counter() - t0)
    t1s, tRs = sorted(t1s), sorted(tRs)
    per = (tRs[len(tRs) // 2] - t1s[len(t1s) // 2]) / (reps - 1)
    return per * 1e9


# revision 12
# speedup vs baseline: 1.0004x; 1.0004x over previous
"""Trainium2 Bass kernel for nn_BaselineGCN (2-layer GCN + BN + mean-pool + MLP head).

Strategy (8 NeuronCores):
 - Nodes sharded contiguously across cores; each core owns the in-edges of its
   node shard (dst-sharding, per the graph-partitioning hint).
 - gcn_norm factorized: deg/dinv computed host-side (index/weight preprocessing);
   per-edge message = w_e * h'[src] with h' = dinv * (h @ W); per-dst scale by
   dinv[dst] folded into the ACT-engine epilogue.
 - The per-edge gather h'[src] runs on-device via SWDGE dma_gather (256B rows)
   from an AllGather'ed replica of h' in each core's DRAM, round-robined over 4
   SWDGE queues.
 - segment_sum becomes TensorE matmuls: per 128-edge chunk, a host-precomputed
   one-hot-times-weight fp8 matrix B[e, dst_local] is loaded as lhsT and PE
   accumulates B.T @ gathered_rows into the dst-block's PSUM tile.  The BN bias
   term (Cb) and the self-loop (+h'[dst]) are injected into the same PSUM
   accumulation via a K=1 outer-product matmul and a bf16 identity matmul, so
   the whole per-block epilogue is ONE ScalarE op: y = Relu(acc * dinv[dst]).
 - VectorE is kept almost empty: the v1 kernel was DVE+GPSIMD co-bottlenecked
   (DVE ~70% busy on epilogue/cast chains created backpressure that limited
   SWDGE descriptor-gen queue concurrency to ~2 of 4).
 - Graph mean-pool is a host-precomputed fp8 one-hot matmul accumulated inline
   with layer-1 epilogues; partials summed with an AllReduce; the tiny MLP
   head + log_softmax run on every core.
"""
import sys
import time

sys.path.insert(0, "/opt/trn_rl_repo")

import numpy as np

P = 128          # partitions / block size
NWIN = 4         # gather index windows (int16 range)
MAXCALL = 1024   # max indices per dma_gather (SWDGE ring capacity)
DMA_SCRATCH = 32768  # SWDGE ring carveout bytes (ring = this // 16 descs)
NQUEUES = 4      # SWDGE queues to round-robin
GBUFS = 16       # gather tile lookahead
BBUFS = 24       # DVE-built per-chunk B tile lookahead
PACCB = 4        # PSUM accumulator banks


# ---------------------------------------------------------------- host prep --

def _ceil(a, b):
    return -(-a // b)


class GCNStructure:
    """Graph partitioning + stream layout. Capacities are maxed across cores so
    the single SPMD program fits every core's data."""

    def __init__(self, src, dst, ew, batch, N, G, ncores):
        self.N, self.G, self.C = N, G, ncores
        NSH = N // ncores
        NB = _ceil(NSH, P)
        WS = _ceil(N // 2, NWIN // 2)  # window size in PAIRED rows
        assert N % ncores == 0
        assert WS <= 32767, "gather window exceeds int16"
        self.NSH, self.NB, self.WS = NSH, NB, WS
        self.LB = NSH - (NB - 1) * P  # rows in last block

        core = dst // NSH
        blk = (dst % NSH) // P
        # table permutation for split AllGathers: half H of every core's shard
        # is gathered into table-half H; row = H*WS + src_core*HWS + local//2
        HWS = NSH // 4  # pair-rows per (core, half)
        self.HWS = HWS
        sc = src // NSH
        sl = src % NSH
        H = sl // (NSH // 2)
        permrow = sc * HWS + (sl % (NSH // 2)) // 2
        # cell = (half, parity): row packs 2 nodes per 256B
        win = H * 2 + (src & 1)
        key = (core * NB + blk) * NWIN + win
        # secondary sort by permuted table row: ascending gather addresses
        # within each (core, block, cell) group improve DRAM read locality
        order = np.lexsort((permrow, key))
        self.permrow_s = permrow[order]
        self.src_s, self.dst_s, self.ew_s = src[order], dst[order], ew[order]
        counts = np.bincount(key, minlength=ncores * NB * NWIN).reshape(ncores, NB, NWIN)
        self.counts = counts
        cap = counts.max(axis=0)
        cap = _ceil(np.maximum(cap, 0), P) * P  # per (b, w), 0 stays 0
        self.cap = cap  # [NB, NWIN]

        # supergroups: consecutive blocks such that per-window call <= MAXCALL
        self.sgs = []
        cur = [0]
        for b in range(1, NB):
            trial = cur + [b]
            if all(cap[trial, w].sum() <= MAXCALL for w in range(NWIN)):
                cur = trial
            else:
                self.sgs.append(cur)
                cur = [b]
        self.sgs.append(cur)

        # layout: gather calls in (sg, w) order; chunks in (sg, b, w, j) order
        self.gcols = {}    # (sgi, w) -> columns in that call's tile
        self.icol = {}     # (sgi, w) -> start col (units of 16-idx) in idx stream
        self.coloff = {}   # (b, w) -> column offset inside its call tile
        sid = 0
        for sgi, sg in enumerate(self.sgs):
            for w in range(NWIN):
                cols = int(cap[sg, w].sum()) // P
                self.gcols[(sgi, w)] = cols
                self.icol[(sgi, w)] = sid
                off = 0
                for b in sg:
                    self.coloff[(b, w)] = off
                    off += int(cap[b, w]) // P
                sid += cols * 8  # n/16 = cols*128/16
        self.SID = max(sid, 8)
        self.CT = max(int(cap.sum()) // P, 1)
        self.GMAX = max(max(self.gcols.values(), default=1), 1)
        self.SGMAX = max(sum(int(cap[b].sum()) // P for b in sg) for sg in self.sgs)

        # per-core edge offsets into the sorted arrays, per (b, w)
        cum = np.zeros(ncores * NB * NWIN + 1, np.int64)
        np.cumsum(counts.reshape(-1), out=cum[1:])
        self.grp_start = cum  # index by (c*NB+b)*NWIN+w

        # batch / counts for pooling
        self.cnt = np.bincount(batch, minlength=G).astype(np.float32)
        self.inv_cnt = (1.0 / np.maximum(self.cnt, 1.0)).astype(np.float32)

    def core_streams(self, c, dinv):
        """Build per-core device streams: idx [128, SID] i16,
        dstloc/val [128, CT] f32, dinvb [128, NB] f32."""
        NB, WS, NSH = self.NB, self.WS, self.NSH
        idx_cols = np.zeros((128, self.SID), np.int16)
        dstloc = np.zeros((128, self.CT), np.float32)
        val = np.zeros((128, self.CT), np.float32)

        t = 0
        for sgi, sg in enumerate(self.sgs):
            # gather stream: (w, b) order
            for w in range(NWIN):
                col = self.icol[(sgi, w)]
                parts = []
                for b in sg:
                    g0 = self.grp_start[(c * NB + b) * NWIN + w]
                    g1 = self.grp_start[(c * NB + b) * NWIN + w + 1]
                    loc = self.permrow_s[g0:g1].astype(np.int16)
                    pad = int(self.cap[b, w]) - (g1 - g0)
                    parts.append(np.concatenate([loc, np.zeros(pad, np.int16)]))
                if parts:
                    flat = np.concatenate(parts)
                    if flat.size:
                        wrapped = np.tile(flat.reshape(-1, 16).T, (8, 1))
                        idx_cols[:, col:col + flat.size // 16] = wrapped
            # value/dst streams: (b, w, chunk) order
            for b in sg:
                for w in range(NWIN):
                    g0 = self.grp_start[(c * NB + b) * NWIN + w]
                    g1 = self.grp_start[(c * NB + b) * NWIN + w + 1]
                    n = g1 - g0
                    capbw = int(self.cap[b, w])
                    if capbw == 0:
                        continue
                    dl = np.zeros(capbw, np.float32)
                    vv = np.zeros(capbw, np.float32)
                    dl[:n] = (self.dst_s[g0:g1] - (c * NSH + b * P)).astype(np.float32)
                    vv[:n] = self.ew_s[g0:g1]
                    k = capbw // P
                    dstloc[:, t:t + k] = dl.reshape(k, P).T
                    val[:, t:t + k] = vv.reshape(k, P).T
                    t += k

        dinvb = np.zeros((128, NB), np.float32)
        sh_dinv = dinv[c * NSH:(c + 1) * NSH]
        for b in range(NB):
            nb = P if b < NB - 1 else self.LB
            dinvb[:nb, b] = sh_dinv[b * P:b * P + nb]
        return idx_cols, dstloc, val, dinvb

    def core_pool_oh(self, c, batch):
        """fp8 one-hot pooling matrix [128, NB*128]: [p, b*128+g] = (batch==g)."""
        import ml_dtypes
        FP8 = np.dtype(ml_dtypes.float8_e4m3)
        NB, NSH = self.NB, self.NSH
        oh = np.zeros((128, NB * 128), FP8)
        sh = batch[c * NSH:(c + 1) * NSH]
        for b in range(NB):
            nb = P if b < NB - 1 else self.LB
            g = sh[b * P:b * P + nb].astype(np.int64)
            oh[np.arange(nb), b * 128 + g] = 1.0
        return oh


def _host_prep(x, edge_index, batch, edge_attr, params, ncores):
    """All index-based preprocessing + BN folding. Returns (struct, in_maps)."""
    N, INDIM = x.shape
    G = int(params["cnt_G"])
    EPS = 1e-5

    src = np.asarray(edge_index[0], np.int64)
    dst = np.asarray(edge_index[1], np.int64)
    ew = np.asarray(edge_attr, np.float32)
    batch = np.asarray(batch, np.int64)

    deg = np.bincount(dst, weights=ew.astype(np.float64), minlength=N) + 1.0
    dinv = (1.0 / np.sqrt(deg)).astype(np.float32)

    st = GCNStructure(src, dst, ew, batch, N, G, ncores)

    def bnfold(g, be, m, v, bias):
        s = (g / np.sqrt(v + EPS)).astype(np.float32)
        cc = ((bias - m) * s + be).astype(np.float32)
        return s, cc

    S0, C0 = bnfold(params["g0"], params["be0"], params["m0"], params["v0"], params["b0"])
    S1, C1 = bnfold(params["g1"], params["be1"], params["m1"], params["v1"], params["b1"])
    Sf, Cf = bnfold(params["gf"], params["bef"], params["mf"], params["vf"], params["bf1"])

    # BN scale folded into the weight matrices (linear before the bias add)
    W0s = (np.asarray(params["W0"], np.float32) * S0[None, :]).astype(np.float32)
    W1s = (np.asarray(params["W1"], np.float32) * S1[None, :]).astype(np.float32)
    Wf1s = (np.asarray(params["Wf1"], np.float32) * Sf[None, :]).astype(np.float32)

    HID = params["W0"].shape[1]
    HHID = params["Wf1"].shape[1]
    NCLASS = params["Wf2"].shape[1]

    ident = np.eye(128, dtype=np.float32)
    iota = np.tile(np.arange(128, dtype=np.float32)[None, :], (128, 1))

    NSH, NB = st.NSH, st.NB

    import ml_dtypes
    FP8 = np.dtype(ml_dtypes.float8_e4m3)
    BF16 = np.dtype(ml_dtypes.bfloat16)

    xpadT = np.zeros((ncores, INDIM, NB * P), BF16)
    xv = np.asarray(x, np.float32)
    for c in range(ncores):
        xpadT[c, :, :NSH] = xv[c * NSH:(c + 1) * NSH].T.astype(BF16)

    in_maps = []
    for c in range(ncores):
        idx_cols, dstloc, val, dinvb = st.core_streams(c, dinv)
        pool_oh = st.core_pool_oh(c, batch)
        # invd_flat[0, b*128+j] = 1/dinv at node (c, b, j); 0 for pad rows.
        # (single partition: Ldweights requires lhsT at partition 0)
        invd_flat = np.zeros((1, NB * 128), np.float32)
        sh_dinv = dinv[c * NSH:(c + 1) * NSH]
        for b in range(NB):
            nb = P if b < NB - 1 else st.LB
            invd_flat[0, b * 128:b * 128 + nb] = 1.0 / sh_dinv[b * P:b * P + nb]
        # cRep[b, :] = C (bias row replicated so lhsT/rhs base partitions match)
        cRep0 = np.tile(C0[None, :], (128, 1)).astype(BF16)
        cRep1 = np.tile(C1[None, :], (128, 1)).astype(BF16)
        in_maps.append(dict(
            xshT=xpadT[c],
            idxs=idx_cols,
            dstloc=dstloc, bval=val,
            pool_oh=pool_oh, dinvb=dinvb,
            invd_flat=invd_flat.astype(BF16),
            cRep0=cRep0, cRep1=cRep1,
            w0=W0s.astype(BF16), w1=W1s, wf1=Wf1s,
            wf2=np.asarray(params["Wf2"], np.float32),
            cfb=np.tile(Cf[None, :], (128, 1)),
            bf2b=np.tile(np.asarray(params["bf2"], np.float32)[None, :], (128, 1)),
            invcnt=st.inv_cnt[:, None].copy(),
            ident=ident, identb=ident.astype(BF16), iota=iota.astype(BF16),
        ))
    dims = dict(INDIM=INDIM, HID=HID, HHID=HHID, NCLASS=NCLASS)
    return st, in_maps, dims


# ------------------------------------------------------------- bass program --

def build_nc(st, dims, ncores, reps=1, fake_coll=False, no_gather=False, no_compute=False):
    from concourse import bass, mybir, bacc, tile

    INDIM, HID, HHID, NCLASS = dims["INDIM"], dims["HID"], dims["HHID"], dims["NCLASS"]
    N, G, NB, NSH, WS, LB = st.N, st.G, st.NB, st.NSH, st.WS, st.LB
    f32 = mybir.dt.float32
    bf16 = mybir.dt.bfloat16
    fp8 = mybir.dt.float8e4
    Alu = mybir.AluOpType
    Act = mybir.ActivationFunctionType

    nc = bacc.Bacc("TRN2", target_bir_lowering=False, debug=False,
                   enable_asserts=True, num_devices=ncores,
                   num_swdge_queues=NQUEUES,
                   dynamic_dma_scratch_size=DMA_SCRATCH)

    I = {}
    def inp(name, shape, dt=f32):
        I[name] = nc.dram_tensor(name, shape, dt, kind="ExternalInput")
        return I[name]

    inp("xshT", [INDIM, NB * P], bf16)
    inp("idxs", [128, st.SID], mybir.dt.int16)
    inp("dstloc", [128, st.CT]); inp("bval", [128, st.CT])
    inp("pool_oh", [128, NB * 128], mybir.dt.float8e4)
    inp("dinvb", [128, NB])
    inp("invd_flat", [1, NB * 128], bf16)
    inp("cRep0", [128, HID], bf16); inp("cRep1", [128, HID], bf16)
    inp("w0", [INDIM, HID], bf16); inp("w1", [HID, HID])
    inp("wf1", [HID, HHID]); inp("wf2", [HHID, NCLASS])
    inp("cfb", [128, HHID])
    inp("bf2b", [128, NCLASS])
    inp("invcnt", [128, 1])
    inp("ident", [128, 128]); inp("identb", [128, 128], bf16)
    inp("iota", [128, 128], bf16)
    out_d = nc.dram_tensor("out", [G, NCLASS], f32, kind="ExternalOutput")

    qctr = [0]
    def next_q():
        q = qctr[0] % NQUEUES
        qctr[0] += 1
        return q

    with tile.TileContext(nc) as tc:
        import contextlib
        with contextlib.ExitStack() as ctx:
            const = ctx.enter_context(tc.tile_pool(name="const", bufs=1))
            stream = ctx.enter_context(tc.tile_pool(name="stream", bufs=1))
            xio = ctx.enter_context(tc.tile_pool(name="xio", bufs=3))
            xts = ctx.enter_context(tc.tile_pool(name="xts", bufs=3))
            hpool = ctx.enter_context(tc.tile_pool(name="hpool", bufs=NB))
            ypool = ctx.enter_context(tc.tile_pool(name="ypool", bufs=NB))
            y1pool = ctx.enter_context(tc.tile_pool(name="y1pool", bufs=NB))
            gpool = ctx.enter_context(tc.tile_pool(name="gpool", bufs=GBUFS))
            bpool = ctx.enter_context(tc.tile_pool(name="bpool", bufs=BBUFS))
            tmp = ctx.enter_context(tc.tile_pool(name="tmp", bufs=6))
            ptrans = ctx.enter_context(tc.tile_pool(name="ptrans", bufs=1, space="PSUM"))
            phw = ctx.enter_context(tc.tile_pool(name="phw", bufs=2, space="PSUM"))
            pacc = ctx.enter_context(tc.tile_pool(name="pacc", bufs=PACCB, space="PSUM"))
            ppool = ctx.enter_context(tc.tile_pool(name="ppool", bufs=1, space="PSUM"))
            dram = ctx.enter_context(tc.tile_pool(name="dram", bufs=1, space="DRAM"))

            # ---- constants into SBUF
            C = {}
            cdts = dict(w0=bf16, invd_flat=bf16, cRep0=bf16, cRep1=bf16, identb=bf16,
                        iota=bf16, pool_oh=fp8)
            for nm in ["w0", "w1", "wf1", "wf2", "cfb", "bf2b", "invcnt",
                       "ident", "identb", "iota", "invd_flat", "cRep0", "cRep1",
                       "pool_oh"]:
                shape = list(I[nm].shape)
                tile_ = const.tile(shape, cdts.get(nm, f32), tag=nm)
                nc.sync.dma_start(out=tile_[:], in_=I[nm][:])
                C[nm] = tile_
            idx_t = stream.tile([128, st.SID], mybir.dt.int16, tag="idx")
            nc.sync.dma_start(out=idx_t[:], in_=I["idxs"][:])
            dstloc_t = stream.tile([128, st.CT], f32, tag="dstloc")
            nc.sync.dma_start(out=dstloc_t[:], in_=I["dstloc"][:])
            bval_t = stream.tile([128, st.CT], f32, tag="bval")
            nc.sync.dma_start(out=bval_t[:], in_=I["bval"][:])
            dinv_t = stream.tile([128, NB], f32, tag="dnv")
            nc.sync.dma_start(out=dinv_t[:], in_=I["dinvb"][:])

            shspace = "Shared" if ncores > 4 else "Local"
            HSH = NSH // 2  # nodes per half-shard
            bounce = [[dram.tile([HSH, HID], bf16, tag=f"bnc{l}{h}",
                                 name=f"bounce{l}{h}") for h in range(2)]
                      for l in range(2)]
            ar_in = dram.tile([G, HID], f32, tag="arin")

            # ---- phase A: h0' = dinv * (x @ W0), shard -> AllGather table0
            for _rep in range(reps):
              # per-rep Shared tables: a Shared DRAM tile allows only one writer
              # split per half for split AllGathers (overlap with compute)
              table = [[dram.tile([WS, 2 * HID], bf16, tag=f"tab{l}{h}_{_rep}",
                                  name=f"table{l}{h}_{_rep}",
                                  addr_space=shspace) for h in range(2)]
                       for l in range(2)]
              ar_out = dram.tile([G, HID], f32, tag=f"arout_{_rep}", addr_space=shspace)

              def bounce_write(l, b, nb, tile_):
                  # write rows [b*P, b*P+nb) of the shard into half bounces
                  r0, r1 = b * P, b * P + nb
                  if r0 < HSH:
                      e = min(r1, HSH)
                      nc.sync.dma_start(out=bounce[l][0][r0:e, :], in_=tile_[:e - r0, :])
                  if r1 > HSH:
                      s = max(r0, HSH)
                      nc.sync.dma_start(out=bounce[l][1][s - HSH:r1 - HSH, :],
                                        in_=tile_[s - r0:nb, :])

              def allgather_half(l, h):
                  if fake_coll:
                      nc.sync.dma_start(out=table[l][h][0:HSH // 2, :],
                                        in_=bounce[l][h][:])
                  else:
                      nc.gpsimd.collective_compute(
                          "AllGather", Alu.bypass,
                          replica_groups=[list(range(ncores))],
                          ins=[bounce[l][h].opt()], outs=[table[l][h].opt()],
                      )

              # last block fully inside half 0: fire AG half 0 after it
              bsplit = (HSH - 1) // P  # block containing row HSH-1

              h_tiles = []
              for b in range(NB):
                  nb = P if b < NB - 1 else LB
                  xt = xio.tile([128, 128], bf16, tag="xt")
                  nc.sync.dma_start(out=xt[:INDIM, :], in_=I["xshT"][:, b * P:(b + 1) * P])
                  hp = phw.tile([128, HID], f32, tag="hp")
                  nc.tensor.matmul(hp[:], lhsT=xt[:INDIM, :], rhs=C["w0"][:],
                                   start=True, stop=True)
                  hb16 = hpool.tile([128, HID], bf16, tag="h")
                  nc.scalar.activation(out=hb16[:], in_=hp[:], func=Act.Copy,
                                       scale=dinv_t[:, b:b + 1])
                  bounce_write(0, b, nb, hb16)
                  h_tiles.append(hb16)
                  if b == bsplit:
                      allgather_half(0, 0)
              allgather_half(0, 1)

              # ---- GCN layers
              pp = ppool.tile([128, HID], f32, tag="pool")
              for l in range(2):
                  cRep = C["cRep0"] if l == 0 else C["cRep1"]
                  h1_tiles = []
                  t = 0
                  for sgi, sg in enumerate(st.sgs):
                      gt = {}
                      for w in range(NWIN):
                          cols = st.gcols[(sgi, w)]
                          if cols == 0:
                              continue
                          gbf = gpool.tile([128, st.GMAX, 2 * HID], bf16, tag="g")
                          ic = st.icol[(sgi, w)]
                          gt[w] = gbf
                          if no_gather:
                              continue
                          nc.gpsimd.dma_gather(
                              out_ap=gbf[:, :cols, :],
                              in_ap=table[l][w // 2][:, :],
                              idxs_ap=idx_t[:, ic:ic + cols * 8],
                              num_idxs=cols * P,
                              num_idxs_reg=cols * P,
                              elem_size=2 * HID,
                              queue_num=next_q(),
                          )
                      for b in sg:
                          nchunks = 0 if no_compute else int(st.cap[b].sum()) // P
                          acc = pacc.tile([128, HID], f32, tag="acc")
                          # inject Cb/dinv[dst] (outer product, K=1), then the
                          # self-loop h'[dst] (identity matmul), then the edges
                          nc.tensor.matmul(acc[:],
                                           lhsT=C["invd_flat"][0:1, b * 128:(b + 1) * 128],
                                           rhs=cRep[0:1, :],
                                           start=True, stop=False)
                          nc.tensor.matmul(acc[:], lhsT=C["identb"][:],
                                           rhs=h_tiles[b][:],
                                           start=False, stop=(nchunks == 0))
                          done = 0
                          for w in range(NWIN if not no_compute else 0):
                              kk = int(st.cap[b, w]) // P
                              for j in range(kk):
                                  Bt = bpool.tile([128, 128], bf16, tag="B")
                                  nc.vector.tensor_scalar(
                                      out=Bt[:], in0=C["iota"][:],
                                      scalar1=dstloc_t[:, t:t + 1],
                                      scalar2=bval_t[:, t:t + 1],
                                      op0=Alu.is_equal, op1=Alu.mult)
                                  nc.tensor.matmul(
                                      acc[:], lhsT=Bt[:],
                                      rhs=gt[w][:, st.coloff[(b, w)] + j,
                                                (w % 2) * HID:(w % 2 + 1) * HID],
                                      start=False, stop=(done == nchunks - 1))
                                  done += 1
                                  t += 1
                          nb = P if b < NB - 1 else LB
                          if l == 0:
                              # y0 (f32) -> transpose -> h1' = dinv*(y0 @ W1)
                              yb = ypool.tile([128, HID], f32, tag="y")
                              nc.scalar.activation(out=yb[:], in_=acc[:], func=Act.Relu,
                                                   scale=dinv_t[:, b:b + 1])
                              pt = ptrans.tile([128, 128], f32, tag="pt")
                              nc.tensor.transpose(pt[:HID, :], yb[:], C["ident"][:])
                              yTs = xts.tile([128, 128], f32, tag="xT")
                              nc.scalar.activation(out=yTs[:HID, :], in_=pt[:HID, :],
                                                   func=Act.Copy)
                              hp = phw.tile([128, HID], f32, tag="hp")
                              nc.tensor.matmul(hp[:], lhsT=yTs[:HID, :], rhs=C["w1"][:],
                                               start=True, stop=True)
                              h1b = hpool.tile([128, HID], bf16, tag="h")
                              nc.scalar.activation(out=h1b[:], in_=hp[:], func=Act.Copy,
                                                   scale=dinv_t[:, b:b + 1])
                              bounce_write(1, b, nb, h1b)
                              h1_tiles.append(h1b)
                          else:
                              # y1 (bf16) -> inline mean-pool accumulation
                              yb = y1pool.tile([128, HID], bf16, tag="y1")
                              nc.scalar.activation(out=yb[:], in_=acc[:], func=Act.Relu,
                                                   scale=dinv_t[:, b:b + 1])
                              nc.tensor.matmul(pp[:G, :],
                                               lhsT=C["pool_oh"][:nb, b * 128:b * 128 + G],
                                               rhs=yb[:nb, :],
                                               start=(b == 0), stop=(b == NB - 1))
                  if l == 0:
                      h_tiles = h1_tiles
                      allgather_half(1, 0)
                      allgather_half(1, 1)

              # ---- mean pool partials -> AllReduce -> head
              pooled = tmp.tile([128, HID], f32, tag="pl")
              nc.scalar.activation(out=pooled[:G, :], in_=pp[:G, :], func=Act.Copy)
              nc.sync.dma_start(out=ar_in[:], in_=pooled[:G, :])
              if fake_coll:
                  nc.sync.dma_start(out=ar_out[:], in_=ar_in[:])
              else:
                  nc.gpsimd.collective_compute(
                      "AllReduce", Alu.add,
                      replica_groups=[list(range(ncores))],
                      ins=[ar_in.opt()], outs=[ar_out.opt()],
                  )
              pooled2 = tmp.tile([128, HID], f32, tag="pl2")
              nc.sync.dma_start(out=pooled2[:G, :], in_=ar_out[:])
              nc.vector.tensor_scalar(out=pooled2[:G, :], in0=pooled2[:G, :],
                                      scalar1=C["invcnt"][:G, :], scalar2=None,
                                      op0=Alu.mult)

              # z = relu(Sf * (pooled @ Wf1) + Cf)
              pt = ptrans.tile([128, 128], f32, tag="pt")
              nc.tensor.transpose(pt[:HID, :G], pooled2[:G, :], C["ident"][:])
              pTs = xts.tile([128, 128], f32, tag="xT")
              nc.vector.tensor_copy(out=pTs[:HID, :G], in_=pt[:HID, :G])
              zp = phw.tile([128, HHID], f32, tag="hp")
              nc.tensor.matmul(zp[:G, :], lhsT=pTs[:HID, :G], rhs=C["wf1"][:],
                               start=True, stop=True)
              z = tmp.tile([128, HHID], f32, tag="z")
              nc.vector.tensor_tensor(out=z[:G, :], in0=zp[:G, :], in1=C["cfb"][:G, :], op=Alu.add)
              nc.vector.tensor_scalar(out=z[:G, :], in0=z[:G, :], scalar1=0.0,
                                      scalar2=None, op0=Alu.max)

              # logits = z @ Wf2 + bf2; out = log_softmax(logits)
              pt2 = ptrans.tile([128, 128], f32, tag="pt")
              nc.tensor.transpose(pt2[:HHID, :G], z[:G, :], C["ident"][:])
              zTs = xts.tile([128, 128], f32, tag="xT")
              nc.vector.tensor_copy(out=zTs[:HHID, :G], in_=pt2[:HHID, :G])
              lp = phw.tile([128, NCLASS], f32, tag="hp")
              nc.tensor.matmul(lp[:G, :], lhsT=zTs[:HHID, :G], rhs=C["wf2"][:],
                               start=True, stop=True)
              lg = tmp.tile([128, NCLASS], f32, tag="lg")
              nc.vector.tensor_tensor(out=lg[:G, :], in0=lp[:G, :], in1=C["bf2b"][:G, :], op=Alu.add)
              mx = tmp.tile([128, 1], f32, tag="mx")
              nc.vector.reduce_max(mx[:G, :], lg[:G, :], axis=mybir.AxisListType.X)
              nc.vector.tensor_scalar(out=lg[:G, :], in0=lg[:G, :], scalar1=mx[:G, :],
                                      scalar2=None, op0=Alu.subtract)
              ex = tmp.tile([128, NCLASS], f32, tag="ex")
              nc.scalar.activation(out=ex[:G, :], in_=lg[:G, :], func=Act.Exp)
              sm = tmp.tile([128, 1], f32, tag="sm")
              nc.vector.reduce_sum(sm[:G, :], ex[:G, :], axis=mybir.AxisListType.X)
              lsm = tmp.tile([128, 1], f32, tag="ls")
              nc.scalar.activation(out=lsm[:G, :], in_=sm[:G, :], func=Act.Ln)
              nc.vector.tensor_scalar(out=lg[:G, :], in0=lg[:G, :], scalar1=lsm[:G, :],
                                      scalar2=None, op0=Alu.subtract)
              nc.sync.dma_start(out=out_d[:], in_=lg[:G, :])

    nc.compile()
    return nc




# ------------------------------------------------------------ PJRT runner --

class SpmdRunner:
    """Run the compiled 8-core Bass module via PJRT (axon), mirroring
    concourse.bass2jax.run_bass_via_pjrt but keeping the jitted callable."""

    def __init__(self, nc, n_cores):
        import jax
        from jax.sharding import Mesh, PartitionSpec
        from jax.experimental.shard_map import shard_map
        from concourse import bass2jax, mybir as _mb
        from concourse.bass2jax import _bass_exec_p, install_neuronx_cc_hook
        install_neuronx_cc_hook()
        self.jax = jax
        self.nc = nc
        self.n_cores = n_cores
        partition_name = nc.partition_id_tensor.name if nc.partition_id_tensor else None
        in_names, out_names, out_avals, zero_outs = [], [], [], []
        for alloc in nc.m.functions[0].allocations:
            if not isinstance(alloc, _mb.MemoryLocationSet):
                continue
            name = alloc.memorylocations[0].name
            if alloc.kind == "ExternalInput":
                if name != partition_name:
                    in_names.append(name)
            elif alloc.kind == "ExternalOutput":
                shape = tuple(alloc.tensor_shape)
                dtype = _mb.dt.np(alloc.dtype)
                out_names.append(name)
                out_avals.append(jax.core.ShapedArray(shape, dtype))
                zero_outs.append(np.zeros(shape, dtype))
        self.in_names, self.out_names = in_names, out_names
        self.out_avals, self.zero_outs = out_avals, zero_outs
        n_params, n_outs = len(in_names), len(out_avals)
        self.n_params = n_params
        all_in_names = in_names + out_names + ([partition_name] if partition_name else [])

        def _body(*args):
            operands = list(args)
            if partition_name is not None:
                operands.append(bass2jax.partition_id_tensor())
            return tuple(_bass_exec_p.bind(
                *operands, out_avals=tuple(out_avals), in_names=tuple(all_in_names),
                out_names=tuple(out_names), lowering_input_output_aliases=(),
                sim_require_finite=True, sim_require_nnan=True, nc=nc))

        devices = jax.devices()[:n_cores]
        assert len(devices) == n_cores
        mesh = Mesh(np.asarray(devices), ("core",))
        self._sharding = jax.sharding.NamedSharding(mesh, PartitionSpec("core"))
        in_specs = (PartitionSpec("core"),) * (n_params + n_outs)
        out_specs = (PartitionSpec("core"),) * len(out_names)
        self._fn = jax.jit(
            shard_map(_body, mesh=mesh, in_specs=in_specs,
                      out_specs=out_specs, check_rep=False),
            keep_unused=True)

    def prepare(self, in_maps):
        per_core = [[np.asarray(m[name]) for name in self.in_names] for m in in_maps]
        concat_in = [np.concatenate([per_core[c][i] for c in range(self.n_cores)], axis=0)
                     for i in range(self.n_params)]
        concat_zeros = [np.zeros((self.n_cores * z.shape[0], *z.shape[1:]), z.dtype)
                        for z in self.zero_outs]
        return concat_in + concat_zeros

    def run(self, in_maps):
        out_arrs = self._fn(*self.prepare(in_maps))
        self.jax.block_until_ready(out_arrs)
        return self._split(out_arrs)

    def _split(self, out_arrs):
        return [{name: np.asarray(out_arrs[i]).reshape(self.n_cores, *self.out_avals[i].shape)[c]
                 for i, name in enumerate(self.out_names)}
                for c in range(self.n_cores)]

    def time(self, in_maps, iters=8):
        import time as _t
        args = self.prepare(in_maps)
        dargs = [self.jax.device_put(a, self._sharding) for a in args]
        out = self._fn(*dargs)
        self.jax.block_until_ready(out)
        results = self._split(out)
        times = []
        for _ in range(iters):
            t0 = _t.perf_counter()
            o = self._fn(*dargs)
            self.jax.block_until_ready(o)
            times.append(_t.perf_counter() - t0)
        return results, times


# ------------------------------------------------------------------- driver --

_CACHE = {}


def _get_runner(st, dims, ncores):
    nc = build_nc(st, dims, ncores)
    return SpmdRunner(nc, ncores)


def kernel(**inputs):
    x = np.asarray(inputs["x"], np.float32)
    edge_index = np.asarray(inputs["edge_index"])
    batch = np.asarray(inputs["batch"])
    edge_attr = np.asarray(inputs["edge_attr"], np.float32)
    G = 128
    params = {k: np.asarray(v) for k, v in inputs.items()
              if k not in ("x", "edge_index", "batch", "edge_attr", "pos")}
    params["cnt_G"] = G
    ncores = 8

    st, in_maps, dims = _host_prep(x, edge_index, batch, edge_attr, params, ncores)

    key = ("k", x.shape, edge_index.shape, st.SID, st.CT, st.GMAX,
           tuple(tuple(s) for s in st.sgs))
    if key not in _CACHE:
        _CACHE[key] = _get_runner(st, dims, ncores)
    runner = _CACHE[key]
    _LAST.update(st=st, dims=dims, ncores=ncores, in_maps=in_maps, runner=runner)
    results = runner.run(in_maps)
    return results[0]["out"]


_LAST = {}


def estimate_exec_ns(reps=16, iters=10):
    """Per-execution device time via wall-clock delta between a 1-rep NEFF and
    an in-NEFF `reps`-times-repeated body (cancels the axon dispatch floor).
    Median-based: the axon tunnel has heavy-tailed per-call jitter."""
    import time as _t
    import jax
    st, dims, ncores = _LAST["st"], _LAST["dims"], _LAST["ncores"]
    in_maps, r1 = _LAST["in_maps"], _LAST["runner"]
    rR = SpmdRunner(build_nc(st, dims, ncores, reps=reps), ncores)
    a1 = [jax.device_put(a, r1._sharding) for a in r1.prepare(in_maps)]
    aR = [jax.device_put(a, rR._sharding) for a in rR.prepare(in_maps)]
    jax.block_until_ready(r1._fn(*a1)); jax.block_until_ready(rR._fn(*aR))
    t1s, tRs = [], []
    for _ in range(iters):
        t0 = _t.perf_counter(); jax.block_until_ready(r1._fn(*a1)); t1s.append(_t.perf_counter() - t0)
        t0 = _t.perf_counter(); jax.block_until_ready(rR._fn(*aR)); tRs.append(_t.perf_counter() - t0)
    t1s, tRs = sorted(t1s), sorted(tRs)
    per = (tRs[len(tRs) // 2] - t1s[len(t1s) // 2]) / (reps - 1)
    return per * 1e9


# revision 13
# speedup vs baseline: 1.3989x; 1.3984x over previous
"""Trainium2 Bass kernel for nn_BaselineGCN (2-layer GCN + BN + mean-pool + MLP head).

Strategy (8 NeuronCores):
 - Nodes sharded contiguously across cores; each core owns the in-edges of its
   node shard (dst-sharding, per the graph-partitioning hint).
 - gcn_norm factorized: deg/dinv computed host-side (index/weight preprocessing);
   per-edge message = w_e * h'[src] with h' = dinv * (h @ W); per-dst scale by
   dinv[dst] folded into the ACT-engine epilogue.
 - The per-edge gather h'[src] runs on-device via SWDGE dma_gather (256B rows)
   from an AllGather'ed replica of h' in each core's DRAM, round-robined over 4
   SWDGE queues.
 - segment_sum becomes TensorE matmuls: per 128-edge chunk, a host-precomputed
   one-hot-times-weight fp8 matrix B[e, dst_local] is loaded as lhsT and PE
   accumulates B.T @ gathered_rows into the dst-block's PSUM tile.  The BN bias
   term (Cb) and the self-loop (+h'[dst]) are injected into the same PSUM
   accumulation via a K=1 outer-product matmul and a bf16 identity matmul, so
   the whole per-block epilogue is ONE ScalarE op: y = Relu(acc * dinv[dst]).
 - VectorE is kept almost empty: the v1 kernel was DVE+GPSIMD co-bottlenecked
   (DVE ~70% busy on epilogue/cast chains created backpressure that limited
   SWDGE descriptor-gen queue concurrency to ~2 of 4).
 - Graph mean-pool is a host-precomputed fp8 one-hot matmul accumulated inline
   with layer-1 epilogues; partials summed with an AllReduce; the tiny MLP
   head + log_softmax run on every core.
"""
import sys
import time

sys.path.insert(0, "/opt/trn_rl_repo")

import numpy as np

P = 128          # partitions / block size
NWIN = 4         # gather index windows (int16 range)
MAXCALL = 1024   # max indices per dma_gather (SWDGE ring capacity)
DMA_SCRATCH = 32768  # SWDGE ring carveout bytes (ring = this // 16 descs)
NQUEUES = 4      # SWDGE queues to round-robin
GBUFS = 16       # gather tile lookahead
BBUFS = 4        # B-slab lookahead (per-supergroup fp8 slabs)
PACCB = 4        # PSUM accumulator banks


# ---------------------------------------------------------------- host prep --

def _ceil(a, b):
    return -(-a // b)


class GCNStructure:
    """Graph partitioning + stream layout. Capacities are maxed across cores so
    the single SPMD program fits every core's data."""

    def __init__(self, src, dst, ew, batch, N, G, ncores):
        self.N, self.G, self.C = N, G, ncores
        NSH = N // ncores
        NB = _ceil(NSH, P)
        WS = _ceil(N // 2, NWIN // 2)  # window size in PAIRED rows
        assert N % ncores == 0
        assert WS <= 32767, "gather window exceeds int16"
        self.NSH, self.NB, self.WS = NSH, NB, WS
        self.LB = NSH - (NB - 1) * P  # rows in last block

        core = dst // NSH
        blk = (dst % NSH) // P
        # table permutation for split AllGathers: half H of every core's shard
        # is gathered into table-half H; row = H*WS + src_core*HWS + local//2
        HWS = NSH // 4  # pair-rows per (core, half)
        self.HWS = HWS
        sc = src // NSH
        sl = src % NSH
        H = sl // (NSH // 2)
        permrow = sc * HWS + (sl % (NSH // 2)) // 2
        # cell = (half, parity): row packs 2 nodes per 256B
        win = H * 2 + (src & 1)
        key = (core * NB + blk) * NWIN + win
        # secondary sort by permuted table row: ascending gather addresses
        # within each (core, block, cell) group improve DRAM read locality
        order = np.lexsort((permrow, key))
        self.permrow_s = permrow[order]
        self.src_s, self.dst_s, self.ew_s = src[order], dst[order], ew[order]
        counts = np.bincount(key, minlength=ncores * NB * NWIN).reshape(ncores, NB, NWIN)
        self.counts = counts
        cap = counts.max(axis=0)
        cap = _ceil(np.maximum(cap, 0), P) * P  # per (b, w), 0 stays 0
        self.cap = cap  # [NB, NWIN]

        # supergroups: consecutive blocks such that per-window call <= MAXCALL
        self.sgs = []
        cur = [0]
        for b in range(1, NB):
            trial = cur + [b]
            if all(cap[trial, w].sum() <= MAXCALL for w in range(NWIN)):
                cur = trial
            else:
                self.sgs.append(cur)
                cur = [b]
        self.sgs.append(cur)

        # layout: gather calls in (sg, w) order; chunks in (sg, b, w, j) order
        self.gcols = {}    # (sgi, w) -> columns in that call's tile
        self.icol = {}     # (sgi, w) -> start col (units of 16-idx) in idx stream
        self.coloff = {}   # (b, w) -> column offset inside its call tile
        sid = 0
        for sgi, sg in enumerate(self.sgs):
            for w in range(NWIN):
                cols = int(cap[sg, w].sum()) // P
                self.gcols[(sgi, w)] = cols
                self.icol[(sgi, w)] = sid
                off = 0
                for b in sg:
                    self.coloff[(b, w)] = off
                    off += int(cap[b, w]) // P
                sid += cols * 8  # n/16 = cols*128/16
        self.SID = max(sid, 8)
        self.CT = max(int(cap.sum()) // P, 1)
        self.GMAX = max(max(self.gcols.values(), default=1), 1)
        self.SGMAX = max(sum(int(cap[b].sum()) // P for b in sg) for sg in self.sgs)

        # per-core edge offsets into the sorted arrays, per (b, w)
        cum = np.zeros(ncores * NB * NWIN + 1, np.int64)
        np.cumsum(counts.reshape(-1), out=cum[1:])
        self.grp_start = cum  # index by (c*NB+b)*NWIN+w

        # batch / counts for pooling
        self.cnt = np.bincount(batch, minlength=G).astype(np.float32)
        self.inv_cnt = (1.0 / np.maximum(self.cnt, 1.0)).astype(np.float32)

    def core_streams(self, c, dinv):
        """Build per-core device streams: idx [128, SID] i16,
        dstloc/val [128, CT] f32, dinvb [128, NB] f32."""
        NB, WS, NSH = self.NB, self.WS, self.NSH
        idx_cols = np.zeros((128, self.SID), np.int16)
        dstloc = np.zeros((128, self.CT), np.float32)
        val = np.zeros((128, self.CT), np.float32)

        t = 0
        for sgi, sg in enumerate(self.sgs):
            # gather stream: (w, b) order
            for w in range(NWIN):
                col = self.icol[(sgi, w)]
                parts = []
                for b in sg:
                    g0 = self.grp_start[(c * NB + b) * NWIN + w]
                    g1 = self.grp_start[(c * NB + b) * NWIN + w + 1]
                    loc = self.permrow_s[g0:g1].astype(np.int16)
                    pad = int(self.cap[b, w]) - (g1 - g0)
                    parts.append(np.concatenate([loc, np.zeros(pad, np.int16)]))
                if parts:
                    flat = np.concatenate(parts)
                    if flat.size:
                        wrapped = np.tile(flat.reshape(-1, 16).T, (8, 1))
                        idx_cols[:, col:col + flat.size // 16] = wrapped
            # value/dst streams: (b, w, chunk) order
            for b in sg:
                for w in range(NWIN):
                    g0 = self.grp_start[(c * NB + b) * NWIN + w]
                    g1 = self.grp_start[(c * NB + b) * NWIN + w + 1]
                    n = g1 - g0
                    capbw = int(self.cap[b, w])
                    if capbw == 0:
                        continue
                    dl = np.zeros(capbw, np.float32)
                    vv = np.zeros(capbw, np.float32)
                    dl[:n] = (self.dst_s[g0:g1] - (c * NSH + b * P)).astype(np.float32)
                    vv[:n] = self.ew_s[g0:g1]
                    k = capbw // P
                    dstloc[:, t:t + k] = dl.reshape(k, P).T
                    val[:, t:t + k] = vv.reshape(k, P).T
                    t += k

        dinvb = np.zeros((128, NB), np.float32)
        sh_dinv = dinv[c * NSH:(c + 1) * NSH]
        for b in range(NB):
            nb = P if b < NB - 1 else self.LB
            dinvb[:nb, b] = sh_dinv[b * P:b * P + nb]
        return idx_cols, dstloc, val, dinvb

    def core_pool_oh(self, c, batch):
        """fp8 one-hot pooling matrix [128, NB*128]: [p, b*128+g] = (batch==g)."""
        import ml_dtypes
        FP8 = np.dtype(ml_dtypes.float8_e4m3)
        NB, NSH = self.NB, self.NSH
        oh = np.zeros((128, NB * 128), FP8)
        sh = batch[c * NSH:(c + 1) * NSH]
        for b in range(NB):
            nb = P if b < NB - 1 else self.LB
            g = sh[b * P:b * P + nb].astype(np.int64)
            oh[np.arange(nb), b * 128 + g] = 1.0
        return oh


def _host_prep(x, edge_index, batch, edge_attr, params, ncores):
    """All index-based preprocessing + BN folding. Returns (struct, in_maps)."""
    N, INDIM = x.shape
    G = int(params["cnt_G"])
    EPS = 1e-5

    src = np.asarray(edge_index[0], np.int64)
    dst = np.asarray(edge_index[1], np.int64)
    ew = np.asarray(edge_attr, np.float32)
    batch = np.asarray(batch, np.int64)

    deg = np.bincount(dst, weights=ew.astype(np.float64), minlength=N) + 1.0
    dinv = (1.0 / np.sqrt(deg)).astype(np.float32)

    st = GCNStructure(src, dst, ew, batch, N, G, ncores)

    def bnfold(g, be, m, v, bias):
        s = (g / np.sqrt(v + EPS)).astype(np.float32)
        cc = ((bias - m) * s + be).astype(np.float32)
        return s, cc

    S0, C0 = bnfold(params["g0"], params["be0"], params["m0"], params["v0"], params["b0"])
    S1, C1 = bnfold(params["g1"], params["be1"], params["m1"], params["v1"], params["b1"])
    Sf, Cf = bnfold(params["gf"], params["bef"], params["mf"], params["vf"], params["bf1"])

    # BN scale folded into the weight matrices (linear before the bias add)
    W0s = (np.asarray(params["W0"], np.float32) * S0[None, :]).astype(np.float32)
    W1s = (np.asarray(params["W1"], np.float32) * S1[None, :]).astype(np.float32)
    Wf1s = (np.asarray(params["Wf1"], np.float32) * Sf[None, :]).astype(np.float32)

    HID = params["W0"].shape[1]
    HHID = params["Wf1"].shape[1]
    NCLASS = params["Wf2"].shape[1]

    ident = np.eye(128, dtype=np.float32)

    NSH, NB = st.NSH, st.NB

    import ml_dtypes
    FP8 = np.dtype(ml_dtypes.float8_e4m3)
    BF16 = np.dtype(ml_dtypes.bfloat16)

    xpadT = np.zeros((ncores, INDIM, NB * P), BF16)
    xv = np.asarray(x, np.float32)
    for c in range(ncores):
        xpadT[c, :, :NSH] = xv[c * NSH:(c + 1) * NSH].T.astype(BF16)

    in_maps = []
    for c in range(ncores):
        idx_cols, dstloc, val, dinvb = st.core_streams(c, dinv)
        pool_oh = st.core_pool_oh(c, batch)
        # precomputed one-hot-times-weight B tiles, chunk-major [128, CT*128]
        bt3 = np.zeros((128, st.CT, 128), FP8)
        np.put_along_axis(bt3, dstloc.astype(np.int64)[:, :, None],
                          val.astype(FP8)[:, :, None], axis=2)
        # invd_flat[0, b*128+j] = 1/dinv at node (c, b, j); 0 for pad rows.
        # (single partition: Ldweights requires lhsT at partition 0)
        invd_flat = np.zeros((1, NB * 128), np.float32)
        sh_dinv = dinv[c * NSH:(c + 1) * NSH]
        for b in range(NB):
            nb = P if b < NB - 1 else st.LB
            invd_flat[0, b * 128:b * 128 + nb] = 1.0 / sh_dinv[b * P:b * P + nb]
        # cRep[b, :] = C (bias row replicated so lhsT/rhs base partitions match)
        cRep0 = np.tile(C0[None, :], (128, 1)).astype(BF16)
        cRep1 = np.tile(C1[None, :], (128, 1)).astype(BF16)
        in_maps.append(dict(
            xshT=xpadT[c],
            idxs=idx_cols,
            btiles=bt3.reshape(128, st.CT * 128),
            pool_oh=pool_oh, dinvb=dinvb,
            invd_flat=invd_flat.astype(BF16),
            cRep0=cRep0, cRep1=cRep1,
            w0=W0s.astype(BF16), w1=W1s, wf1=Wf1s,
            wf2=np.asarray(params["Wf2"], np.float32),
            cfb=np.tile(Cf[None, :], (128, 1)),
            bf2b=np.tile(np.asarray(params["bf2"], np.float32)[None, :], (128, 1)),
            invcnt=st.inv_cnt[:, None].copy(),
            ident=ident, identb=ident.astype(BF16),
        ))
    dims = dict(INDIM=INDIM, HID=HID, HHID=HHID, NCLASS=NCLASS)
    return st, in_maps, dims


# ------------------------------------------------------------- bass program --

def build_nc(st, dims, ncores, reps=1, fake_coll=False, no_gather=False, no_compute=False):
    from concourse import bass, mybir, bacc, tile

    INDIM, HID, HHID, NCLASS = dims["INDIM"], dims["HID"], dims["HHID"], dims["NCLASS"]
    N, G, NB, NSH, WS, LB = st.N, st.G, st.NB, st.NSH, st.WS, st.LB
    f32 = mybir.dt.float32
    bf16 = mybir.dt.bfloat16
    fp8 = mybir.dt.float8e4
    Alu = mybir.AluOpType
    Act = mybir.ActivationFunctionType

    nc = bacc.Bacc("TRN2", target_bir_lowering=False, debug=False,
                   enable_asserts=True, num_devices=ncores,
                   num_swdge_queues=NQUEUES,
                   dynamic_dma_scratch_size=DMA_SCRATCH)

    I = {}
    def inp(name, shape, dt=f32):
        I[name] = nc.dram_tensor(name, shape, dt, kind="ExternalInput")
        return I[name]

    inp("xshT", [INDIM, NB * P], bf16)
    inp("idxs", [128, st.SID], mybir.dt.int16)
    inp("btiles", [128, st.CT * 128], mybir.dt.float8e4)
    inp("pool_oh", [128, NB * 128], mybir.dt.float8e4)
    inp("dinvb", [128, NB])
    inp("invd_flat", [1, NB * 128], bf16)
    inp("cRep0", [128, HID], bf16); inp("cRep1", [128, HID], bf16)
    inp("w0", [INDIM, HID], bf16); inp("w1", [HID, HID])
    inp("wf1", [HID, HHID]); inp("wf2", [HHID, NCLASS])
    inp("cfb", [128, HHID])
    inp("bf2b", [128, NCLASS])
    inp("invcnt", [128, 1])
    inp("ident", [128, 128]); inp("identb", [128, 128], bf16)
    out_d = nc.dram_tensor("out", [G, NCLASS], f32, kind="ExternalOutput")

    qctr = [0]
    def next_q():
        q = qctr[0] % NQUEUES
        qctr[0] += 1
        return q

    with tile.TileContext(nc) as tc:
        import contextlib
        with contextlib.ExitStack() as ctx:
            const = ctx.enter_context(tc.tile_pool(name="const", bufs=1))
            stream = ctx.enter_context(tc.tile_pool(name="stream", bufs=1))
            xio = ctx.enter_context(tc.tile_pool(name="xio", bufs=3))
            xts = ctx.enter_context(tc.tile_pool(name="xts", bufs=3))
            hpool = ctx.enter_context(tc.tile_pool(name="hpool", bufs=NB))
            ypool = ctx.enter_context(tc.tile_pool(name="ypool", bufs=NB))
            y1pool = ctx.enter_context(tc.tile_pool(name="y1pool", bufs=NB))
            gpool = ctx.enter_context(tc.tile_pool(name="gpool", bufs=GBUFS))
            bpool = ctx.enter_context(tc.tile_pool(name="bpool", bufs=BBUFS))
            tmp = ctx.enter_context(tc.tile_pool(name="tmp", bufs=6))
            ptrans = ctx.enter_context(tc.tile_pool(name="ptrans", bufs=1, space="PSUM"))
            phw = ctx.enter_context(tc.tile_pool(name="phw", bufs=2, space="PSUM"))
            pacc = ctx.enter_context(tc.tile_pool(name="pacc", bufs=PACCB, space="PSUM"))
            ppool = ctx.enter_context(tc.tile_pool(name="ppool", bufs=1, space="PSUM"))
            dram = ctx.enter_context(tc.tile_pool(name="dram", bufs=1, space="DRAM"))

            # ---- constants into SBUF
            C = {}
            cdts = dict(w0=bf16, invd_flat=bf16, cRep0=bf16, cRep1=bf16, identb=bf16,
                        pool_oh=fp8)
            for nm in ["w0", "w1", "wf1", "wf2", "cfb", "bf2b", "invcnt",
                       "ident", "identb", "invd_flat", "cRep0", "cRep1",
                       "pool_oh"]:
                shape = list(I[nm].shape)
                tile_ = const.tile(shape, cdts.get(nm, f32), tag=nm)
                nc.sync.dma_start(out=tile_[:], in_=I[nm][:])
                C[nm] = tile_
            idx_t = stream.tile([128, st.SID], mybir.dt.int16, tag="idx")
            nc.sync.dma_start(out=idx_t[:], in_=I["idxs"][:])
            dinv_t = stream.tile([128, NB], f32, tag="dnv")
            nc.sync.dma_start(out=dinv_t[:], in_=I["dinvb"][:])

            shspace = "Shared" if ncores > 4 else "Local"
            HSH = NSH // 2  # nodes per half-shard
            bounce = [[dram.tile([HSH, HID], bf16, tag=f"bnc{l}{h}",
                                 name=f"bounce{l}{h}") for h in range(2)]
                      for l in range(2)]
            ar_in = dram.tile([G, HID], f32, tag="arin")

            # ---- phase A: h0' = dinv * (x @ W0), shard -> AllGather table0
            for _rep in range(reps):
              # per-rep Shared tables: a Shared DRAM tile allows only one writer
              # split per half for split AllGathers (overlap with compute)
              table = [[dram.tile([WS, 2 * HID], bf16, tag=f"tab{l}{h}_{_rep}",
                                  name=f"table{l}{h}_{_rep}",
                                  addr_space=shspace) for h in range(2)]
                       for l in range(2)]
              ar_out = dram.tile([G, HID], f32, tag=f"arout_{_rep}", addr_space=shspace)

              def bounce_write(l, b, nb, tile_):
                  # write rows [b*P, b*P+nb) of the shard into half bounces
                  r0, r1 = b * P, b * P + nb
                  if r0 < HSH:
                      e = min(r1, HSH)
                      nc.sync.dma_start(out=bounce[l][0][r0:e, :], in_=tile_[:e - r0, :])
                  if r1 > HSH:
                      s = max(r0, HSH)
                      nc.sync.dma_start(out=bounce[l][1][s - HSH:r1 - HSH, :],
                                        in_=tile_[s - r0:nb, :])

              def allgather_half(l, h):
                  if fake_coll:
                      nc.sync.dma_start(out=table[l][h][0:HSH // 2, :],
                                        in_=bounce[l][h][:])
                  else:
                      nc.gpsimd.collective_compute(
                          "AllGather", Alu.bypass,
                          replica_groups=[list(range(ncores))],
                          ins=[bounce[l][h].opt()], outs=[table[l][h].opt()],
                      )

              # last block fully inside half 0: fire AG half 0 after it
              bsplit = (HSH - 1) // P  # block containing row HSH-1

              h_tiles = []
              for b in range(NB):
                  nb = P if b < NB - 1 else LB
                  xt = xio.tile([128, 128], bf16, tag="xt")
                  nc.sync.dma_start(out=xt[:INDIM, :], in_=I["xshT"][:, b * P:(b + 1) * P])
                  hp = phw.tile([128, HID], f32, tag="hp")
                  nc.tensor.matmul(hp[:], lhsT=xt[:INDIM, :], rhs=C["w0"][:],
                                   start=True, stop=True)
                  hb16 = hpool.tile([128, HID], bf16, tag="h")
                  nc.scalar.activation(out=hb16[:], in_=hp[:], func=Act.Copy,
                                       scale=dinv_t[:, b:b + 1])
                  bounce_write(0, b, nb, hb16)
                  h_tiles.append(hb16)
                  if b == bsplit:
                      allgather_half(0, 0)
              allgather_half(0, 1)

              # ---- GCN layers
              pp = ppool.tile([128, HID], f32, tag="pool")
              for l in range(2):
                  cRep = C["cRep0"] if l == 0 else C["cRep1"]
                  h1_tiles = []
                  t = 0
                  for sgi, sg in enumerate(st.sgs):
                      gt = {}
                      for w in range(NWIN):
                          cols = st.gcols[(sgi, w)]
                          if cols == 0:
                              continue
                          gbf = gpool.tile([128, st.GMAX, 2 * HID], bf16, tag="g")
                          ic = st.icol[(sgi, w)]
                          gt[w] = gbf
                          if no_gather:
                              continue
                          nc.gpsimd.dma_gather(
                              out_ap=gbf[:, :cols, :],
                              in_ap=table[l][w // 2][:, :],
                              idxs_ap=idx_t[:, ic:ic + cols * 8],
                              num_idxs=cols * P,
                              num_idxs_reg=cols * P,
                              elem_size=2 * HID,
                              queue_num=next_q(),
                          )
                      sgch = sum(int(st.cap[b].sum()) // P for b in sg)
                      if sgch and not no_compute:
                          bsl = bpool.tile([128, st.SGMAX * 128], fp8, tag="B")
                          nc.sync.dma_start(
                              out=bsl[:, :sgch * 128],
                              in_=I["btiles"][:, t * 128:(t + sgch) * 128])
                      tsg = 0
                      for b in sg:
                          nchunks = 0 if no_compute else int(st.cap[b].sum()) // P
                          acc = pacc.tile([128, HID], f32, tag="acc")
                          # inject Cb/dinv[dst] (outer product, K=1), then the
                          # self-loop h'[dst] (identity matmul), then the edges
                          nc.tensor.matmul(acc[:],
                                           lhsT=C["invd_flat"][0:1, b * 128:(b + 1) * 128],
                                           rhs=cRep[0:1, :],
                                           start=True, stop=False)
                          nc.tensor.matmul(acc[:], lhsT=C["identb"][:],
                                           rhs=h_tiles[b][:],
                                           start=False, stop=(nchunks == 0))
                          done = 0
                          for w in range(NWIN if not no_compute else 0):
                              kk = int(st.cap[b, w]) // P
                              for j in range(kk):
                                  nc.tensor.matmul(
                                      acc[:], lhsT=bsl[:, tsg * 128:(tsg + 1) * 128],
                                      rhs=gt[w][:, st.coloff[(b, w)] + j,
                                                (w % 2) * HID:(w % 2 + 1) * HID],
                                      start=False, stop=(done == nchunks - 1))
                                  done += 1
                                  t += 1
                                  tsg += 1
                          nb = P if b < NB - 1 else LB
                          if l == 0:
                              # y0 (f32) -> transpose -> h1' = dinv*(y0 @ W1)
                              yb = ypool.tile([128, HID], f32, tag="y")
                              nc.scalar.activation(out=yb[:], in_=acc[:], func=Act.Relu,
                                                   scale=dinv_t[:, b:b + 1])
                              pt = ptrans.tile([128, 128], f32, tag="pt")
                              nc.tensor.transpose(pt[:HID, :], yb[:], C["ident"][:])
                              yTs = xts.tile([128, 128], f32, tag="xT")
                              nc.scalar.activation(out=yTs[:HID, :], in_=pt[:HID, :],
                                                   func=Act.Copy)
                              hp = phw.tile([128, HID], f32, tag="hp")
                              nc.tensor.matmul(hp[:], lhsT=yTs[:HID, :], rhs=C["w1"][:],
                                               start=True, stop=True)
                              h1b = hpool.tile([128, HID], bf16, tag="h")
                              nc.scalar.activation(out=h1b[:], in_=hp[:], func=Act.Copy,
                                                   scale=dinv_t[:, b:b + 1])
                              bounce_write(1, b, nb, h1b)
                              h1_tiles.append(h1b)
                          else:
                              # y1 (bf16) -> inline mean-pool accumulation
                              yb = y1pool.tile([128, HID], bf16, tag="y1")
                              nc.scalar.activation(out=yb[:], in_=acc[:], func=Act.Relu,
                                                   scale=dinv_t[:, b:b + 1])
                              nc.tensor.matmul(pp[:G, :],
                                               lhsT=C["pool_oh"][:nb, b * 128:b * 128 + G],
                                               rhs=yb[:nb, :],
                                               start=(b == 0), stop=(b == NB - 1))
                  if l == 0:
                      h_tiles = h1_tiles
                      allgather_half(1, 0)
                      allgather_half(1, 1)

              # ---- mean pool partials -> AllReduce -> head
              pooled = tmp.tile([128, HID], f32, tag="pl")
              nc.scalar.activation(out=pooled[:G, :], in_=pp[:G, :], func=Act.Copy)
              nc.sync.dma_start(out=ar_in[:], in_=pooled[:G, :])
              if fake_coll:
                  nc.sync.dma_start(out=ar_out[:], in_=ar_in[:])
              else:
                  nc.gpsimd.collective_compute(
                      "AllReduce", Alu.add,
                      replica_groups=[list(range(ncores))],
                      ins=[ar_in.opt()], outs=[ar_out.opt()],
                  )
              pooled2 = tmp.tile([128, HID], f32, tag="pl2")
              nc.sync.dma_start(out=pooled2[:G, :], in_=ar_out[:])
              nc.vector.tensor_scalar(out=pooled2[:G, :], in0=pooled2[:G, :],
                                      scalar1=C["invcnt"][:G, :], scalar2=None,
                                      op0=Alu.mult)

              # z = relu(Sf * (pooled @ Wf1) + Cf)
              pt = ptrans.tile([128, 128], f32, tag="pt")
              nc.tensor.transpose(pt[:HID, :G], pooled2[:G, :], C["ident"][:])
              pTs = xts.tile([128, 128], f32, tag="xT")
              nc.vector.tensor_copy(out=pTs[:HID, :G], in_=pt[:HID, :G])
              zp = phw.tile([128, HHID], f32, tag="hp")
              nc.tensor.matmul(zp[:G, :], lhsT=pTs[:HID, :G], rhs=C["wf1"][:],
                               start=True, stop=True)
              z = tmp.tile([128, HHID], f32, tag="z")
              nc.vector.tensor_tensor(out=z[:G, :], in0=zp[:G, :], in1=C["cfb"][:G, :], op=Alu.add)
              nc.vector.tensor_scalar(out=z[:G, :], in0=z[:G, :], scalar1=0.0,
                                      scalar2=None, op0=Alu.max)

              # logits = z @ Wf2 + bf2; out = log_softmax(logits)
              pt2 = ptrans.tile([128, 128], f32, tag="pt")
              nc.tensor.transpose(pt2[:HHID, :G], z[:G, :], C["ident"][:])
              zTs = xts.tile([128, 128], f32, tag="xT")
              nc.vector.tensor_copy(out=zTs[:HHID, :G], in_=pt2[:HHID, :G])
              lp = phw.tile([128, NCLASS], f32, tag="hp")
              nc.tensor.matmul(lp[:G, :], lhsT=zTs[:HHID, :G], rhs=C["wf2"][:],
                               start=True, stop=True)
              lg = tmp.tile([128, NCLASS], f32, tag="lg")
              nc.vector.tensor_tensor(out=lg[:G, :], in0=lp[:G, :], in1=C["bf2b"][:G, :], op=Alu.add)
              mx = tmp.tile([128, 1], f32, tag="mx")
              nc.vector.reduce_max(mx[:G, :], lg[:G, :], axis=mybir.AxisListType.X)
              nc.vector.tensor_scalar(out=lg[:G, :], in0=lg[:G, :], scalar1=mx[:G, :],
                                      scalar2=None, op0=Alu.subtract)
              ex = tmp.tile([128, NCLASS], f32, tag="ex")
              nc.scalar.activation(out=ex[:G, :], in_=lg[:G, :], func=Act.Exp)
              sm = tmp.tile([128, 1], f32, tag="sm")
              nc.vector.reduce_sum(sm[:G, :], ex[:G, :], axis=mybir.AxisListType.X)
              lsm = tmp.tile([128, 1], f32, tag="ls")
              nc.scalar.activation(out=lsm[:G, :], in_=sm[:G, :], func=Act.Ln)
              nc.vector.tensor_scalar(out=lg[:G, :], in0=lg[:G, :], scalar1=lsm[:G, :],
                                      scalar2=None, op0=Alu.subtract)
              nc.sync.dma_start(out=out_d[:], in_=lg[:G, :])

    nc.compile()
    return nc




# ------------------------------------------------------------ PJRT runner --

class SpmdRunner:
    """Run the compiled 8-core Bass module via PJRT (axon), mirroring
    concourse.bass2jax.run_bass_via_pjrt but keeping the jitted callable."""

    def __init__(self, nc, n_cores):
        import jax
        from jax.sharding import Mesh, PartitionSpec
        from jax.experimental.shard_map import shard_map
        from concourse import bass2jax, mybir as _mb
        from concourse.bass2jax import _bass_exec_p, install_neuronx_cc_hook
        install_neuronx_cc_hook()
        self.jax = jax
        self.nc = nc
        self.n_cores = n_cores
        partition_name = nc.partition_id_tensor.name if nc.partition_id_tensor else None
        in_names, out_names, out_avals, zero_outs = [], [], [], []
        for alloc in nc.m.functions[0].allocations:
            if not isinstance(alloc, _mb.MemoryLocationSet):
                continue
            name = alloc.memorylocations[0].name
            if alloc.kind == "ExternalInput":
                if name != partition_name:
                    in_names.append(name)
            elif alloc.kind == "ExternalOutput":
                shape = tuple(alloc.tensor_shape)
                dtype = _mb.dt.np(alloc.dtype)
                out_names.append(name)
                out_avals.append(jax.core.ShapedArray(shape, dtype))
                zero_outs.append(np.zeros(shape, dtype))
        self.in_names, self.out_names = in_names, out_names
        self.out_avals, self.zero_outs = out_avals, zero_outs
        n_params, n_outs = len(in_names), len(out_avals)
        self.n_params = n_params
        all_in_names = in_names + out_names + ([partition_name] if partition_name else [])

        def _body(*args):
            operands = list(args)
            if partition_name is not None:
                operands.append(bass2jax.partition_id_tensor())
            return tuple(_bass_exec_p.bind(
                *operands, out_avals=tuple(out_avals), in_names=tuple(all_in_names),
                out_names=tuple(out_names), lowering_input_output_aliases=(),
                sim_require_finite=True, sim_require_nnan=True, nc=nc))

        devices = jax.devices()[:n_cores]
        assert len(devices) == n_cores
        mesh = Mesh(np.asarray(devices), ("core",))
        self._sharding = jax.sharding.NamedSharding(mesh, PartitionSpec("core"))
        in_specs = (PartitionSpec("core"),) * (n_params + n_outs)
        out_specs = (PartitionSpec("core"),) * len(out_names)
        self._fn = jax.jit(
            shard_map(_body, mesh=mesh, in_specs=in_specs,
                      out_specs=out_specs, check_rep=False),
            keep_unused=True)

    def prepare(self, in_maps):
        per_core = [[np.asarray(m[name]) for name in self.in_names] for m in in_maps]
        concat_in = [np.concatenate([per_core[c][i] for c in range(self.n_cores)], axis=0)
                     for i in range(self.n_params)]
        concat_zeros = [np.zeros((self.n_cores * z.shape[0], *z.shape[1:]), z.dtype)
                        for z in self.zero_outs]
        return concat_in + concat_zeros

    def run(self, in_maps):
        out_arrs = self._fn(*self.prepare(in_maps))
        self.jax.block_until_ready(out_arrs)
        return self._split(out_arrs)

    def _split(self, out_arrs):
        return [{name: np.asarray(out_arrs[i]).reshape(self.n_cores, *self.out_avals[i].shape)[c]
                 for i, name in enumerate(self.out_names)}
                for c in range(self.n_cores)]

    def time(self, in_maps, iters=8):
        import time as _t
        args = self.prepare(in_maps)
        dargs = [self.jax.device_put(a, self._sharding) for a in args]
        out = self._fn(*dargs)
        self.jax.block_until_ready(out)
        results = self._split(out)
        times = []
        for _ in range(iters):
            t0 = _t.perf_counter()
            o = self._fn(*dargs)
            self.jax.block_until_ready(o)
            times.append(_t.perf_counter() - t0)
        return results, times


# ------------------------------------------------------------------- driver --

_CACHE = {}


def _get_runner(st, dims, ncores):
    nc = build_nc(st, dims, ncores)
    return SpmdRunner(nc, ncores)


def kernel(**inputs):
    x = np.asarray(inputs["x"], np.float32)
    edge_index = np.asarray(inputs["edge_index"])
    batch = np.asarray(inputs["batch"])
    edge_attr = np.asarray(inputs["edge_attr"], np.float32)
    G = 128
    params = {k: np.asarray(v) for k, v in inputs.items()
              if k not in ("x", "edge_index", "batch", "edge_attr", "pos")}
    params["cnt_G"] = G
    ncores = 8

    st, in_maps, dims = _host_prep(x, edge_index, batch, edge_attr, params, ncores)

    key = ("k", x.shape, edge_index.shape, st.SID, st.CT, st.GMAX,
           tuple(tuple(s) for s in st.sgs))
    if key not in _CACHE:
        _CACHE[key] = _get_runner(st, dims, ncores)
    runner = _CACHE[key]
    _LAST.update(st=st, dims=dims, ncores=ncores, in_maps=in_maps, runner=runner)
    results = runner.run(in_maps)
    return results[0]["out"]


_LAST = {}


def estimate_exec_ns(reps=16, iters=10):
    """Per-execution device time via wall-clock delta between a 1-rep NEFF and
    an in-NEFF `reps`-times-repeated body (cancels the axon dispatch floor).
    Median-based: the axon tunnel has heavy-tailed per-call jitter."""
    import time as _t
    import jax
    st, dims, ncores = _LAST["st"], _LAST["dims"], _LAST["ncores"]
    in_maps, r1 = _LAST["in_maps"], _LAST["runner"]
    rR = SpmdRunner(build_nc(st, dims, ncores, reps=reps), ncores)
    a1 = [jax.device_put(a, r1._sharding) for a in r1.prepare(in_maps)]
    aR = [jax.device_put(a, rR._sharding) for a in rR.prepare(in_maps)]
    jax.block_until_ready(r1._fn(*a1)); jax.block_until_ready(rR._fn(*aR))
    t1s, tRs = [], []
    for _ in range(iters):
        t0 = _t.perf_counter(); jax.block_until_ready(r1._fn(*a1)); t1s.append(_t.perf_counter() - t0)
        t0 = _t.perf_counter(); jax.block_until_ready(rR._fn(*aR)); tRs.append(_t.perf_counter() - t0)
    t1s, tRs = sorted(t1s), sorted(tRs)
    per = (tRs[len(tRs) // 2] - t1s[len(t1s) // 2]) / (reps - 1)
    return per * 1e9


# revision 14
# speedup vs baseline: 1.4002x; 1.0009x over previous
"""Trainium2 Bass kernel for nn_BaselineGCN (2-layer GCN + BN + mean-pool + MLP head).

Strategy (8 NeuronCores):
 - Nodes sharded contiguously across cores; each core owns the in-edges of its
   node shard (dst-sharding, per the graph-partitioning hint).
 - gcn_norm factorized: deg/dinv computed host-side (index/weight preprocessing);
   per-edge message = w_e * h'[src] with h' = dinv * (h @ W); per-dst scale by
   dinv[dst] folded into the ACT-engine epilogue.
 - The per-edge gather h'[src] runs on-device via SWDGE dma_gather (256B rows)
   from an AllGather'ed replica of h' in each core's DRAM, round-robined over 4
   SWDGE queues.
 - segment_sum becomes TensorE matmuls: per 128-edge chunk, a host-precomputed
   one-hot-times-weight fp8 matrix B[e, dst_local] is loaded as lhsT and PE
   accumulates B.T @ gathered_rows into the dst-block's PSUM tile.  The BN bias
   term (Cb) and the self-loop (+h'[dst]) are injected into the same PSUM
   accumulation via a K=1 outer-product matmul and a bf16 identity matmul, so
   the whole per-block epilogue is ONE ScalarE op: y = Relu(acc * dinv[dst]).
 - VectorE is kept almost empty: the v1 kernel was DVE+GPSIMD co-bottlenecked
   (DVE ~70% busy on epilogue/cast chains created backpressure that limited
   SWDGE descriptor-gen queue concurrency to ~2 of 4).
 - Graph mean-pool is a host-precomputed fp8 one-hot matmul accumulated inline
   with layer-1 epilogues; partials summed with an AllReduce; the tiny MLP
   head + log_softmax run on every core.
"""
import sys
import time

sys.path.insert(0, "/opt/trn_rl_repo")

import numpy as np

P = 128          # partitions / block size
NWIN = 4         # gather index windows (int16 range)
MAXCALL = 1024   # max indices per dma_gather (SWDGE ring capacity)
DMA_SCRATCH = 16384  # SWDGE ring carveout bytes (ring = this // 16 descs)
NQUEUES = 4      # SWDGE queues to round-robin
GBUFS = 16       # gather tile lookahead
BBUFS = 4        # B-slab lookahead (per-supergroup fp8 slabs)
PACCB = 4        # PSUM accumulator banks


# ---------------------------------------------------------------- host prep --

def _ceil(a, b):
    return -(-a // b)


class GCNStructure:
    """Graph partitioning + stream layout. Capacities are maxed across cores so
    the single SPMD program fits every core's data."""

    def __init__(self, src, dst, ew, batch, N, G, ncores):
        self.N, self.G, self.C = N, G, ncores
        NSH = N // ncores
        NB = _ceil(NSH, P)
        WS = _ceil(N // 2, NWIN // 2)  # window size in PAIRED rows
        assert N % ncores == 0
        assert WS <= 32767, "gather window exceeds int16"
        self.NSH, self.NB, self.WS = NSH, NB, WS
        self.LB = NSH - (NB - 1) * P  # rows in last block

        core = dst // NSH
        blk = (dst % NSH) // P
        # table permutation for split AllGathers: half H of every core's shard
        # is gathered into table-half H; row = H*WS + src_core*HWS + local//2
        HWS = NSH // 4  # pair-rows per (core, half)
        self.HWS = HWS
        sc = src // NSH
        sl = src % NSH
        H = sl // (NSH // 2)
        permrow = sc * HWS + (sl % (NSH // 2)) // 2
        # cell = (half, parity): row packs 2 nodes per 256B
        win = H * 2 + (src & 1)
        key = (core * NB + blk) * NWIN + win
        # secondary sort by permuted table row: ascending gather addresses
        # within each (core, block, cell) group improve DRAM read locality
        order = np.lexsort((permrow, key))
        self.permrow_s = permrow[order]
        self.src_s, self.dst_s, self.ew_s = src[order], dst[order], ew[order]
        counts = np.bincount(key, minlength=ncores * NB * NWIN).reshape(ncores, NB, NWIN)
        self.counts = counts
        cap = counts.max(axis=0)
        cap = _ceil(np.maximum(cap, 0), P) * P  # per (b, w), 0 stays 0
        self.cap = cap  # [NB, NWIN]

        # supergroups: consecutive blocks such that per-window call <= MAXCALL
        self.sgs = []
        cur = [0]
        for b in range(1, NB):
            trial = cur + [b]
            if all(cap[trial, w].sum() <= MAXCALL for w in range(NWIN)):
                cur = trial
            else:
                self.sgs.append(cur)
                cur = [b]
        self.sgs.append(cur)

        # layout: gather calls in (sg, w) order; chunks in (sg, b, w, j) order
        self.gcols = {}    # (sgi, w) -> columns in that call's tile
        self.icol = {}     # (sgi, w) -> start col (units of 16-idx) in idx stream
        self.coloff = {}   # (b, w) -> column offset inside its call tile
        sid = 0
        for sgi, sg in enumerate(self.sgs):
            for w in range(NWIN):
                cols = int(cap[sg, w].sum()) // P
                self.gcols[(sgi, w)] = cols
                self.icol[(sgi, w)] = sid
                off = 0
                for b in sg:
                    self.coloff[(b, w)] = off
                    off += int(cap[b, w]) // P
                sid += cols * 8  # n/16 = cols*128/16
        self.SID = max(sid, 8)
        self.CT = max(int(cap.sum()) // P, 1)
        self.GMAX = max(max(self.gcols.values(), default=1), 1)
        self.SGMAX = max(sum(int(cap[b].sum()) // P for b in sg) for sg in self.sgs)

        # per-core edge offsets into the sorted arrays, per (b, w)
        cum = np.zeros(ncores * NB * NWIN + 1, np.int64)
        np.cumsum(counts.reshape(-1), out=cum[1:])
        self.grp_start = cum  # index by (c*NB+b)*NWIN+w

        # batch / counts for pooling
        self.cnt = np.bincount(batch, minlength=G).astype(np.float32)
        self.inv_cnt = (1.0 / np.maximum(self.cnt, 1.0)).astype(np.float32)

    def core_streams(self, c, dinv):
        """Build per-core device streams: idx [128, SID] i16,
        dstloc/val [128, CT] f32, dinvb [128, NB] f32."""
        NB, WS, NSH = self.NB, self.WS, self.NSH
        idx_cols = np.zeros((128, self.SID), np.int16)
        dstloc = np.zeros((128, self.CT), np.float32)
        val = np.zeros((128, self.CT), np.float32)

        t = 0
        for sgi, sg in enumerate(self.sgs):
            # gather stream: (w, b) order
            for w in range(NWIN):
                col = self.icol[(sgi, w)]
                parts = []
                for b in sg:
                    g0 = self.grp_start[(c * NB + b) * NWIN + w]
                    g1 = self.grp_start[(c * NB + b) * NWIN + w + 1]
                    loc = self.permrow_s[g0:g1].astype(np.int16)
                    pad = int(self.cap[b, w]) - (g1 - g0)
                    parts.append(np.concatenate([loc, np.zeros(pad, np.int16)]))
                if parts:
                    flat = np.concatenate(parts)
                    if flat.size:
                        wrapped = np.tile(flat.reshape(-1, 16).T, (8, 1))
                        idx_cols[:, col:col + flat.size // 16] = wrapped
            # value/dst streams: (b, w, chunk) order
            for b in sg:
                for w in range(NWIN):
                    g0 = self.grp_start[(c * NB + b) * NWIN + w]
                    g1 = self.grp_start[(c * NB + b) * NWIN + w + 1]
                    n = g1 - g0
                    capbw = int(self.cap[b, w])
                    if capbw == 0:
                        continue
                    dl = np.zeros(capbw, np.float32)
                    vv = np.zeros(capbw, np.float32)
                    dl[:n] = (self.dst_s[g0:g1] - (c * NSH + b * P)).astype(np.float32)
                    vv[:n] = self.ew_s[g0:g1]
                    k = capbw // P
                    dstloc[:, t:t + k] = dl.reshape(k, P).T
                    val[:, t:t + k] = vv.reshape(k, P).T
                    t += k

        dinvb = np.zeros((128, NB), np.float32)
        sh_dinv = dinv[c * NSH:(c + 1) * NSH]
        for b in range(NB):
            nb = P if b < NB - 1 else self.LB
            dinvb[:nb, b] = sh_dinv[b * P:b * P + nb]
        return idx_cols, dstloc, val, dinvb

    def core_pool_oh(self, c, batch):
        """fp8 one-hot pooling matrix [128, NB*128]: [p, b*128+g] = (batch==g)."""
        import ml_dtypes
        FP8 = np.dtype(ml_dtypes.float8_e4m3)
        NB, NSH = self.NB, self.NSH
        oh = np.zeros((128, NB * 128), FP8)
        sh = batch[c * NSH:(c + 1) * NSH]
        for b in range(NB):
            nb = P if b < NB - 1 else self.LB
            g = sh[b * P:b * P + nb].astype(np.int64)
            oh[np.arange(nb), b * 128 + g] = 1.0
        return oh


def _host_prep(x, edge_index, batch, edge_attr, params, ncores):
    """All index-based preprocessing + BN folding. Returns (struct, in_maps)."""
    N, INDIM = x.shape
    G = int(params["cnt_G"])
    EPS = 1e-5

    src = np.asarray(edge_index[0], np.int64)
    dst = np.asarray(edge_index[1], np.int64)
    ew = np.asarray(edge_attr, np.float32)
    batch = np.asarray(batch, np.int64)

    deg = np.bincount(dst, weights=ew.astype(np.float64), minlength=N) + 1.0
    dinv = (1.0 / np.sqrt(deg)).astype(np.float32)

    st = GCNStructure(src, dst, ew, batch, N, G, ncores)

    def bnfold(g, be, m, v, bias):
        s = (g / np.sqrt(v + EPS)).astype(np.float32)
        cc = ((bias - m) * s + be).astype(np.float32)
        return s, cc

    S0, C0 = bnfold(params["g0"], params["be0"], params["m0"], params["v0"], params["b0"])
    S1, C1 = bnfold(params["g1"], params["be1"], params["m1"], params["v1"], params["b1"])
    Sf, Cf = bnfold(params["gf"], params["bef"], params["mf"], params["vf"], params["bf1"])

    # BN scale folded into the weight matrices (linear before the bias add)
    W0s = (np.asarray(params["W0"], np.float32) * S0[None, :]).astype(np.float32)
    W1s = (np.asarray(params["W1"], np.float32) * S1[None, :]).astype(np.float32)
    Wf1s = (np.asarray(params["Wf1"], np.float32) * Sf[None, :]).astype(np.float32)

    HID = params["W0"].shape[1]
    HHID = params["Wf1"].shape[1]
    NCLASS = params["Wf2"].shape[1]

    ident = np.eye(128, dtype=np.float32)

    NSH, NB = st.NSH, st.NB

    import ml_dtypes
    FP8 = np.dtype(ml_dtypes.float8_e4m3)
    BF16 = np.dtype(ml_dtypes.bfloat16)

    xpadT = np.zeros((ncores, INDIM, NB * P), BF16)
    xv = np.asarray(x, np.float32)
    for c in range(ncores):
        xpadT[c, :, :NSH] = xv[c * NSH:(c + 1) * NSH].T.astype(BF16)

    in_maps = []
    for c in range(ncores):
        idx_cols, dstloc, val, dinvb = st.core_streams(c, dinv)
        pool_oh = st.core_pool_oh(c, batch)
        # precomputed one-hot-times-weight B tiles, chunk-major [128, CT*128]
        bt3 = np.zeros((128, st.CT, 128), FP8)
        np.put_along_axis(bt3, dstloc.astype(np.int64)[:, :, None],
                          val.astype(FP8)[:, :, None], axis=2)
        # invd_flat[0, b*128+j] = 1/dinv at node (c, b, j); 0 for pad rows.
        # (single partition: Ldweights requires lhsT at partition 0)
        invd_flat = np.zeros((1, NB * 128), np.float32)
        sh_dinv = dinv[c * NSH:(c + 1) * NSH]
        for b in range(NB):
            nb = P if b < NB - 1 else st.LB
            invd_flat[0, b * 128:b * 128 + nb] = 1.0 / sh_dinv[b * P:b * P + nb]
        # cRep[b, :] = C (bias row replicated so lhsT/rhs base partitions match)
        cRep0 = np.tile(C0[None, :], (128, 1)).astype(BF16)
        cRep1 = np.tile(C1[None, :], (128, 1)).astype(BF16)
        in_maps.append(dict(
            xshT=xpadT[c],
            idxs=idx_cols,
            btiles=bt3.reshape(128, st.CT * 128),
            pool_oh=pool_oh, dinvb=dinvb,
            invd_flat=invd_flat.astype(BF16),
            cRep0=cRep0, cRep1=cRep1,
            w0=W0s.astype(BF16), w1=W1s, wf1=Wf1s,
            wf2=np.asarray(params["Wf2"], np.float32),
            cfb=np.tile(Cf[None, :], (128, 1)),
            bf2b=np.tile(np.asarray(params["bf2"], np.float32)[None, :], (128, 1)),
            invcnt=st.inv_cnt[:, None].copy(),
            ident=ident, identb=ident.astype(BF16),
        ))
    dims = dict(INDIM=INDIM, HID=HID, HHID=HHID, NCLASS=NCLASS)
    return st, in_maps, dims


# ------------------------------------------------------------- bass program --

def build_nc(st, dims, ncores, reps=1, fake_coll=False, no_gather=False, no_compute=False):
    from concourse import bass, mybir, bacc, tile

    INDIM, HID, HHID, NCLASS = dims["INDIM"], dims["HID"], dims["HHID"], dims["NCLASS"]
    N, G, NB, NSH, WS, LB = st.N, st.G, st.NB, st.NSH, st.WS, st.LB
    f32 = mybir.dt.float32
    bf16 = mybir.dt.bfloat16
    fp8 = mybir.dt.float8e4
    Alu = mybir.AluOpType
    Act = mybir.ActivationFunctionType

    nc = bacc.Bacc("TRN2", target_bir_lowering=False, debug=False,
                   enable_asserts=True, num_devices=ncores,
                   num_swdge_queues=NQUEUES,
                   dynamic_dma_scratch_size=DMA_SCRATCH)

    I = {}
    def inp(name, shape, dt=f32):
        I[name] = nc.dram_tensor(name, shape, dt, kind="ExternalInput")
        return I[name]

    inp("xshT", [INDIM, NB * P], bf16)
    inp("idxs", [128, st.SID], mybir.dt.int16)
    inp("btiles", [128, st.CT * 128], mybir.dt.float8e4)
    inp("pool_oh", [128, NB * 128], mybir.dt.float8e4)
    inp("dinvb", [128, NB])
    inp("invd_flat", [1, NB * 128], bf16)
    inp("cRep0", [128, HID], bf16); inp("cRep1", [128, HID], bf16)
    inp("w0", [INDIM, HID], bf16); inp("w1", [HID, HID])
    inp("wf1", [HID, HHID]); inp("wf2", [HHID, NCLASS])
    inp("cfb", [128, HHID])
    inp("bf2b", [128, NCLASS])
    inp("invcnt", [128, 1])
    inp("ident", [128, 128]); inp("identb", [128, 128], bf16)
    out_d = nc.dram_tensor("out", [G, NCLASS], f32, kind="ExternalOutput")

    qctr = [0]
    def next_q():
        q = qctr[0] % NQUEUES
        qctr[0] += 1
        return q

    with tile.TileContext(nc) as tc:
        import contextlib
        with contextlib.ExitStack() as ctx:
            const = ctx.enter_context(tc.tile_pool(name="const", bufs=1))
            stream = ctx.enter_context(tc.tile_pool(name="stream", bufs=1))
            xio = ctx.enter_context(tc.tile_pool(name="xio", bufs=3))
            xts = ctx.enter_context(tc.tile_pool(name="xts", bufs=3))
            hpool = ctx.enter_context(tc.tile_pool(name="hpool", bufs=NB))
            ypool = ctx.enter_context(tc.tile_pool(name="ypool", bufs=NB))
            y1pool = ctx.enter_context(tc.tile_pool(name="y1pool", bufs=NB))
            gpool = ctx.enter_context(tc.tile_pool(name="gpool", bufs=GBUFS))
            bpool = ctx.enter_context(tc.tile_pool(name="bpool", bufs=BBUFS))
            tmp = ctx.enter_context(tc.tile_pool(name="tmp", bufs=6))
            ptrans = ctx.enter_context(tc.tile_pool(name="ptrans", bufs=1, space="PSUM"))
            phw = ctx.enter_context(tc.tile_pool(name="phw", bufs=2, space="PSUM"))
            pacc = ctx.enter_context(tc.tile_pool(name="pacc", bufs=PACCB, space="PSUM"))
            ppool = ctx.enter_context(tc.tile_pool(name="ppool", bufs=1, space="PSUM"))
            dram = ctx.enter_context(tc.tile_pool(name="dram", bufs=1, space="DRAM"))

            # ---- constants into SBUF
            C = {}
            cdts = dict(w0=bf16, invd_flat=bf16, cRep0=bf16, cRep1=bf16, identb=bf16,
                        pool_oh=fp8)
            for nm in ["w0", "w1", "wf1", "wf2", "cfb", "bf2b", "invcnt",
                       "ident", "identb", "invd_flat", "cRep0", "cRep1",
                       "pool_oh"]:
                shape = list(I[nm].shape)
                tile_ = const.tile(shape, cdts.get(nm, f32), tag=nm)
                nc.sync.dma_start(out=tile_[:], in_=I[nm][:])
                C[nm] = tile_
            idx_t = stream.tile([128, st.SID], mybir.dt.int16, tag="idx")
            nc.sync.dma_start(out=idx_t[:], in_=I["idxs"][:])
            dinv_t = stream.tile([128, NB], f32, tag="dnv")
            nc.sync.dma_start(out=dinv_t[:], in_=I["dinvb"][:])

            shspace = "Shared" if ncores > 4 else "Local"
            HSH = NSH // 2  # nodes per half-shard
            bounce = [[dram.tile([HSH, HID], bf16, tag=f"bnc{l}{h}",
                                 name=f"bounce{l}{h}") for h in range(2)]
                      for l in range(2)]
            ar_in = dram.tile([G, HID], f32, tag="arin")

            # ---- phase A: h0' = dinv * (x @ W0), shard -> AllGather table0
            for _rep in range(reps):
              # per-rep Shared tables: a Shared DRAM tile allows only one writer
              # split per half for split AllGathers (overlap with compute)
              table = [[dram.tile([WS, 2 * HID], bf16, tag=f"tab{l}{h}_{_rep}",
                                  name=f"table{l}{h}_{_rep}",
                                  addr_space=shspace) for h in range(2)]
                       for l in range(2)]
              ar_out = dram.tile([G, HID], f32, tag=f"arout_{_rep}", addr_space=shspace)

              def bounce_write(l, b, nb, tile_):
                  # write rows [b*P, b*P+nb) of the shard into half bounces
                  r0, r1 = b * P, b * P + nb
                  if r0 < HSH:
                      e = min(r1, HSH)
                      nc.sync.dma_start(out=bounce[l][0][r0:e, :], in_=tile_[:e - r0, :])
                  if r1 > HSH:
                      s = max(r0, HSH)
                      nc.sync.dma_start(out=bounce[l][1][s - HSH:r1 - HSH, :],
                                        in_=tile_[s - r0:nb, :])

              def allgather_half(l, h):
                  if fake_coll:
                      nc.sync.dma_start(out=table[l][h][0:HSH // 2, :],
                                        in_=bounce[l][h][:])
                  else:
                      nc.gpsimd.collective_compute(
                          "AllGather", Alu.bypass,
                          replica_groups=[list(range(ncores))],
                          ins=[bounce[l][h].opt()], outs=[table[l][h].opt()],
                      )

              # last block fully inside half 0: fire AG half 0 after it
              bsplit = (HSH - 1) // P  # block containing row HSH-1

              h_tiles = []
              for b in range(NB):
                  nb = P if b < NB - 1 else LB
                  xt = xio.tile([128, 128], bf16, tag="xt")
                  nc.sync.dma_start(out=xt[:INDIM, :], in_=I["xshT"][:, b * P:(b + 1) * P])
                  hp = phw.tile([128, HID], f32, tag="hp")
                  nc.tensor.matmul(hp[:], lhsT=xt[:INDIM, :], rhs=C["w0"][:],
                                   start=True, stop=True)
                  hb16 = hpool.tile([128, HID], bf16, tag="h")
                  nc.scalar.activation(out=hb16[:], in_=hp[:], func=Act.Copy,
                                       scale=dinv_t[:, b:b + 1])
                  bounce_write(0, b, nb, hb16)
                  h_tiles.append(hb16)
                  if b == bsplit:
                      allgather_half(0, 0)
              allgather_half(0, 1)

              # ---- GCN layers
              pp = ppool.tile([128, HID], f32, tag="pool")
              for l in range(2):
                  cRep = C["cRep0"] if l == 0 else C["cRep1"]
                  h1_tiles = []
                  t = 0
                  for sgi, sg in enumerate(st.sgs):
                      gt = {}
                      for w in range(NWIN):
                          cols = st.gcols[(sgi, w)]
                          if cols == 0:
                              continue
                          gbf = gpool.tile([128, st.GMAX, 2 * HID], bf16, tag="g")
                          ic = st.icol[(sgi, w)]
                          gt[w] = gbf
                          if no_gather:
                              continue
                          nc.gpsimd.dma_gather(
                              out_ap=gbf[:, :cols, :],
                              in_ap=table[l][w // 2][:, :],
                              idxs_ap=idx_t[:, ic:ic + cols * 8],
                              num_idxs=cols * P,
                              num_idxs_reg=cols * P,
                              elem_size=2 * HID,
                              queue_num=next_q(),
                          )
                      sgch = sum(int(st.cap[b].sum()) // P for b in sg)
                      if sgch and not no_compute:
                          bsl = bpool.tile([128, st.SGMAX * 128], fp8, tag="B")
                          nc.sync.dma_start(
                              out=bsl[:, :sgch * 128],
                              in_=I["btiles"][:, t * 128:(t + sgch) * 128])
                      tsg = 0
                      for b in sg:
                          nchunks = 0 if no_compute else int(st.cap[b].sum()) // P
                          acc = pacc.tile([128, HID], f32, tag="acc")
                          # inject Cb/dinv[dst] (outer product, K=1), then the
                          # self-loop h'[dst] (identity matmul), then the edges
                          nc.tensor.matmul(acc[:],
                                           lhsT=C["invd_flat"][0:1, b * 128:(b + 1) * 128],
                                           rhs=cRep[0:1, :],
                                           start=True, stop=False)
                          nc.tensor.matmul(acc[:], lhsT=C["identb"][:],
                                           rhs=h_tiles[b][:],
                                           start=False, stop=(nchunks == 0))
                          done = 0
                          for w in range(NWIN if not no_compute else 0):
                              kk = int(st.cap[b, w]) // P
                              for j in range(kk):
                                  nc.tensor.matmul(
                                      acc[:], lhsT=bsl[:, tsg * 128:(tsg + 1) * 128],
                                      rhs=gt[w][:, st.coloff[(b, w)] + j,
                                                (w % 2) * HID:(w % 2 + 1) * HID],
                                      start=False, stop=(done == nchunks - 1))
                                  done += 1
                                  t += 1
                                  tsg += 1
                          nb = P if b < NB - 1 else LB
                          if l == 0:
                              # y0 (f32) -> transpose -> h1' = dinv*(y0 @ W1)
                              yb = ypool.tile([128, HID], f32, tag="y")
                              nc.scalar.activation(out=yb[:], in_=acc[:], func=Act.Relu,
                                                   scale=dinv_t[:, b:b + 1])
                              pt = ptrans.tile([128, 128], f32, tag="pt")
                              nc.tensor.transpose(pt[:HID, :], yb[:], C["ident"][:])
                              yTs = xts.tile([128, 128], f32, tag="xT")
                              nc.scalar.activation(out=yTs[:HID, :], in_=pt[:HID, :],
                                                   func=Act.Copy)
                              hp = phw.tile([128, HID], f32, tag="hp")
                              nc.tensor.matmul(hp[:], lhsT=yTs[:HID, :], rhs=C["w1"][:],
                                               start=True, stop=True)
                              h1b = hpool.tile([128, HID], bf16, tag="h")
                              nc.scalar.activation(out=h1b[:], in_=hp[:], func=Act.Copy,
                                                   scale=dinv_t[:, b:b + 1])
                              bounce_write(1, b, nb, h1b)
                              h1_tiles.append(h1b)
                          else:
                              # y1 (bf16) -> inline mean-pool accumulation
                              yb = y1pool.tile([128, HID], bf16, tag="y1")
                              nc.scalar.activation(out=yb[:], in_=acc[:], func=Act.Relu,
                                                   scale=dinv_t[:, b:b + 1])
                              nc.tensor.matmul(pp[:G, :],
                                               lhsT=C["pool_oh"][:nb, b * 128:b * 128 + G],
                                               rhs=yb[:nb, :],
                                               start=(b == 0), stop=(b == NB - 1))
                  if l == 0:
                      h_tiles = h1_tiles
                      allgather_half(1, 0)
                      allgather_half(1, 1)

              # ---- mean pool partials -> AllReduce -> head
              pooled = tmp.tile([128, HID], f32, tag="pl")
              nc.scalar.activation(out=pooled[:G, :], in_=pp[:G, :], func=Act.Copy)
              nc.sync.dma_start(out=ar_in[:], in_=pooled[:G, :])
              if fake_coll:
                  nc.sync.dma_start(out=ar_out[:], in_=ar_in[:])
              else:
                  nc.gpsimd.collective_compute(
                      "AllReduce", Alu.add,
                      replica_groups=[list(range(ncores))],
                      ins=[ar_in.opt()], outs=[ar_out.opt()],
                  )
              pooled2 = tmp.tile([128, HID], f32, tag="pl2")
              nc.sync.dma_start(out=pooled2[:G, :], in_=ar_out[:])
              nc.vector.tensor_scalar(out=pooled2[:G, :], in0=pooled2[:G, :],
                                      scalar1=C["invcnt"][:G, :], scalar2=None,
                                      op0=Alu.mult)

              # z = relu(Sf * (pooled @ Wf1) + Cf)
              pt = ptrans.tile([128, 128], f32, tag="pt")
              nc.tensor.transpose(pt[:HID, :G], pooled2[:G, :], C["ident"][:])
              pTs = xts.tile([128, 128], f32, tag="xT")
              nc.vector.tensor_copy(out=pTs[:HID, :G], in_=pt[:HID, :G])
              zp = phw.tile([128, HHID], f32, tag="hp")
              nc.tensor.matmul(zp[:G, :], lhsT=pTs[:HID, :G], rhs=C["wf1"][:],
                               start=True, stop=True)
              z = tmp.tile([128, HHID], f32, tag="z")
              nc.vector.tensor_tensor(out=z[:G, :], in0=zp[:G, :], in1=C["cfb"][:G, :], op=Alu.add)
              nc.vector.tensor_scalar(out=z[:G, :], in0=z[:G, :], scalar1=0.0,
                                      scalar2=None, op0=Alu.max)

              # logits = z @ Wf2 + bf2; out = log_softmax(logits)
              pt2 = ptrans.tile([128, 128], f32, tag="pt")
              nc.tensor.transpose(pt2[:HHID, :G], z[:G, :], C["ident"][:])
              zTs = xts.tile([128, 128], f32, tag="xT")
              nc.vector.tensor_copy(out=zTs[:HHID, :G], in_=pt2[:HHID, :G])
              lp = phw.tile([128, NCLASS], f32, tag="hp")
              nc.tensor.matmul(lp[:G, :], lhsT=zTs[:HHID, :G], rhs=C["wf2"][:],
                               start=True, stop=True)
              lg = tmp.tile([128, NCLASS], f32, tag="lg")
              nc.vector.tensor_tensor(out=lg[:G, :], in0=lp[:G, :], in1=C["bf2b"][:G, :], op=Alu.add)
              mx = tmp.tile([128, 1], f32, tag="mx")
              nc.vector.reduce_max(mx[:G, :], lg[:G, :], axis=mybir.AxisListType.X)
              nc.vector.tensor_scalar(out=lg[:G, :], in0=lg[:G, :], scalar1=mx[:G, :],
                                      scalar2=None, op0=Alu.subtract)
              ex = tmp.tile([128, NCLASS], f32, tag="ex")
              nc.scalar.activation(out=ex[:G, :], in_=lg[:G, :], func=Act.Exp)
              sm = tmp.tile([128, 1], f32, tag="sm")
              nc.vector.reduce_sum(sm[:G, :], ex[:G, :], axis=mybir.AxisListType.X)
              lsm = tmp.tile([128, 1], f32, tag="ls")
              nc.scalar.activation(out=lsm[:G, :], in_=sm[:G, :], func=Act.Ln)
              nc.vector.tensor_scalar(out=lg[:G, :], in0=lg[:G, :], scalar1=lsm[:G, :],
                                      scalar2=None, op0=Alu.subtract)
              nc.sync.dma_start(out=out_d[:], in_=lg[:G, :])

    nc.compile()
    return nc




# ------------------------------------------------------------ PJRT runner --

class SpmdRunner:
    """Run the compiled 8-core Bass module via PJRT (axon), mirroring
    concourse.bass2jax.run_bass_via_pjrt but keeping the jitted callable."""

    def __init__(self, nc, n_cores):
        import jax
        from jax.sharding import Mesh, PartitionSpec
        from jax.experimental.shard_map import shard_map
        from concourse import bass2jax, mybir as _mb
        from concourse.bass2jax import _bass_exec_p, install_neuronx_cc_hook
        install_neuronx_cc_hook()
        self.jax = jax
        self.nc = nc
        self.n_cores = n_cores
        partition_name = nc.partition_id_tensor.name if nc.partition_id_tensor else None
        in_names, out_names, out_avals, zero_outs = [], [], [], []
        for alloc in nc.m.functions[0].allocations:
            if not isinstance(alloc, _mb.MemoryLocationSet):
                continue
            name = alloc.memorylocations[0].name
            if alloc.kind == "ExternalInput":
                if name != partition_name:
                    in_names.append(name)
            elif alloc.kind == "ExternalOutput":
                shape = tuple(alloc.tensor_shape)
                dtype = _mb.dt.np(alloc.dtype)
                out_names.append(name)
                out_avals.append(jax.core.ShapedArray(shape, dtype))
                zero_outs.append(np.zeros(shape, dtype))
        self.in_names, self.out_names = in_names, out_names
        self.out_avals, self.zero_outs = out_avals, zero_outs
        n_params, n_outs = len(in_names), len(out_avals)
        self.n_params = n_params
        all_in_names = in_names + out_names + ([partition_name] if partition_name else [])

        def _body(*args):
            operands = list(args)
            if partition_name is not None:
                operands.append(bass2jax.partition_id_tensor())
            return tuple(_bass_exec_p.bind(
                *operands, out_avals=tuple(out_avals), in_names=tuple(all_in_names),
                out_names=tuple(out_names), lowering_input_output_aliases=(),
                sim_require_finite=True, sim_require_nnan=True, nc=nc))

        devices = jax.devices()[:n_cores]
        assert len(devices) == n_cores
        mesh = Mesh(np.asarray(devices), ("core",))
        self._sharding = jax.sharding.NamedSharding(mesh, PartitionSpec("core"))
        in_specs = (PartitionSpec("core"),) * (n_params + n_outs)
        out_specs = (PartitionSpec("core"),) * len(out_names)
        self._fn = jax.jit(
            shard_map(_body, mesh=mesh, in_specs=in_specs,
                      out_specs=out_specs, check_rep=False),
            keep_unused=True)

    def prepare(self, in_maps):
        per_core = [[np.asarray(m[name]) for name in self.in_names] for m in in_maps]
        concat_in = [np.concatenate([per_core[c][i] for c in range(self.n_cores)], axis=0)
                     for i in range(self.n_params)]
        concat_zeros = [np.zeros((self.n_cores * z.shape[0], *z.shape[1:]), z.dtype)
                        for z in self.zero_outs]
        return concat_in + concat_zeros

    def run(self, in_maps):
        out_arrs = self._fn(*self.prepare(in_maps))
        self.jax.block_until_ready(out_arrs)
        return self._split(out_arrs)

    def _split(self, out_arrs):
        return [{name: np.asarray(out_arrs[i]).reshape(self.n_cores, *self.out_avals[i].shape)[c]
                 for i, name in enumerate(self.out_names)}
                for c in range(self.n_cores)]

    def time(self, in_maps, iters=8):
        import time as _t
        args = self.prepare(in_maps)
        dargs = [self.jax.device_put(a, self._sharding) for a in args]
        out = self._fn(*dargs)
        self.jax.block_until_ready(out)
        results = self._split(out)
        times = []
        for _ in range(iters):
            t0 = _t.perf_counter()
            o = self._fn(*dargs)
            self.jax.block_until_ready(o)
            times.append(_t.perf_counter() - t0)
        return results, times


# ------------------------------------------------------------------- driver --

_CACHE = {}


def _get_runner(st, dims, ncores):
    nc = build_nc(st, dims, ncores)
    return SpmdRunner(nc, ncores)


def kernel(**inputs):
    x = np.asarray(inputs["x"], np.float32)
    edge_index = np.asarray(inputs["edge_index"])
    batch = np.asarray(inputs["batch"])
    edge_attr = np.asarray(inputs["edge_attr"], np.float32)
    G = 128
    params = {k: np.asarray(v) for k, v in inputs.items()
              if k not in ("x", "edge_index", "batch", "edge_attr", "pos")}
    params["cnt_G"] = G
    ncores = 8

    st, in_maps, dims = _host_prep(x, edge_index, batch, edge_attr, params, ncores)

    key = ("k", x.shape, edge_index.shape, st.SID, st.CT, st.GMAX,
           tuple(tuple(s) for s in st.sgs))
    if key not in _CACHE:
        _CACHE[key] = _get_runner(st, dims, ncores)
    runner = _CACHE[key]
    _LAST.update(st=st, dims=dims, ncores=ncores, in_maps=in_maps, runner=runner)
    results = runner.run(in_maps)
    return results[0]["out"]


_LAST = {}


def estimate_exec_ns(reps=16, iters=10):
    """Per-execution device time via wall-clock delta between a 1-rep NEFF and
    an in-NEFF `reps`-times-repeated body (cancels the axon dispatch floor).
    Median-based: the axon tunnel has heavy-tailed per-call jitter."""
    import time as _t
    import jax
    st, dims, ncores = _LAST["st"], _LAST["dims"], _LAST["ncores"]
    in_maps, r1 = _LAST["in_maps"], _LAST["runner"]
    rR = SpmdRunner(build_nc(st, dims, ncores, reps=reps), ncores)
    a1 = [jax.device_put(a, r1._sharding) for a in r1.prepare(in_maps)]
    aR = [jax.device_put(a, rR._sharding) for a in rR.prepare(in_maps)]
    jax.block_until_ready(r1._fn(*a1)); jax.block_until_ready(rR._fn(*aR))
    t1s, tRs = [], []
    for _ in range(iters):
        t0 = _t.perf_counter(); jax.block_until_ready(r1._fn(*a1)); t1s.append(_t.perf_counter() - t0)
        t0 = _t.perf_counter(); jax.block_until_ready(rR._fn(*aR)); tRs.append(_t.perf_counter() - t0)
    t1s, tRs = sorted(t1s), sorted(tRs)
    per = (tRs[len(tRs) // 2] - t1s[len(t1s) // 2]) / (reps - 1)
    return per * 1e9


# revision 15
# speedup vs baseline: 1.5672x; 1.1192x over previous
"""Trainium2 Bass kernel for nn_BaselineGCN (2-layer GCN + BN + mean-pool + MLP head).

Strategy (8 NeuronCores):
 - Nodes sharded contiguously across cores; each core owns the in-edges of its
   node shard (dst-sharding, per the graph-partitioning hint).
 - gcn_norm factorized: deg/dinv computed host-side (index/weight preprocessing);
   per-edge message = w_e * h'[src] with h' = dinv * (h @ W); per-dst scale by
   dinv[dst] folded into the ACT-engine epilogue.
 - The per-edge gather h'[src] runs on-device via SWDGE dma_gather (256B rows)
   from an AllGather'ed replica of h' in each core's DRAM, round-robined over 4
   SWDGE queues.
 - segment_sum becomes TensorE matmuls: per 128-edge chunk, a host-precomputed
   one-hot-times-weight fp8 matrix B[e, dst_local] is loaded as lhsT and PE
   accumulates B.T @ gathered_rows into the dst-block's PSUM tile.  The BN bias
   term (Cb) and the self-loop (+h'[dst]) are injected into the same PSUM
   accumulation via a K=1 outer-product matmul and a bf16 identity matmul, so
   the whole per-block epilogue is ONE ScalarE op: y = Relu(acc * dinv[dst]).
 - VectorE is kept almost empty: the v1 kernel was DVE+GPSIMD co-bottlenecked
   (DVE ~70% busy on epilogue/cast chains created backpressure that limited
   SWDGE descriptor-gen queue concurrency to ~2 of 4).
 - Graph mean-pool is a host-precomputed fp8 one-hot matmul accumulated inline
   with layer-1 epilogues; partials summed with an AllReduce; the tiny MLP
   head + log_softmax run on every core.
"""
import sys
import time

sys.path.insert(0, "/opt/trn_rl_repo")

import numpy as np

P = 128          # partitions / block size
NWIN = 4         # gather index windows (int16 range)
MAXCALL = 1024   # max indices per dma_gather (SWDGE ring capacity)
DMA_SCRATCH = 16384  # SWDGE ring carveout bytes (ring = this // 16 descs)
NQUEUES = 4      # SWDGE queues to round-robin
GBUFS = 16       # gather tile lookahead
BBUFS = 4        # B-slab lookahead (per-supergroup fp8 slabs)
PACCB = 4        # PSUM accumulator banks


# ---------------------------------------------------------------- host prep --

def _ceil(a, b):
    return -(-a // b)


class GCNStructure:
    """Graph partitioning + stream layout. Capacities are maxed across cores so
    the single SPMD program fits every core's data."""

    def __init__(self, src, dst, ew, batch, N, G, ncores):
        self.N, self.G, self.C = N, G, ncores
        NSH = N // ncores
        NB = _ceil(NSH, P)
        WS = _ceil(N // 2, NWIN // 2)  # window size in PAIRED rows
        assert N % ncores == 0
        assert WS <= 32767, "gather window exceeds int16"
        self.NSH, self.NB, self.WS = NSH, NB, WS
        self.LB = NSH - (NB - 1) * P  # rows in last block

        core = dst // NSH
        blk = (dst % NSH) // P
        # cell = (row-window, parity): row = src//2 packs 2 nodes per 256B
        win = ((src // 2) // WS) * 2 + (src & 1)
        key = (core * NB + blk) * NWIN + win
        # secondary sort by src: ascending gather addresses within each
        # (core, block, cell) group measurably improve DRAM read locality
        order = np.lexsort((src, key))
        self.src_s, self.dst_s, self.ew_s = src[order], dst[order], ew[order]
        counts = np.bincount(key, minlength=ncores * NB * NWIN).reshape(ncores, NB, NWIN)
        self.counts = counts
        cap = counts.max(axis=0)
        cap = _ceil(np.maximum(cap, 0), P) * P  # per (b, w), 0 stays 0
        self.cap = cap  # [NB, NWIN]

        # supergroups: consecutive blocks such that per-window call <= MAXCALL
        self.sgs = []
        cur = [0]
        for b in range(1, NB):
            trial = cur + [b]
            if all(cap[trial, w].sum() <= MAXCALL for w in range(NWIN)):
                cur = trial
            else:
                self.sgs.append(cur)
                cur = [b]
        self.sgs.append(cur)

        # layout: gather calls in (sg, w) order; chunks in (sg, b, w, j) order
        self.gcols = {}    # (sgi, w) -> columns in that call's tile
        self.icol = {}     # (sgi, w) -> start col (units of 16-idx) in idx stream
        self.coloff = {}   # (b, w) -> column offset inside its call tile
        sid = 0
        for sgi, sg in enumerate(self.sgs):
            for w in range(NWIN):
                cols = int(cap[sg, w].sum()) // P
                self.gcols[(sgi, w)] = cols
                self.icol[(sgi, w)] = sid
                off = 0
                for b in sg:
                    self.coloff[(b, w)] = off
                    off += int(cap[b, w]) // P
                sid += cols * 8  # n/16 = cols*128/16
        self.SID = max(sid, 8)
        self.CT = max(int(cap.sum()) // P, 1)
        self.GMAX = max(max(self.gcols.values(), default=1), 1)
        self.SGMAX = max(sum(int(cap[b].sum()) // P for b in sg) for sg in self.sgs)

        # per-core edge offsets into the sorted arrays, per (b, w)
        cum = np.zeros(ncores * NB * NWIN + 1, np.int64)
        np.cumsum(counts.reshape(-1), out=cum[1:])
        self.grp_start = cum  # index by (c*NB+b)*NWIN+w

        # batch / counts for pooling
        self.cnt = np.bincount(batch, minlength=G).astype(np.float32)
        self.inv_cnt = (1.0 / np.maximum(self.cnt, 1.0)).astype(np.float32)

    def core_streams(self, c, dinv):
        """Build per-core device streams: idx [128, SID] i16,
        dstloc/val [128, CT] f32, dinvb [128, NB] f32."""
        NB, WS, NSH = self.NB, self.WS, self.NSH
        idx_cols = np.zeros((128, self.SID), np.int16)
        dstloc = np.zeros((128, self.CT), np.float32)
        val = np.zeros((128, self.CT), np.float32)

        t = 0
        for sgi, sg in enumerate(self.sgs):
            # gather stream: (w, b) order
            for w in range(NWIN):
                col = self.icol[(sgi, w)]
                parts = []
                for b in sg:
                    g0 = self.grp_start[(c * NB + b) * NWIN + w]
                    g1 = self.grp_start[(c * NB + b) * NWIN + w + 1]
                    loc = (self.src_s[g0:g1] // 2 - (w // 2) * WS).astype(np.int16)
                    pad = int(self.cap[b, w]) - (g1 - g0)
                    parts.append(np.concatenate([loc, np.zeros(pad, np.int16)]))
                if parts:
                    flat = np.concatenate(parts)
                    if flat.size:
                        wrapped = np.tile(flat.reshape(-1, 16).T, (8, 1))
                        idx_cols[:, col:col + flat.size // 16] = wrapped
            # value/dst streams: (b, w, chunk) order
            for b in sg:
                for w in range(NWIN):
                    g0 = self.grp_start[(c * NB + b) * NWIN + w]
                    g1 = self.grp_start[(c * NB + b) * NWIN + w + 1]
                    n = g1 - g0
                    capbw = int(self.cap[b, w])
                    if capbw == 0:
                        continue
                    dl = np.zeros(capbw, np.float32)
                    vv = np.zeros(capbw, np.float32)
                    dl[:n] = (self.dst_s[g0:g1] - (c * NSH + b * P)).astype(np.float32)
                    vv[:n] = self.ew_s[g0:g1]
                    k = capbw // P
                    dstloc[:, t:t + k] = dl.reshape(k, P).T
                    val[:, t:t + k] = vv.reshape(k, P).T
                    t += k

        dinvb = np.zeros((128, NB), np.float32)
        sh_dinv = dinv[c * NSH:(c + 1) * NSH]
        for b in range(NB):
            nb = P if b < NB - 1 else self.LB
            dinvb[:nb, b] = sh_dinv[b * P:b * P + nb]
        return idx_cols, dstloc, val, dinvb

    def core_pool_oh(self, c, batch):
        """fp8 one-hot pooling matrix [128, NB*128]: [p, b*128+g] = (batch==g)."""
        import ml_dtypes
        FP8 = np.dtype(ml_dtypes.float8_e4m3)
        NB, NSH = self.NB, self.NSH
        oh = np.zeros((128, NB * 128), FP8)
        sh = batch[c * NSH:(c + 1) * NSH]
        for b in range(NB):
            nb = P if b < NB - 1 else self.LB
            g = sh[b * P:b * P + nb].astype(np.int64)
            oh[np.arange(nb), b * 128 + g] = 1.0
        return oh


def _host_prep(x, edge_index, batch, edge_attr, params, ncores):
    """All index-based preprocessing + BN folding. Returns (struct, in_maps)."""
    N, INDIM = x.shape
    G = int(params["cnt_G"])
    EPS = 1e-5

    src = np.asarray(edge_index[0], np.int64)
    dst = np.asarray(edge_index[1], np.int64)
    ew = np.asarray(edge_attr, np.float32)
    batch = np.asarray(batch, np.int64)

    deg = np.bincount(dst, weights=ew.astype(np.float64), minlength=N) + 1.0
    dinv = (1.0 / np.sqrt(deg)).astype(np.float32)

    st = GCNStructure(src, dst, ew, batch, N, G, ncores)

    def bnfold(g, be, m, v, bias):
        s = (g / np.sqrt(v + EPS)).astype(np.float32)
        cc = ((bias - m) * s + be).astype(np.float32)
        return s, cc

    S0, C0 = bnfold(params["g0"], params["be0"], params["m0"], params["v0"], params["b0"])
    S1, C1 = bnfold(params["g1"], params["be1"], params["m1"], params["v1"], params["b1"])
    Sf, Cf = bnfold(params["gf"], params["bef"], params["mf"], params["vf"], params["bf1"])

    # BN scale folded into the weight matrices (linear before the bias add)
    W0s = (np.asarray(params["W0"], np.float32) * S0[None, :]).astype(np.float32)
    W1s = (np.asarray(params["W1"], np.float32) * S1[None, :]).astype(np.float32)
    Wf1s = (np.asarray(params["Wf1"], np.float32) * Sf[None, :]).astype(np.float32)

    HID = params["W0"].shape[1]
    HHID = params["Wf1"].shape[1]
    NCLASS = params["Wf2"].shape[1]

    ident = np.eye(128, dtype=np.float32)

    NSH, NB = st.NSH, st.NB

    import ml_dtypes
    FP8 = np.dtype(ml_dtypes.float8_e4m3)
    BF16 = np.dtype(ml_dtypes.bfloat16)

    xpadT = np.zeros((ncores, INDIM, NB * P), BF16)
    xv = np.asarray(x, np.float32)
    for c in range(ncores):
        xpadT[c, :, :NSH] = xv[c * NSH:(c + 1) * NSH].T.astype(BF16)

    in_maps = []
    for c in range(ncores):
        idx_cols, dstloc, val, dinvb = st.core_streams(c, dinv)
        pool_oh = st.core_pool_oh(c, batch)
        # precomputed one-hot-times-weight B tiles, chunk-major [128, CT*128]
        bt3 = np.zeros((128, st.CT, 128), FP8)
        np.put_along_axis(bt3, dstloc.astype(np.int64)[:, :, None],
                          val.astype(FP8)[:, :, None], axis=2)
        # invd_flat[0, b*128+j] = 1/dinv at node (c, b, j); 0 for pad rows.
        # (single partition: Ldweights requires lhsT at partition 0)
        invd_flat = np.zeros((1, NB * 128), np.float32)
        sh_dinv = dinv[c * NSH:(c + 1) * NSH]
        for b in range(NB):
            nb = P if b < NB - 1 else st.LB
            invd_flat[0, b * 128:b * 128 + nb] = 1.0 / sh_dinv[b * P:b * P + nb]
        # cRep[b, :] = C (bias row replicated so lhsT/rhs base partitions match)
        cRep0 = np.tile(C0[None, :], (128, 1)).astype(BF16)
        cRep1 = np.tile(C1[None, :], (128, 1)).astype(BF16)
        in_maps.append(dict(
            xshT=xpadT[c],
            idxs=idx_cols,
            btiles=bt3.reshape(128, st.CT * 128),
            pool_oh=pool_oh, dinvb=dinvb,
            invd_flat=invd_flat.astype(BF16),
            cRep0=cRep0, cRep1=cRep1,
            w0=W0s.astype(BF16), w1=W1s, wf1=Wf1s,
            wf2=np.asarray(params["Wf2"], np.float32),
            cfb=np.tile(Cf[None, :], (128, 1)),
            bf2b=np.tile(np.asarray(params["bf2"], np.float32)[None, :], (128, 1)),
            invcnt=st.inv_cnt[:, None].copy(),
            ident=ident, identb=ident.astype(BF16),
        ))
    dims = dict(INDIM=INDIM, HID=HID, HHID=HHID, NCLASS=NCLASS)
    return st, in_maps, dims


# ------------------------------------------------------------- bass program --

def build_nc(st, dims, ncores, reps=1, fake_coll=False, no_gather=False, no_compute=False):
    from concourse import bass, mybir, bacc, tile

    INDIM, HID, HHID, NCLASS = dims["INDIM"], dims["HID"], dims["HHID"], dims["NCLASS"]
    N, G, NB, NSH, WS, LB = st.N, st.G, st.NB, st.NSH, st.WS, st.LB
    f32 = mybir.dt.float32
    bf16 = mybir.dt.bfloat16
    fp8 = mybir.dt.float8e4
    Alu = mybir.AluOpType
    Act = mybir.ActivationFunctionType

    nc = bacc.Bacc("TRN2", target_bir_lowering=False, debug=False,
                   enable_asserts=True, num_devices=ncores,
                   num_swdge_queues=NQUEUES,
                   dynamic_dma_scratch_size=DMA_SCRATCH)

    I = {}
    def inp(name, shape, dt=f32):
        I[name] = nc.dram_tensor(name, shape, dt, kind="ExternalInput")
        return I[name]

    inp("xshT", [INDIM, NB * P], bf16)
    inp("idxs", [128, st.SID], mybir.dt.int16)
    inp("btiles", [128, st.CT * 128], mybir.dt.float8e4)
    inp("pool_oh", [128, NB * 128], mybir.dt.float8e4)
    inp("dinvb", [128, NB])
    inp("invd_flat", [1, NB * 128], bf16)
    inp("cRep0", [128, HID], bf16); inp("cRep1", [128, HID], bf16)
    inp("w0", [INDIM, HID], bf16); inp("w1", [HID, HID])
    inp("wf1", [HID, HHID]); inp("wf2", [HHID, NCLASS])
    inp("cfb", [128, HHID])
    inp("bf2b", [128, NCLASS])
    inp("invcnt", [128, 1])
    inp("ident", [128, 128]); inp("identb", [128, 128], bf16)
    out_d = nc.dram_tensor("out", [G, NCLASS], f32, kind="ExternalOutput")

    qctr = [0]
    def next_q():
        q = qctr[0] % NQUEUES
        qctr[0] += 1
        return q

    with tile.TileContext(nc) as tc:
        import contextlib
        with contextlib.ExitStack() as ctx:
            const = ctx.enter_context(tc.tile_pool(name="const", bufs=1))
            stream = ctx.enter_context(tc.tile_pool(name="stream", bufs=1))
            xio = ctx.enter_context(tc.tile_pool(name="xio", bufs=3))
            xts = ctx.enter_context(tc.tile_pool(name="xts", bufs=3))
            hpool = ctx.enter_context(tc.tile_pool(name="hpool", bufs=NB))
            ypool = ctx.enter_context(tc.tile_pool(name="ypool", bufs=NB))
            y1pool = ctx.enter_context(tc.tile_pool(name="y1pool", bufs=NB))
            gpool = ctx.enter_context(tc.tile_pool(name="gpool", bufs=GBUFS))
            bpool = ctx.enter_context(tc.tile_pool(name="bpool", bufs=BBUFS))
            tmp = ctx.enter_context(tc.tile_pool(name="tmp", bufs=6))
            ptrans = ctx.enter_context(tc.tile_pool(name="ptrans", bufs=1, space="PSUM"))
            phw = ctx.enter_context(tc.tile_pool(name="phw", bufs=2, space="PSUM"))
            pacc = ctx.enter_context(tc.tile_pool(name="pacc", bufs=PACCB, space="PSUM"))
            ppool = ctx.enter_context(tc.tile_pool(name="ppool", bufs=1, space="PSUM"))
            dram = ctx.enter_context(tc.tile_pool(name="dram", bufs=1, space="DRAM"))

            # ---- constants into SBUF
            C = {}
            cdts = dict(w0=bf16, invd_flat=bf16, cRep0=bf16, cRep1=bf16, identb=bf16,
                        pool_oh=fp8)
            for nm in ["w0", "w1", "wf1", "wf2", "cfb", "bf2b", "invcnt",
                       "ident", "identb", "invd_flat", "cRep0", "cRep1",
                       "pool_oh"]:
                shape = list(I[nm].shape)
                tile_ = const.tile(shape, cdts.get(nm, f32), tag=nm)
                nc.sync.dma_start(out=tile_[:], in_=I[nm][:])
                C[nm] = tile_
            idx_t = stream.tile([128, st.SID], mybir.dt.int16, tag="idx")
            nc.sync.dma_start(out=idx_t[:], in_=I["idxs"][:])
            dinv_t = stream.tile([128, NB], f32, tag="dnv")
            nc.sync.dma_start(out=dinv_t[:], in_=I["dinvb"][:])

            shspace = "Shared" if ncores > 4 else "Local"
            bounce0 = dram.tile([NSH, HID], bf16, tag="bnc0")
            bounce1 = dram.tile([NSH, HID], bf16, tag="bnc1")
            bounce = [bounce0, bounce1]
            ar_in = dram.tile([G, HID], f32, tag="arin")

            # ---- phase A: h0' = dinv * (x @ W0), shard -> AllGather table0
            for _rep in range(reps):
              # per-rep Shared tables: a Shared DRAM tile allows only one writer
              table0 = dram.tile([N // 2, 2 * HID], bf16, tag=f"tab0_{_rep}", addr_space=shspace)
              table1 = dram.tile([N // 2, 2 * HID], bf16, tag=f"tab1_{_rep}", addr_space=shspace)
              table = [table0, table1]
              ar_out = dram.tile([G, HID], f32, tag=f"arout_{_rep}", addr_space=shspace)
              h_tiles = []
              for b in range(NB):
                  nb = P if b < NB - 1 else LB
                  xt = xio.tile([128, 128], bf16, tag="xt")
                  nc.sync.dma_start(out=xt[:INDIM, :], in_=I["xshT"][:, b * P:(b + 1) * P])
                  hp = phw.tile([128, HID], f32, tag="hp")
                  nc.tensor.matmul(hp[:], lhsT=xt[:INDIM, :], rhs=C["w0"][:],
                                   start=True, stop=True)
                  hb16 = hpool.tile([128, HID], bf16, tag="h")
                  nc.scalar.activation(out=hb16[:], in_=hp[:], func=Act.Copy,
                                       scale=dinv_t[:, b:b + 1])
                  nc.sync.dma_start(out=bounce[0][b * P:b * P + nb, :], in_=hb16[:nb, :])
                  h_tiles.append(hb16)

              if fake_coll:
                  nc.sync.dma_start(out=table[0][0:NSH // 2, :], in_=bounce[0][:])
              else:
                  nc.gpsimd.collective_compute(
                      "AllGather", Alu.bypass,
                      replica_groups=[list(range(ncores))],
                      ins=[bounce[0].opt()], outs=[table[0].opt()],
                  )

              # ---- GCN layers
              pp = ppool.tile([128, HID], f32, tag="pool")
              for l in range(2):
                  cRep = C["cRep0"] if l == 0 else C["cRep1"]
                  h1_tiles = []
                  t = 0
                  for sgi, sg in enumerate(st.sgs):
                      gt = {}
                      for w in range(NWIN):
                          cols = st.gcols[(sgi, w)]
                          if cols == 0:
                              continue
                          gbf = gpool.tile([128, st.GMAX, 2 * HID], bf16, tag="g")
                          ic = st.icol[(sgi, w)]
                          gt[w] = gbf
                          if no_gather:
                              continue
                          nc.gpsimd.dma_gather(
                              out_ap=gbf[:, :cols, :],
                              in_ap=table[l][(w // 2) * WS:min((w // 2 + 1) * WS, N // 2), :],
                              idxs_ap=idx_t[:, ic:ic + cols * 8],
                              num_idxs=cols * P,
                              num_idxs_reg=cols * P,
                              elem_size=2 * HID,
                              queue_num=next_q(),
                          )
                      sgch = sum(int(st.cap[b].sum()) // P for b in sg)
                      if sgch and not no_compute:
                          bsl = bpool.tile([128, st.SGMAX * 128], fp8, tag="B")
                          nc.sync.dma_start(
                              out=bsl[:, :sgch * 128],
                              in_=I["btiles"][:, t * 128:(t + sgch) * 128])
                      tsg = 0
                      for b in sg:
                          nchunks = 0 if no_compute else int(st.cap[b].sum()) // P
                          acc = pacc.tile([128, HID], f32, tag="acc")
                          # inject Cb/dinv[dst] (outer product, K=1), then the
                          # self-loop h'[dst] (identity matmul), then the edges
                          nc.tensor.matmul(acc[:],
                                           lhsT=C["invd_flat"][0:1, b * 128:(b + 1) * 128],
                                           rhs=cRep[0:1, :],
                                           start=True, stop=False)
                          nc.tensor.matmul(acc[:], lhsT=C["identb"][:],
                                           rhs=h_tiles[b][:],
                                           start=False, stop=(nchunks == 0))
                          done = 0
                          for w in range(NWIN if not no_compute else 0):
                              kk = int(st.cap[b, w]) // P
                              for j in range(kk):
                                  nc.tensor.matmul(
                                      acc[:], lhsT=bsl[:, tsg * 128:(tsg + 1) * 128],
                                      rhs=gt[w][:, st.coloff[(b, w)] + j,
                                                (w % 2) * HID:(w % 2 + 1) * HID],
                                      start=False, stop=(done == nchunks - 1))
                                  done += 1
                                  t += 1
                                  tsg += 1
                          nb = P if b < NB - 1 else LB
                          if l == 0:
                              # y0 (f32) -> transpose -> h1' = dinv*(y0 @ W1)
                              yb = ypool.tile([128, HID], f32, tag="y")
                              nc.scalar.activation(out=yb[:], in_=acc[:], func=Act.Relu,
                                                   scale=dinv_t[:, b:b + 1])
                              pt = ptrans.tile([128, 128], f32, tag="pt")
                              nc.tensor.transpose(pt[:HID, :], yb[:], C["ident"][:])
                              yTs = xts.tile([128, 128], f32, tag="xT")
                              nc.scalar.activation(out=yTs[:HID, :], in_=pt[:HID, :],
                                                   func=Act.Copy)
                              hp = phw.tile([128, HID], f32, tag="hp")
                              nc.tensor.matmul(hp[:], lhsT=yTs[:HID, :], rhs=C["w1"][:],
                                               start=True, stop=True)
                              h1b = hpool.tile([128, HID], bf16, tag="h")
                              nc.scalar.activation(out=h1b[:], in_=hp[:], func=Act.Copy,
                                                   scale=dinv_t[:, b:b + 1])
                              nc.sync.dma_start(out=bounce[1][b * P:b * P + nb, :],
                                                in_=h1b[:nb, :])
                              h1_tiles.append(h1b)
                          else:
                              # y1 (bf16) -> inline mean-pool accumulation
                              yb = y1pool.tile([128, HID], bf16, tag="y1")
                              nc.scalar.activation(out=yb[:], in_=acc[:], func=Act.Relu,
                                                   scale=dinv_t[:, b:b + 1])
                              nc.tensor.matmul(pp[:G, :],
                                               lhsT=C["pool_oh"][:nb, b * 128:b * 128 + G],
                                               rhs=yb[:nb, :],
                                               start=(b == 0), stop=(b == NB - 1))
                  if l == 0:
                      h_tiles = h1_tiles
                      if fake_coll:
                          nc.sync.dma_start(out=table[1][0:NSH // 2, :], in_=bounce[1][:])
                      else:
                          nc.gpsimd.collective_compute(
                              "AllGather", Alu.bypass,
                              replica_groups=[list(range(ncores))],
                              ins=[bounce[1].opt()], outs=[table[1].opt()],
                          )

              # ---- mean pool partials -> AllReduce -> head
              pooled = tmp.tile([128, HID], f32, tag="pl")
              nc.scalar.activation(out=pooled[:G, :], in_=pp[:G, :], func=Act.Copy)
              nc.sync.dma_start(out=ar_in[:], in_=pooled[:G, :])
              if fake_coll:
                  nc.sync.dma_start(out=ar_out[:], in_=ar_in[:])
              else:
                  nc.gpsimd.collective_compute(
                      "AllReduce", Alu.add,
                      replica_groups=[list(range(ncores))],
                      ins=[ar_in.opt()], outs=[ar_out.opt()],
                  )
              pooled2 = tmp.tile([128, HID], f32, tag="pl2")
              nc.sync.dma_start(out=pooled2[:G, :], in_=ar_out[:])
              nc.vector.tensor_scalar(out=pooled2[:G, :], in0=pooled2[:G, :],
                                      scalar1=C["invcnt"][:G, :], scalar2=None,
                                      op0=Alu.mult)

              # z = relu(Sf * (pooled @ Wf1) + Cf)
              pt = ptrans.tile([128, 128], f32, tag="pt")
              nc.tensor.transpose(pt[:HID, :G], pooled2[:G, :], C["ident"][:])
              pTs = xts.tile([128, 128], f32, tag="xT")
              nc.vector.tensor_copy(out=pTs[:HID, :G], in_=pt[:HID, :G])
              zp = phw.tile([128, HHID], f32, tag="hp")
              nc.tensor.matmul(zp[:G, :], lhsT=pTs[:HID, :G], rhs=C["wf1"][:],
                               start=True, stop=True)
              z = tmp.tile([128, HHID], f32, tag="z")
              nc.vector.tensor_tensor(out=z[:G, :], in0=zp[:G, :], in1=C["cfb"][:G, :], op=Alu.add)
              nc.vector.tensor_scalar(out=z[:G, :], in0=z[:G, :], scalar1=0.0,
                                      scalar2=None, op0=Alu.max)

              # logits = z @ Wf2 + bf2; out = log_softmax(logits)
              pt2 = ptrans.tile([128, 128], f32, tag="pt")
              nc.tensor.transpose(pt2[:HHID, :G], z[:G, :], C["ident"][:])
              zTs = xts.tile([128, 128], f32, tag="xT")
              nc.vector.tensor_copy(out=zTs[:HHID, :G], in_=pt2[:HHID, :G])
              lp = phw.tile([128, NCLASS], f32, tag="hp")
              nc.tensor.matmul(lp[:G, :], lhsT=zTs[:HHID, :G], rhs=C["wf2"][:],
                               start=True, stop=True)
              lg = tmp.tile([128, NCLASS], f32, tag="lg")
              nc.vector.tensor_tensor(out=lg[:G, :], in0=lp[:G, :], in1=C["bf2b"][:G, :], op=Alu.add)
              mx = tmp.tile([128, 1], f32, tag="mx")
              nc.vector.reduce_max(mx[:G, :], lg[:G, :], axis=mybir.AxisListType.X)
              nc.vector.tensor_scalar(out=lg[:G, :], in0=lg[:G, :], scalar1=mx[:G, :],
                                      scalar2=None, op0=Alu.subtract)
              ex = tmp.tile([128, NCLASS], f32, tag="ex")
              nc.scalar.activation(out=ex[:G, :], in_=lg[:G, :], func=Act.Exp)
              sm = tmp.tile([128, 1], f32, tag="sm")
              nc.vector.reduce_sum(sm[:G, :], ex[:G, :], axis=mybir.AxisListType.X)
              lsm = tmp.tile([128, 1], f32, tag="ls")
              nc.scalar.activation(out=lsm[:G, :], in_=sm[:G, :], func=Act.Ln)
              nc.vector.tensor_scalar(out=lg[:G, :], in0=lg[:G, :], scalar1=lsm[:G, :],
                                      scalar2=None, op0=Alu.subtract)
              nc.sync.dma_start(out=out_d[:], in_=lg[:G, :])

    nc.compile()
    return nc




# ------------------------------------------------------------ PJRT runner --

class SpmdRunner:
    """Run the compiled 8-core Bass module via PJRT (axon), mirroring
    concourse.bass2jax.run_bass_via_pjrt but keeping the jitted callable."""

    def __init__(self, nc, n_cores):
        import jax
        from jax.sharding import Mesh, PartitionSpec
        from jax.experimental.shard_map import shard_map
        from concourse import bass2jax, mybir as _mb
        from concourse.bass2jax import _bass_exec_p, install_neuronx_cc_hook
        install_neuronx_cc_hook()
        self.jax = jax
        self.nc = nc
        self.n_cores = n_cores
        partition_name = nc.partition_id_tensor.name if nc.partition_id_tensor else None
        in_names, out_names, out_avals, zero_outs = [], [], [], []
        for alloc in nc.m.functions[0].allocations:
            if not isinstance(alloc, _mb.MemoryLocationSet):
                continue
            name = alloc.memorylocations[0].name
            if alloc.kind == "ExternalInput":
                if name != partition_name:
                    in_names.append(name)
            elif alloc.kind == "ExternalOutput":
                shape = tuple(alloc.tensor_shape)
                dtype = _mb.dt.np(alloc.dtype)
                out_names.append(name)
                out_avals.append(jax.core.ShapedArray(shape, dtype))
                zero_outs.append(np.zeros(shape, dtype))
        self.in_names, self.out_names = in_names, out_names
        self.out_avals, self.zero_outs = out_avals, zero_outs
        n_params, n_outs = len(in_names), len(out_avals)
        self.n_params = n_params
        all_in_names = in_names + out_names + ([partition_name] if partition_name else [])

        def _body(*args):
            operands = list(args)
            if partition_name is not None:
                operands.append(bass2jax.partition_id_tensor())
            return tuple(_bass_exec_p.bind(
                *operands, out_avals=tuple(out_avals), in_names=tuple(all_in_names),
                out_names=tuple(out_names), lowering_input_output_aliases=(),
                sim_require_finite=True, sim_require_nnan=True, nc=nc))

        devices = jax.devices()[:n_cores]
        assert len(devices) == n_cores
        mesh = Mesh(np.asarray(devices), ("core",))
        self._sharding = jax.sharding.NamedSharding(mesh, PartitionSpec("core"))
        in_specs = (PartitionSpec("core"),) * (n_params + n_outs)
        out_specs = (PartitionSpec("core"),) * len(out_names)
        self._fn = jax.jit(
            shard_map(_body, mesh=mesh, in_specs=in_specs,
                      out_specs=out_specs, check_rep=False),
            keep_unused=True)

    def prepare(self, in_maps):
        per_core = [[np.asarray(m[name]) for name in self.in_names] for m in in_maps]
        concat_in = [np.concatenate([per_core[c][i] for c in range(self.n_cores)], axis=0)
                     for i in range(self.n_params)]
        concat_zeros = [np.zeros((self.n_cores * z.shape[0], *z.shape[1:]), z.dtype)
                        for z in self.zero_outs]
        return concat_in + concat_zeros

    def run(self, in_maps):
        out_arrs = self._fn(*self.prepare(in_maps))
        self.jax.block_until_ready(out_arrs)
        return self._split(out_arrs)

    def _split(self, out_arrs):
        return [{name: np.asarray(out_arrs[i]).reshape(self.n_cores, *self.out_avals[i].shape)[c]
                 for i, name in enumerate(self.out_names)}
                for c in range(self.n_cores)]

    def time(self, in_maps, iters=8):
        import time as _t
        args = self.prepare(in_maps)
        dargs = [self.jax.device_put(a, self._sharding) for a in args]
        out = self._fn(*dargs)
        self.jax.block_until_ready(out)
        results = self._split(out)
        times = []
        for _ in range(iters):
            t0 = _t.perf_counter()
            o = self._fn(*dargs)
            self.jax.block_until_ready(o)
            times.append(_t.perf_counter() - t0)
        return results, times


# ------------------------------------------------------------------- driver --

_CACHE = {}


def _get_runner(st, dims, ncores):
    nc = build_nc(st, dims, ncores)
    return SpmdRunner(nc, ncores)


def kernel(**inputs):
    x = np.asarray(inputs["x"], np.float32)
    edge_index = np.asarray(inputs["edge_index"])
    batch = np.asarray(inputs["batch"])
    edge_attr = np.asarray(inputs["edge_attr"], np.float32)
    G = 128
    params = {k: np.asarray(v) for k, v in inputs.items()
              if k not in ("x", "edge_index", "batch", "edge_attr", "pos")}
    params["cnt_G"] = G
    ncores = 8

    st, in_maps, dims = _host_prep(x, edge_index, batch, edge_attr, params, ncores)

    key = ("k", x.shape, edge_index.shape, st.SID, st.CT, st.GMAX,
           tuple(tuple(s) for s in st.sgs))
    if key not in _CACHE:
        _CACHE[key] = _get_runner(st, dims, ncores)
    runner = _CACHE[key]
    _LAST.update(st=st, dims=dims, ncores=ncores, in_maps=in_maps, runner=runner)
    results = runner.run(in_maps)
    return results[0]["out"]


_LAST = {}


def estimate_exec_ns(reps=16, iters=10):
    """Per-execution device time via wall-clock delta between a 1-rep NEFF and
    an in-NEFF `reps`-times-repeated body (cancels the axon dispatch floor).
    Median-based: the axon tunnel has heavy-tailed per-call jitter."""
    import time as _t
    import jax
    st, dims, ncores = _LAST["st"], _LAST["dims"], _LAST["ncores"]
    in_maps, r1 = _LAST["in_maps"], _LAST["runner"]
    rR = SpmdRunner(build_nc(st, dims, ncores, reps=reps), ncores)
    a1 = [jax.device_put(a, r1._sharding) for a in r1.prepare(in_maps)]
    aR = [jax.device_put(a, rR._sharding) for a in rR.prepare(in_maps)]
    jax.block_until_ready(r1._fn(*a1)); jax.block_until_ready(rR._fn(*aR))
    t1s, tRs = [], []
    for _ in range(iters):
        t0 = _t.perf_counter(); jax.block_until_ready(r1._fn(*a1)); t1s.append(_t.perf_counter() - t0)
        t0 = _t.perf_counter(); jax.block_until_ready(rR._fn(*aR)); tRs.append(_t.perf_counter() - t0)
    t1s, tRs = sorted(t1s), sorted(tRs)
    per = (tRs[len(tRs) // 2] - t1s[len(t1s) // 2]) / (reps - 1)
    return per * 1e9


# revision 16
# speedup vs baseline: 1.6414x; 1.0474x over previous
"""Trainium2 Bass kernel for nn_BaselineGCN (2-layer GCN + BN + mean-pool + MLP head).

Strategy (8 NeuronCores):
 - Nodes sharded contiguously across cores; each core owns the in-edges of its
   node shard (dst-sharding, per the graph-partitioning hint).
 - gcn_norm factorized: deg/dinv computed host-side (index/weight preprocessing);
   per-edge message = w_e * h'[src] with h' = dinv * (h @ W); per-dst scale by
   dinv[dst] folded into the ACT-engine epilogue.
 - The per-edge gather h'[src] runs on-device via SWDGE dma_gather (256B rows)
   from an AllGather'ed replica of h' in each core's DRAM, round-robined over 4
   SWDGE queues.
 - segment_sum becomes TensorE matmuls: per 128-edge chunk, a host-precomputed
   one-hot-times-weight fp8 matrix B[e, dst_local] is loaded as lhsT and PE
   accumulates B.T @ gathered_rows into the dst-block's PSUM tile.  The BN bias
   term (Cb) and the self-loop (+h'[dst]) are injected into the same PSUM
   accumulation via a K=1 outer-product matmul and a bf16 identity matmul, so
   the whole per-block epilogue is ONE ScalarE op: y = Relu(acc * dinv[dst]).
 - VectorE is kept almost empty: the v1 kernel was DVE+GPSIMD co-bottlenecked
   (DVE ~70% busy on epilogue/cast chains created backpressure that limited
   SWDGE descriptor-gen queue concurrency to ~2 of 4).
 - Graph mean-pool is a host-precomputed fp8 one-hot matmul accumulated inline
   with layer-1 epilogues; partials summed with an AllReduce; the tiny MLP
   head + log_softmax run on every core.
"""
import sys
import time

sys.path.insert(0, "/opt/trn_rl_repo")

import numpy as np

P = 128          # partitions / block size
NWIN = 4         # gather index windows (int16 range)
MAXCALL = 1024   # max indices per dma_gather (SWDGE ring capacity)
DMA_SCRATCH = 16384  # SWDGE ring carveout bytes (ring = this // 16 descs)
NQUEUES = 4      # SWDGE queues to round-robin
GBUFS = 16       # gather tile lookahead
BBUFS = 4        # B-slab lookahead (per-supergroup fp8 slabs)
PACCB = 4        # PSUM accumulator banks


# ---------------------------------------------------------------- host prep --

def _ceil(a, b):
    return -(-a // b)


class GCNStructure:
    """Graph partitioning + stream layout. Capacities are maxed across cores so
    the single SPMD program fits every core's data."""

    def __init__(self, src, dst, ew, batch, N, G, ncores):
        self.N, self.G, self.C = N, G, ncores
        NSH = N // ncores
        NB = _ceil(NSH, P)
        WS = _ceil(N // 2, NWIN // 2)  # window size in PAIRED rows
        assert N % ncores == 0
        assert WS <= 32767, "gather window exceeds int16"
        self.NSH, self.NB, self.WS = NSH, NB, WS
        self.LB = NSH - (NB - 1) * P  # rows in last block

        core = dst // NSH
        blk = (dst % NSH) // P
        # cell = (row-window, parity): row = src//2 packs 2 nodes per 256B
        win = ((src // 2) // WS) * 2 + (src & 1)
        key = (core * NB + blk) * NWIN + win
        # secondary sort by src: ascending gather addresses within each
        # (core, block, cell) group measurably improve DRAM read locality
        order = np.lexsort((src, key))
        self.src_s, self.dst_s, self.ew_s = src[order], dst[order], ew[order]
        counts = np.bincount(key, minlength=ncores * NB * NWIN).reshape(ncores, NB, NWIN)
        self.counts = counts
        cap = counts.max(axis=0)
        cap = _ceil(np.maximum(cap, 0), P) * P  # per (b, w), 0 stays 0
        self.cap = cap  # [NB, NWIN]

        # supergroups: consecutive blocks such that per-window call <= MAXCALL
        self.sgs = []
        cur = [0]
        for b in range(1, NB):
            trial = cur + [b]
            if all(cap[trial, w].sum() <= MAXCALL for w in range(NWIN)):
                cur = trial
            else:
                self.sgs.append(cur)
                cur = [b]
        self.sgs.append(cur)

        # layout: gather calls in (sg, w) order; chunks in (sg, b, w, j) order
        self.gcols = {}    # (sgi, w) -> columns in that call's tile
        self.icol = {}     # (sgi, w) -> start col (units of 16-idx) in idx stream
        self.coloff = {}   # (b, w) -> column offset inside its call tile
        sid = 0
        for sgi, sg in enumerate(self.sgs):
            for w in range(NWIN):
                cols = int(cap[sg, w].sum()) // P
                self.gcols[(sgi, w)] = cols
                self.icol[(sgi, w)] = sid
                off = 0
                for b in sg:
                    self.coloff[(b, w)] = off
                    off += int(cap[b, w]) // P
                sid += cols * 8  # n/16 = cols*128/16
        self.SID = max(sid, 8)
        self.CT = max(int(cap.sum()) // P, 1)
        self.GMAX = max(max(self.gcols.values(), default=1), 1)
        self.SGMAX = max(sum(int(cap[b].sum()) // P for b in sg) for sg in self.sgs)

        # per-core edge offsets into the sorted arrays, per (b, w)
        cum = np.zeros(ncores * NB * NWIN + 1, np.int64)
        np.cumsum(counts.reshape(-1), out=cum[1:])
        self.grp_start = cum  # index by (c*NB+b)*NWIN+w

        # batch / counts for pooling
        self.cnt = np.bincount(batch, minlength=G).astype(np.float32)
        self.inv_cnt = (1.0 / np.maximum(self.cnt, 1.0)).astype(np.float32)

    def core_streams(self, c, dinv):
        """Build per-core device streams: idx [128, SID] i16,
        dstloc/val [128, CT] f32, dinvb [128, NB] f32."""
        NB, WS, NSH = self.NB, self.WS, self.NSH
        idx_cols = np.zeros((128, self.SID), np.int16)
        dstloc = np.zeros((128, self.CT), np.float32)
        val = np.zeros((128, self.CT), np.float32)

        t = 0
        for sgi, sg in enumerate(self.sgs):
            # gather stream: (w, b) order
            for w in range(NWIN):
                col = self.icol[(sgi, w)]
                parts = []
                for b in sg:
                    g0 = self.grp_start[(c * NB + b) * NWIN + w]
                    g1 = self.grp_start[(c * NB + b) * NWIN + w + 1]
                    loc = (self.src_s[g0:g1] // 2 - (w // 2) * WS).astype(np.int16)
                    pad = int(self.cap[b, w]) - (g1 - g0)
                    parts.append(np.concatenate([loc, np.zeros(pad, np.int16)]))
                if parts:
                    flat = np.concatenate(parts)
                    if flat.size:
                        wrapped = np.tile(flat.reshape(-1, 16).T, (8, 1))
                        idx_cols[:, col:col + flat.size // 16] = wrapped
            # value/dst streams: (b, w, chunk) order
            for b in sg:
                for w in range(NWIN):
                    g0 = self.grp_start[(c * NB + b) * NWIN + w]
                    g1 = self.grp_start[(c * NB + b) * NWIN + w + 1]
                    n = g1 - g0
                    capbw = int(self.cap[b, w])
                    if capbw == 0:
                        continue
                    dl = np.zeros(capbw, np.float32)
                    vv = np.zeros(capbw, np.float32)
                    dl[:n] = (self.dst_s[g0:g1] - (c * NSH + b * P)).astype(np.float32)
                    vv[:n] = self.ew_s[g0:g1]
                    k = capbw // P
                    dstloc[:, t:t + k] = dl.reshape(k, P).T
                    val[:, t:t + k] = vv.reshape(k, P).T
                    t += k

        dinvb = np.zeros((128, NB), np.float32)
        sh_dinv = dinv[c * NSH:(c + 1) * NSH]
        for b in range(NB):
            nb = P if b < NB - 1 else self.LB
            dinvb[:nb, b] = sh_dinv[b * P:b * P + nb]
        return idx_cols, dstloc, val, dinvb

    def core_pool_oh(self, c, batch):
        """fp8 one-hot pooling matrix [128, NB*128]: [p, b*128+g] = (batch==g)."""
        import ml_dtypes
        FP8 = np.dtype(ml_dtypes.float8_e4m3)
        NB, NSH = self.NB, self.NSH
        oh = np.zeros((128, NB * 128), FP8)
        sh = batch[c * NSH:(c + 1) * NSH]
        for b in range(NB):
            nb = P if b < NB - 1 else self.LB
            g = sh[b * P:b * P + nb].astype(np.int64)
            oh[np.arange(nb), b * 128 + g] = 1.0
        return oh


def _host_prep(x, edge_index, batch, edge_attr, params, ncores):
    """All index-based preprocessing + BN folding. Returns (struct, in_maps)."""
    N, INDIM = x.shape
    G = int(params["cnt_G"])
    EPS = 1e-5

    src = np.asarray(edge_index[0], np.int64)
    dst = np.asarray(edge_index[1], np.int64)
    ew = np.asarray(edge_attr, np.float32)
    batch = np.asarray(batch, np.int64)

    deg = np.bincount(dst, weights=ew.astype(np.float64), minlength=N) + 1.0
    dinv = (1.0 / np.sqrt(deg)).astype(np.float32)

    st = GCNStructure(src, dst, ew, batch, N, G, ncores)

    def bnfold(g, be, m, v, bias):
        s = (g / np.sqrt(v + EPS)).astype(np.float32)
        cc = ((bias - m) * s + be).astype(np.float32)
        return s, cc

    S0, C0 = bnfold(params["g0"], params["be0"], params["m0"], params["v0"], params["b0"])
    S1, C1 = bnfold(params["g1"], params["be1"], params["m1"], params["v1"], params["b1"])
    Sf, Cf = bnfold(params["gf"], params["bef"], params["mf"], params["vf"], params["bf1"])

    # BN scale folded into the weight matrices (linear before the bias add)
    W0s = (np.asarray(params["W0"], np.float32) * S0[None, :]).astype(np.float32)
    W1s = (np.asarray(params["W1"], np.float32) * S1[None, :]).astype(np.float32)
    Wf1s = (np.asarray(params["Wf1"], np.float32) * Sf[None, :]).astype(np.float32)

    HID = params["W0"].shape[1]
    HHID = params["Wf1"].shape[1]
    NCLASS = params["Wf2"].shape[1]

    ident = np.eye(128, dtype=np.float32)

    NSH, NB = st.NSH, st.NB

    import ml_dtypes
    FP8 = np.dtype(ml_dtypes.float8_e4m3)
    BF16 = np.dtype(ml_dtypes.bfloat16)

    xpadT = np.zeros((ncores, INDIM, NB * P), BF16)
    xv = np.asarray(x, np.float32)
    for c in range(ncores):
        xpadT[c, :, :NSH] = xv[c * NSH:(c + 1) * NSH].T.astype(BF16)

    in_maps = []
    for c in range(ncores):
        idx_cols, dstloc, val, dinvb = st.core_streams(c, dinv)
        pool_oh = st.core_pool_oh(c, batch)
        # precomputed one-hot-times-weight B tiles, chunk-major [128, CT*128]
        bt3 = np.zeros((128, st.CT, 128), FP8)
        np.put_along_axis(bt3, dstloc.astype(np.int64)[:, :, None],
                          val.astype(FP8)[:, :, None], axis=2)
        # invd_flat[0, b*128+j] = 1/dinv at node (c, b, j); 0 for pad rows.
        # (single partition: Ldweights requires lhsT at partition 0)
        invd_flat = np.zeros((1, NB * 128), np.float32)
        sh_dinv = dinv[c * NSH:(c + 1) * NSH]
        for b in range(NB):
            nb = P if b < NB - 1 else st.LB
            invd_flat[0, b * 128:b * 128 + nb] = 1.0 / sh_dinv[b * P:b * P + nb]
        # cRep[b, :] = C (bias row replicated so lhsT/rhs base partitions match)
        cRep0 = np.tile(C0[None, :], (128, 1)).astype(BF16)
        cRep1 = np.tile(C1[None, :], (128, 1)).astype(BF16)
        in_maps.append(dict(
            xshT=xpadT[c],
            idxs=idx_cols,
            btiles=bt3.reshape(128, st.CT * 128),
            pool_oh=pool_oh, dinvb=dinvb,
            invd_flat=invd_flat.astype(BF16),
            cRep0=cRep0, cRep1=cRep1,
            w0=W0s.astype(BF16), w1=W1s, wf1=Wf1s,
            wf2=np.asarray(params["Wf2"], np.float32),
            cfb=np.tile(Cf[None, :], (128, 1)),
            bf2b=np.tile(np.asarray(params["bf2"], np.float32)[None, :], (128, 1)),
            invcnt=st.inv_cnt[:, None].copy(),
            ident=ident, identb=ident.astype(BF16),
        ))
    dims = dict(INDIM=INDIM, HID=HID, HHID=HHID, NCLASS=NCLASS)
    return st, in_maps, dims


# ------------------------------------------------------------- bass program --

def build_nc(st, dims, ncores, reps=1, fake_coll=False, no_gather=False, no_compute=False):
    from concourse import bass, mybir, bacc, tile

    INDIM, HID, HHID, NCLASS = dims["INDIM"], dims["HID"], dims["HHID"], dims["NCLASS"]
    N, G, NB, NSH, WS, LB = st.N, st.G, st.NB, st.NSH, st.WS, st.LB
    f32 = mybir.dt.float32
    bf16 = mybir.dt.bfloat16
    fp8 = mybir.dt.float8e4
    Alu = mybir.AluOpType
    Act = mybir.ActivationFunctionType

    nc = bacc.Bacc("TRN2", target_bir_lowering=False, debug=False,
                   enable_asserts=True, num_devices=ncores,
                   num_swdge_queues=NQUEUES,
                   dynamic_dma_scratch_size=DMA_SCRATCH)

    I = {}
    def inp(name, shape, dt=f32):
        I[name] = nc.dram_tensor(name, shape, dt, kind="ExternalInput")
        return I[name]

    inp("xshT", [INDIM, NB * P], bf16)
    inp("idxs", [128, st.SID], mybir.dt.int16)
    inp("btiles", [128, st.CT * 128], mybir.dt.float8e4)
    inp("pool_oh", [128, NB * 128], mybir.dt.float8e4)
    inp("dinvb", [128, NB])
    inp("invd_flat", [1, NB * 128], bf16)
    inp("cRep0", [128, HID], bf16); inp("cRep1", [128, HID], bf16)
    inp("w0", [INDIM, HID], bf16); inp("w1", [HID, HID])
    inp("wf1", [HID, HHID]); inp("wf2", [HHID, NCLASS])
    inp("cfb", [128, HHID])
    inp("bf2b", [128, NCLASS])
    inp("invcnt", [128, 1])
    inp("ident", [128, 128]); inp("identb", [128, 128], bf16)
    out_d = nc.dram_tensor("out", [G, NCLASS], f32, kind="ExternalOutput")

    qctr = [0]
    def next_q():
        q = qctr[0] % NQUEUES
        qctr[0] += 1
        return q

    with tile.TileContext(nc) as tc:
        import contextlib
        with contextlib.ExitStack() as ctx:
            const = ctx.enter_context(tc.tile_pool(name="const", bufs=1))
            stream = ctx.enter_context(tc.tile_pool(name="stream", bufs=1))
            xio = ctx.enter_context(tc.tile_pool(name="xio", bufs=3))
            xts = ctx.enter_context(tc.tile_pool(name="xts", bufs=3))
            hpool = ctx.enter_context(tc.tile_pool(name="hpool", bufs=NB))
            ypool = ctx.enter_context(tc.tile_pool(name="ypool", bufs=NB))
            y1pool = ctx.enter_context(tc.tile_pool(name="y1pool", bufs=NB))
            gpool = ctx.enter_context(tc.tile_pool(name="gpool", bufs=GBUFS))
            bpool = ctx.enter_context(tc.tile_pool(name="bpool", bufs=BBUFS))
            tmp = ctx.enter_context(tc.tile_pool(name="tmp", bufs=6))
            ptrans = ctx.enter_context(tc.tile_pool(name="ptrans", bufs=1, space="PSUM"))
            phw = ctx.enter_context(tc.tile_pool(name="phw", bufs=2, space="PSUM"))
            pacc = ctx.enter_context(tc.tile_pool(name="pacc", bufs=PACCB, space="PSUM"))
            ppool = ctx.enter_context(tc.tile_pool(name="ppool", bufs=1, space="PSUM"))
            dram = ctx.enter_context(tc.tile_pool(name="dram", bufs=1, space="DRAM"))

            # ---- constants into SBUF
            C = {}
            cdts = dict(w0=bf16, invd_flat=bf16, cRep0=bf16, cRep1=bf16, identb=bf16,
                        pool_oh=fp8)
            for nm in ["w0", "w1", "wf1", "wf2", "cfb", "bf2b", "invcnt",
                       "ident", "identb", "invd_flat", "cRep0", "cRep1",
                       "pool_oh"]:
                shape = list(I[nm].shape)
                tile_ = const.tile(shape, cdts.get(nm, f32), tag=nm)
                nc.sync.dma_start(out=tile_[:], in_=I[nm][:])
                C[nm] = tile_
            idx_t = stream.tile([128, st.SID], mybir.dt.int16, tag="idx")
            nc.sync.dma_start(out=idx_t[:], in_=I["idxs"][:])
            xbig = stream.tile([INDIM, NB * P], bf16, tag="xbig")
            nc.sync.dma_start(out=xbig[:], in_=I["xshT"][:])
            dinv_t = stream.tile([128, NB], f32, tag="dnv")
            nc.sync.dma_start(out=dinv_t[:], in_=I["dinvb"][:])

            shspace = "Shared" if ncores > 4 else "Local"
            bounce0 = dram.tile([NSH, HID], bf16, tag="bnc0")
            bounce1 = dram.tile([NSH, HID], bf16, tag="bnc1")
            bounce = [bounce0, bounce1]
            ar_in = dram.tile([G, HID], f32, tag="arin")

            # ---- phase A: h0' = dinv * (x @ W0), shard -> AllGather table0
            for _rep in range(reps):
              # per-rep Shared tables: a Shared DRAM tile allows only one writer
              table0 = dram.tile([N // 2, 2 * HID], bf16, tag=f"tab0_{_rep}", addr_space=shspace)
              table1 = dram.tile([N // 2, 2 * HID], bf16, tag=f"tab1_{_rep}", addr_space=shspace)
              table = [table0, table1]
              ar_out = dram.tile([G, HID], f32, tag=f"arout_{_rep}", addr_space=shspace)
              h_tiles = []
              for b in range(NB):
                  nb = P if b < NB - 1 else LB
                  hp = phw.tile([128, HID], f32, tag="hp")
                  nc.tensor.matmul(hp[:], lhsT=xbig[:, b * P:(b + 1) * P], rhs=C["w0"][:],
                                   start=True, stop=True)
                  hb16 = hpool.tile([128, HID], bf16, tag="h")
                  nc.scalar.activation(out=hb16[:], in_=hp[:], func=Act.Copy,
                                       scale=dinv_t[:, b:b + 1])
                  nc.scalar.dma_start(out=bounce[0][b * P:b * P + nb, :], in_=hb16[:nb, :])
                  h_tiles.append(hb16)

              if fake_coll:
                  nc.sync.dma_start(out=table[0][0:NSH // 2, :], in_=bounce[0][:])
              else:
                  nc.gpsimd.collective_compute(
                      "AllGather", Alu.bypass,
                      replica_groups=[list(range(ncores))],
                      ins=[bounce[0].opt()], outs=[table[0].opt()],
                  )

              # ---- GCN layers
              pp = ppool.tile([128, HID], f32, tag="pool")
              for l in range(2):
                  cRep = C["cRep0"] if l == 0 else C["cRep1"]
                  h1_tiles = []
                  t = 0
                  for sgi, sg in enumerate(st.sgs):
                      gt = {}
                      for w in range(NWIN):
                          cols = st.gcols[(sgi, w)]
                          if cols == 0:
                              continue
                          gbf = gpool.tile([128, st.GMAX, 2 * HID], bf16, tag="g")
                          ic = st.icol[(sgi, w)]
                          gt[w] = gbf
                          if no_gather:
                              continue
                          nc.gpsimd.dma_gather(
                              out_ap=gbf[:, :cols, :],
                              in_ap=table[l][(w // 2) * WS:min((w // 2 + 1) * WS, N // 2), :],
                              idxs_ap=idx_t[:, ic:ic + cols * 8],
                              num_idxs=cols * P,
                              num_idxs_reg=cols * P,
                              elem_size=2 * HID,
                              queue_num=next_q(),
                          )
                      sgch = sum(int(st.cap[b].sum()) // P for b in sg)
                      if sgch and not no_compute:
                          bsl = bpool.tile([128, st.SGMAX * 128], fp8, tag="B")
                          nc.sync.dma_start(
                              out=bsl[:, :sgch * 128],
                              in_=I["btiles"][:, t * 128:(t + sgch) * 128])
                      tsg = 0
                      for b in sg:
                          nchunks = 0 if no_compute else int(st.cap[b].sum()) // P
                          acc = pacc.tile([128, HID], f32, tag="acc")
                          # inject Cb/dinv[dst] (outer product, K=1), then the
                          # self-loop h'[dst] (identity matmul), then the edges
                          nc.tensor.matmul(acc[:],
                                           lhsT=C["invd_flat"][0:1, b * 128:(b + 1) * 128],
                                           rhs=cRep[0:1, :],
                                           start=True, stop=False)
                          nc.tensor.matmul(acc[:], lhsT=C["identb"][:],
                                           rhs=h_tiles[b][:],
                                           start=False, stop=(nchunks == 0))
                          done = 0
                          for w in range(NWIN if not no_compute else 0):
                              kk = int(st.cap[b, w]) // P
                              for j in range(kk):
                                  nc.tensor.matmul(
                                      acc[:], lhsT=bsl[:, tsg * 128:(tsg + 1) * 128],
                                      rhs=gt[w][:, st.coloff[(b, w)] + j,
                                                (w % 2) * HID:(w % 2 + 1) * HID],
                                      start=False, stop=(done == nchunks - 1))
                                  done += 1
                                  t += 1
                                  tsg += 1
                          nb = P if b < NB - 1 else LB
                          if l == 0:
                              # y0 (f32) -> transpose -> h1' = dinv*(y0 @ W1)
                              yb = ypool.tile([128, HID], f32, tag="y")
                              nc.scalar.activation(out=yb[:], in_=acc[:], func=Act.Relu,
                                                   scale=dinv_t[:, b:b + 1])
                              pt = ptrans.tile([128, 128], f32, tag="pt")
                              nc.tensor.transpose(pt[:HID, :], yb[:], C["ident"][:])
                              yTs = xts.tile([128, 128], f32, tag="xT")
                              nc.scalar.activation(out=yTs[:HID, :], in_=pt[:HID, :],
                                                   func=Act.Copy)
                              hp = phw.tile([128, HID], f32, tag="hp")
                              nc.tensor.matmul(hp[:], lhsT=yTs[:HID, :], rhs=C["w1"][:],
                                               start=True, stop=True)
                              h1b = hpool.tile([128, HID], bf16, tag="h")
                              nc.scalar.activation(out=h1b[:], in_=hp[:], func=Act.Copy,
                                                   scale=dinv_t[:, b:b + 1])
                              nc.scalar.dma_start(out=bounce[1][b * P:b * P + nb, :],
                                                  in_=h1b[:nb, :])
                              h1_tiles.append(h1b)
                          else:
                              # y1 (bf16) -> inline mean-pool accumulation
                              yb = y1pool.tile([128, HID], bf16, tag="y1")
                              nc.scalar.activation(out=yb[:], in_=acc[:], func=Act.Relu,
                                                   scale=dinv_t[:, b:b + 1])
                              nc.tensor.matmul(pp[:G, :],
                                               lhsT=C["pool_oh"][:nb, b * 128:b * 128 + G],
                                               rhs=yb[:nb, :],
                                               start=(b == 0), stop=(b == NB - 1))
                  if l == 0:
                      h_tiles = h1_tiles
                      if fake_coll:
                          nc.sync.dma_start(out=table[1][0:NSH // 2, :], in_=bounce[1][:])
                      else:
                          nc.gpsimd.collective_compute(
                              "AllGather", Alu.bypass,
                              replica_groups=[list(range(ncores))],
                              ins=[bounce[1].opt()], outs=[table[1].opt()],
                          )

              # ---- mean pool partials -> AllReduce -> head
              pooled = tmp.tile([128, HID], f32, tag="pl")
              nc.scalar.activation(out=pooled[:G, :], in_=pp[:G, :], func=Act.Copy)
              nc.sync.dma_start(out=ar_in[:], in_=pooled[:G, :])
              if fake_coll:
                  nc.sync.dma_start(out=ar_out[:], in_=ar_in[:])
              else:
                  nc.gpsimd.collective_compute(
                      "AllReduce", Alu.add,
                      replica_groups=[list(range(ncores))],
                      ins=[ar_in.opt()], outs=[ar_out.opt()],
                  )
              pooled2 = tmp.tile([128, HID], f32, tag="pl2")
              nc.sync.dma_start(out=pooled2[:G, :], in_=ar_out[:])
              nc.vector.tensor_scalar(out=pooled2[:G, :], in0=pooled2[:G, :],
                                      scalar1=C["invcnt"][:G, :], scalar2=None,
                                      op0=Alu.mult)

              # z = relu(Sf * (pooled @ Wf1) + Cf)
              pt = ptrans.tile([128, 128], f32, tag="pt")
              nc.tensor.transpose(pt[:HID, :G], pooled2[:G, :], C["ident"][:])
              pTs = xts.tile([128, 128], f32, tag="xT")
              nc.vector.tensor_copy(out=pTs[:HID, :G], in_=pt[:HID, :G])
              zp = phw.tile([128, HHID], f32, tag="hp")
              nc.tensor.matmul(zp[:G, :], lhsT=pTs[:HID, :G], rhs=C["wf1"][:],
                               start=True, stop=True)
              z = tmp.tile([128, HHID], f32, tag="z")
              nc.vector.tensor_tensor(out=z[:G, :], in0=zp[:G, :], in1=C["cfb"][:G, :], op=Alu.add)
              nc.vector.tensor_scalar(out=z[:G, :], in0=z[:G, :], scalar1=0.0,
                                      scalar2=None, op0=Alu.max)

              # logits = z @ Wf2 + bf2; out = log_softmax(logits)
              pt2 = ptrans.tile([128, 128], f32, tag="pt")
              nc.tensor.transpose(pt2[:HHID, :G], z[:G, :], C["ident"][:])
              zTs = xts.tile([128, 128], f32, tag="xT")
              nc.vector.tensor_copy(out=zTs[:HHID, :G], in_=pt2[:HHID, :G])
              lp = phw.tile([128, NCLASS], f32, tag="hp")
              nc.tensor.matmul(lp[:G, :], lhsT=zTs[:HHID, :G], rhs=C["wf2"][:],
                               start=True, stop=True)
              lg = tmp.tile([128, NCLASS], f32, tag="lg")
              nc.vector.tensor_tensor(out=lg[:G, :], in0=lp[:G, :], in1=C["bf2b"][:G, :], op=Alu.add)
              mx = tmp.tile([128, 1], f32, tag="mx")
              nc.vector.reduce_max(mx[:G, :], lg[:G, :], axis=mybir.AxisListType.X)
              nc.vector.tensor_scalar(out=lg[:G, :], in0=lg[:G, :], scalar1=mx[:G, :],
                                      scalar2=None, op0=Alu.subtract)
              ex = tmp.tile([128, NCLASS], f32, tag="ex")
              nc.scalar.activation(out=ex[:G, :], in_=lg[:G, :], func=Act.Exp)
              sm = tmp.tile([128, 1], f32, tag="sm")
              nc.vector.reduce_sum(sm[:G, :], ex[:G, :], axis=mybir.AxisListType.X)
              lsm = tmp.tile([128, 1], f32, tag="ls")
              nc.scalar.activation(out=lsm[:G, :], in_=sm[:G, :], func=Act.Ln)
              nc.vector.tensor_scalar(out=lg[:G, :], in0=lg[:G, :], scalar1=lsm[:G, :],
                                      scalar2=None, op0=Alu.subtract)
              nc.sync.dma_start(out=out_d[:], in_=lg[:G, :])

    nc.compile()
    return nc




# ------------------------------------------------------------ PJRT runner --

class SpmdRunner:
    """Run the compiled 8-core Bass module via PJRT (axon), mirroring
    concourse.bass2jax.run_bass_via_pjrt but keeping the jitted callable."""

    def __init__(self, nc, n_cores):
        import jax
        from jax.sharding import Mesh, PartitionSpec
        from jax.experimental.shard_map import shard_map
        from concourse import bass2jax, mybir as _mb
        from concourse.bass2jax import _bass_exec_p, install_neuronx_cc_hook
        install_neuronx_cc_hook()
        self.jax = jax
        self.nc = nc
        self.n_cores = n_cores
        partition_name = nc.partition_id_tensor.name if nc.partition_id_tensor else None
        in_names, out_names, out_avals, zero_outs = [], [], [], []
        for alloc in nc.m.functions[0].allocations:
            if not isinstance(alloc, _mb.MemoryLocationSet):
                continue
            name = alloc.memorylocations[0].name
            if alloc.kind == "ExternalInput":
                if name != partition_name:
                    in_names.append(name)
            elif alloc.kind == "ExternalOutput":
                shape = tuple(alloc.tensor_shape)
                dtype = _mb.dt.np(alloc.dtype)
                out_names.append(name)
                out_avals.append(jax.core.ShapedArray(shape, dtype))
                zero_outs.append(np.zeros(shape, dtype))
        self.in_names, self.out_names = in_names, out_names
        self.out_avals, self.zero_outs = out_avals, zero_outs
        n_params, n_outs = len(in_names), len(out_avals)
        self.n_params = n_params
        all_in_names = in_names + out_names + ([partition_name] if partition_name else [])

        def _body(*args):
            operands = list(args)
            if partition_name is not None:
                operands.append(bass2jax.partition_id_tensor())
            return tuple(_bass_exec_p.bind(
                *operands, out_avals=tuple(out_avals), in_names=tuple(all_in_names),
                out_names=tuple(out_names), lowering_input_output_aliases=(),
                sim_require_finite=True, sim_require_nnan=True, nc=nc))

        devices = jax.devices()[:n_cores]
        assert len(devices) == n_cores
        mesh = Mesh(np.asarray(devices), ("core",))
        self._sharding = jax.sharding.NamedSharding(mesh, PartitionSpec("core"))
        in_specs = (PartitionSpec("core"),) * (n_params + n_outs)
        out_specs = (PartitionSpec("core"),) * len(out_names)
        self._fn = jax.jit(
            shard_map(_body, mesh=mesh, in_specs=in_specs,
                      out_specs=out_specs, check_rep=False),
            keep_unused=True)

    def prepare(self, in_maps):
        per_core = [[np.asarray(m[name]) for name in self.in_names] for m in in_maps]
        concat_in = [np.concatenate([per_core[c][i] for c in range(self.n_cores)], axis=0)
                     for i in range(self.n_params)]
        concat_zeros = [np.zeros((self.n_cores * z.shape[0], *z.shape[1:]), z.dtype)
                        for z in self.zero_outs]
        return concat_in + concat_zeros

    def run(self, in_maps):
        out_arrs = self._fn(*self.prepare(in_maps))
        self.jax.block_until_ready(out_arrs)
        return self._split(out_arrs)

    def _split(self, out_arrs):
        return [{name: np.asarray(out_arrs[i]).reshape(self.n_cores, *self.out_avals[i].shape)[c]
                 for i, name in enumerate(self.out_names)}
                for c in range(self.n_cores)]

    def time(self, in_maps, iters=8):
        import time as _t
        args = self.prepare(in_maps)
        dargs = [self.jax.device_put(a, self._sharding) for a in args]
        out = self._fn(*dargs)
        self.jax.block_until_ready(out)
        results = self._split(out)
        times = []
        for _ in range(iters):
            t0 = _t.perf_counter()
            o = self._fn(*dargs)
            self.jax.block_until_ready(o)
            times.append(_t.perf_counter() - t0)
        return results, times


# ------------------------------------------------------------------- driver --

_CACHE = {}


def _get_runner(st, dims, ncores):
    nc = build_nc(st, dims, ncores)
    return SpmdRunner(nc, ncores)


def kernel(**inputs):
    x = np.asarray(inputs["x"], np.float32)
    edge_index = np.asarray(inputs["edge_index"])
    batch = np.asarray(inputs["batch"])
    edge_attr = np.asarray(inputs["edge_attr"], np.float32)
    G = 128
    params = {k: np.asarray(v) for k, v in inputs.items()
              if k not in ("x", "edge_index", "batch", "edge_attr", "pos")}
    params["cnt_G"] = G
    ncores = 8

    st, in_maps, dims = _host_prep(x, edge_index, batch, edge_attr, params, ncores)

    key = ("k", x.shape, edge_index.shape, st.SID, st.CT, st.GMAX,
           tuple(tuple(s) for s in st.sgs))
    if key not in _CACHE:
        _CACHE[key] = _get_runner(st, dims, ncores)
    runner = _CACHE[key]
    _LAST.update(st=st, dims=dims, ncores=ncores, in_maps=in_maps, runner=runner)
    results = runner.run(in_maps)
    return results[0]["out"]


_LAST = {}


def estimate_exec_ns(reps=16, iters=10):
    """Per-execution device time via wall-clock delta between a 1-rep NEFF and
    an in-NEFF `reps`-times-repeated body (cancels the axon dispatch floor).
    Median-based: the axon tunnel has heavy-tailed per-call jitter."""
    import time as _t
    import jax
    st, dims, ncores = _LAST["st"], _LAST["dims"], _LAST["ncores"]
    in_maps, r1 = _LAST["in_maps"], _LAST["runner"]
    rR = SpmdRunner(build_nc(st, dims, ncores, reps=reps), ncores)
    a1 = [jax.device_put(a, r1._sharding) for a in r1.prepare(in_maps)]
    aR = [jax.device_put(a, rR._sharding) for a in rR.prepare(in_maps)]
    jax.block_until_ready(r1._fn(*a1)); jax.block_until_ready(rR._fn(*aR))
    t1s, tRs = [], []
    for _ in range(iters):
        t0 = _t.perf_counter(); jax.block_until_ready(r1._fn(*a1)); t1s.append(_t.perf_counter() - t0)
        t0 = _t.perf_counter(); jax.block_until_ready(rR._fn(*aR)); tRs.append(_t.perf_counter() - t0)
    t1s, tRs = sorted(t1s), sorted(tRs)
    per = (tRs[len(tRs) // 2] - t1s[len(t1s) // 2]) / (reps - 1)
    return per * 1e9


# revision 17
# speedup vs baseline: 1.6569x; 1.0094x over previous
"""Trainium2 Bass kernel for nn_BaselineGCN (2-layer GCN + BN + mean-pool + MLP head).

Strategy (8 NeuronCores):
 - Nodes sharded contiguously across cores; each core owns the in-edges of its
   node shard (dst-sharding, per the graph-partitioning hint).
 - gcn_norm factorized: deg/dinv computed host-side (index/weight preprocessing);
   per-edge message = w_e * h'[src] with h' = dinv * (h @ W); per-dst scale by
   dinv[dst] folded into the ACT-engine epilogue.
 - The per-edge gather h'[src] runs on-device via SWDGE dma_gather (256B rows)
   from an AllGather'ed replica of h' in each core's DRAM, round-robined over 4
   SWDGE queues.
 - segment_sum becomes TensorE matmuls: per 128-edge chunk, a host-precomputed
   one-hot-times-weight fp8 matrix B[e, dst_local] is loaded as lhsT and PE
   accumulates B.T @ gathered_rows into the dst-block's PSUM tile.  The BN bias
   term (Cb) and the self-loop (+h'[dst]) are injected into the same PSUM
   accumulation via a K=1 outer-product matmul and a bf16 identity matmul, so
   the whole per-block epilogue is ONE ScalarE op: y = Relu(acc * dinv[dst]).
 - VectorE is kept almost empty: the v1 kernel was DVE+GPSIMD co-bottlenecked
   (DVE ~70% busy on epilogue/cast chains created backpressure that limited
   SWDGE descriptor-gen queue concurrency to ~2 of 4).
 - Graph mean-pool is a host-precomputed fp8 one-hot matmul accumulated inline
   with layer-1 epilogues; partials summed with an AllReduce; the tiny MLP
   head + log_softmax run on every core.
"""
import sys
import time

sys.path.insert(0, "/opt/trn_rl_repo")

import numpy as np

P = 128          # partitions / block size
NWIN = 4         # gather index windows (int16 range)
MAXCALL = 1024   # max indices per dma_gather (SWDGE ring capacity)
DMA_SCRATCH = 32768  # SWDGE ring carveout bytes (ring = this // 16 descs)
NQUEUES = 4      # SWDGE queues to round-robin
GBUFS = 20       # gather tile lookahead
BBUFS = 4        # B-slab lookahead (per-supergroup fp8 slabs)
PACCB = 4        # PSUM accumulator banks


# ---------------------------------------------------------------- host prep --

def _ceil(a, b):
    return -(-a // b)


class GCNStructure:
    """Graph partitioning + stream layout. Capacities are maxed across cores so
    the single SPMD program fits every core's data."""

    def __init__(self, src, dst, ew, batch, N, G, ncores):
        self.N, self.G, self.C = N, G, ncores
        NSH = N // ncores
        NB = _ceil(NSH, P)
        WS = _ceil(N // 2, NWIN // 2)  # window size in PAIRED rows
        assert N % ncores == 0
        assert WS <= 32767, "gather window exceeds int16"
        self.NSH, self.NB, self.WS = NSH, NB, WS
        self.LB = NSH - (NB - 1) * P  # rows in last block

        core = dst // NSH
        blk = (dst % NSH) // P
        # cell = (row-window, parity): row = src//2 packs 2 nodes per 256B
        win = ((src // 2) // WS) * 2 + (src & 1)
        key = (core * NB + blk) * NWIN + win
        # secondary sort by src: ascending gather addresses within each
        # (core, block, cell) group measurably improve DRAM read locality
        order = np.lexsort((src, key))
        self.src_s, self.dst_s, self.ew_s = src[order], dst[order], ew[order]
        counts = np.bincount(key, minlength=ncores * NB * NWIN).reshape(ncores, NB, NWIN)
        self.counts = counts
        cap = counts.max(axis=0)
        cap = _ceil(np.maximum(cap, 0), P) * P  # per (b, w), 0 stays 0
        self.cap = cap  # [NB, NWIN]

        # supergroups: consecutive blocks such that per-window call <= MAXCALL
        self.sgs = []
        cur = [0]
        for b in range(1, NB):
            trial = cur + [b]
            if all(cap[trial, w].sum() <= MAXCALL for w in range(NWIN)):
                cur = trial
            else:
                self.sgs.append(cur)
                cur = [b]
        self.sgs.append(cur)

        # layout: gather calls in (sg, w) order; chunks in (sg, b, w, j) order
        self.gcols = {}    # (sgi, w) -> columns in that call's tile
        self.icol = {}     # (sgi, w) -> start col (units of 16-idx) in idx stream
        self.coloff = {}   # (b, w) -> column offset inside its call tile
        sid = 0
        for sgi, sg in enumerate(self.sgs):
            for w in range(NWIN):
                cols = int(cap[sg, w].sum()) // P
                self.gcols[(sgi, w)] = cols
                self.icol[(sgi, w)] = sid
                off = 0
                for b in sg:
                    self.coloff[(b, w)] = off
                    off += int(cap[b, w]) // P
                sid += cols * 8  # n/16 = cols*128/16
        self.SID = max(sid, 8)
        self.CT = max(int(cap.sum()) // P, 1)
        self.GMAX = max(max(self.gcols.values(), default=1), 1)
        self.SGMAX = max(sum(int(cap[b].sum()) // P for b in sg) for sg in self.sgs)

        # per-core edge offsets into the sorted arrays, per (b, w)
        cum = np.zeros(ncores * NB * NWIN + 1, np.int64)
        np.cumsum(counts.reshape(-1), out=cum[1:])
        self.grp_start = cum  # index by (c*NB+b)*NWIN+w

        # batch / counts for pooling
        self.cnt = np.bincount(batch, minlength=G).astype(np.float32)
        self.inv_cnt = (1.0 / np.maximum(self.cnt, 1.0)).astype(np.float32)

    def core_streams(self, c, dinv):
        """Build per-core device streams: idx [128, SID] i16,
        dstloc/val [128, CT] f32, dinvb [128, NB] f32."""
        NB, WS, NSH = self.NB, self.WS, self.NSH
        idx_cols = np.zeros((128, self.SID), np.int16)
        dstloc = np.zeros((128, self.CT), np.float32)
        val = np.zeros((128, self.CT), np.float32)

        t = 0
        for sgi, sg in enumerate(self.sgs):
            # gather stream: (w, b) order
            for w in range(NWIN):
                col = self.icol[(sgi, w)]
                parts = []
                for b in sg:
                    g0 = self.grp_start[(c * NB + b) * NWIN + w]
                    g1 = self.grp_start[(c * NB + b) * NWIN + w + 1]
                    loc = (self.src_s[g0:g1] // 2 - (w // 2) * WS).astype(np.int16)
                    pad = int(self.cap[b, w]) - (g1 - g0)
                    parts.append(np.concatenate([loc, np.zeros(pad, np.int16)]))
                if parts:
                    flat = np.concatenate(parts)
                    if flat.size:
                        wrapped = np.tile(flat.reshape(-1, 16).T, (8, 1))
                        idx_cols[:, col:col + flat.size // 16] = wrapped
            # value/dst streams: (b, w, chunk) order
            for b in sg:
                for w in range(NWIN):
                    g0 = self.grp_start[(c * NB + b) * NWIN + w]
                    g1 = self.grp_start[(c * NB + b) * NWIN + w + 1]
                    n = g1 - g0
                    capbw = int(self.cap[b, w])
                    if capbw == 0:
                        continue
                    dl = np.zeros(capbw, np.float32)
                    vv = np.zeros(capbw, np.float32)
                    dl[:n] = (self.dst_s[g0:g1] - (c * NSH + b * P)).astype(np.float32)
                    vv[:n] = self.ew_s[g0:g1]
                    k = capbw // P
                    dstloc[:, t:t + k] = dl.reshape(k, P).T
                    val[:, t:t + k] = vv.reshape(k, P).T
                    t += k

        dinvb = np.zeros((128, NB), np.float32)
        sh_dinv = dinv[c * NSH:(c + 1) * NSH]
        for b in range(NB):
            nb = P if b < NB - 1 else self.LB
            dinvb[:nb, b] = sh_dinv[b * P:b * P + nb]
        return idx_cols, dstloc, val, dinvb

    def core_pool_oh(self, c, batch):
        """fp8 one-hot pooling matrix [128, NB*128]: [p, b*128+g] = (batch==g)."""
        import ml_dtypes
        FP8 = np.dtype(ml_dtypes.float8_e4m3)
        NB, NSH = self.NB, self.NSH
        oh = np.zeros((128, NB * 128), FP8)
        sh = batch[c * NSH:(c + 1) * NSH]
        for b in range(NB):
            nb = P if b < NB - 1 else self.LB
            g = sh[b * P:b * P + nb].astype(np.int64)
            oh[np.arange(nb), b * 128 + g] = 1.0
        return oh


def _host_prep(x, edge_index, batch, edge_attr, params, ncores):
    """All index-based preprocessing + BN folding. Returns (struct, in_maps)."""
    N, INDIM = x.shape
    G = int(params["cnt_G"])
    EPS = 1e-5

    src = np.asarray(edge_index[0], np.int64)
    dst = np.asarray(edge_index[1], np.int64)
    ew = np.asarray(edge_attr, np.float32)
    batch = np.asarray(batch, np.int64)

    deg = np.bincount(dst, weights=ew.astype(np.float64), minlength=N) + 1.0
    dinv = (1.0 / np.sqrt(deg)).astype(np.float32)

    st = GCNStructure(src, dst, ew, batch, N, G, ncores)

    def bnfold(g, be, m, v, bias):
        s = (g / np.sqrt(v + EPS)).astype(np.float32)
        cc = ((bias - m) * s + be).astype(np.float32)
        return s, cc

    S0, C0 = bnfold(params["g0"], params["be0"], params["m0"], params["v0"], params["b0"])
    S1, C1 = bnfold(params["g1"], params["be1"], params["m1"], params["v1"], params["b1"])
    Sf, Cf = bnfold(params["gf"], params["bef"], params["mf"], params["vf"], params["bf1"])

    # BN scale folded into the weight matrices (linear before the bias add)
    W0s = (np.asarray(params["W0"], np.float32) * S0[None, :]).astype(np.float32)
    W1s = (np.asarray(params["W1"], np.float32) * S1[None, :]).astype(np.float32)
    Wf1s = (np.asarray(params["Wf1"], np.float32) * Sf[None, :]).astype(np.float32)

    HID = params["W0"].shape[1]
    HHID = params["Wf1"].shape[1]
    NCLASS = params["Wf2"].shape[1]

    ident = np.eye(128, dtype=np.float32)

    NSH, NB = st.NSH, st.NB

    import ml_dtypes
    FP8 = np.dtype(ml_dtypes.float8_e4m3)
    BF16 = np.dtype(ml_dtypes.bfloat16)

    xpadT = np.zeros((ncores, INDIM, NB * P), BF16)
    xv = np.asarray(x, np.float32)
    for c in range(ncores):
        xpadT[c, :, :NSH] = xv[c * NSH:(c + 1) * NSH].T.astype(BF16)

    in_maps = []
    for c in range(ncores):
        idx_cols, dstloc, val, dinvb = st.core_streams(c, dinv)
        pool_oh = st.core_pool_oh(c, batch)
        # precomputed one-hot-times-weight B tiles, chunk-major [128, CT*128]
        bt3 = np.zeros((128, st.CT, 128), FP8)
        np.put_along_axis(bt3, dstloc.astype(np.int64)[:, :, None],
                          val.astype(FP8)[:, :, None], axis=2)
        # invd_flat[0, b*128+j] = 1/dinv at node (c, b, j); 0 for pad rows.
        # (single partition: Ldweights requires lhsT at partition 0)
        invd_flat = np.zeros((1, NB * 128), np.float32)
        sh_dinv = dinv[c * NSH:(c + 1) * NSH]
        for b in range(NB):
            nb = P if b < NB - 1 else st.LB
            invd_flat[0, b * 128:b * 128 + nb] = 1.0 / sh_dinv[b * P:b * P + nb]
        # cRep[b, :] = C (bias row replicated so lhsT/rhs base partitions match)
        cRep0 = np.tile(C0[None, :], (128, 1)).astype(BF16)
        cRep1 = np.tile(C1[None, :], (128, 1)).astype(BF16)
        in_maps.append(dict(
            xshT=xpadT[c],
            idxs=idx_cols,
            btiles=bt3.reshape(128, st.CT * 128),
            pool_oh=pool_oh, dinvb=dinvb,
            invd_flat=invd_flat.astype(BF16),
            cRep0=cRep0, cRep1=cRep1,
            w0=W0s.astype(BF16), w1=W1s, wf1=Wf1s,
            wf2=np.asarray(params["Wf2"], np.float32),
            cfb=np.tile(Cf[None, :], (128, 1)),
            bf2b=np.tile(np.asarray(params["bf2"], np.float32)[None, :], (128, 1)),
            invcnt=st.inv_cnt[:, None].copy(),
            ident=ident, identb=ident.astype(BF16),
        ))
    dims = dict(INDIM=INDIM, HID=HID, HHID=HHID, NCLASS=NCLASS)
    return st, in_maps, dims


# ------------------------------------------------------------- bass program --

def build_nc(st, dims, ncores, reps=1, fake_coll=False, no_gather=False, no_compute=False):
    from concourse import bass, mybir, bacc, tile

    INDIM, HID, HHID, NCLASS = dims["INDIM"], dims["HID"], dims["HHID"], dims["NCLASS"]
    N, G, NB, NSH, WS, LB = st.N, st.G, st.NB, st.NSH, st.WS, st.LB
    f32 = mybir.dt.float32
    bf16 = mybir.dt.bfloat16
    fp8 = mybir.dt.float8e4
    Alu = mybir.AluOpType
    Act = mybir.ActivationFunctionType

    nc = bacc.Bacc("TRN2", target_bir_lowering=False, debug=False,
                   enable_asserts=True, num_devices=ncores,
                   num_swdge_queues=NQUEUES,
                   dynamic_dma_scratch_size=DMA_SCRATCH)

    I = {}
    def inp(name, shape, dt=f32):
        I[name] = nc.dram_tensor(name, shape, dt, kind="ExternalInput")
        return I[name]

    inp("xshT", [INDIM, NB * P], bf16)
    inp("idxs", [128, st.SID], mybir.dt.int16)
    inp("btiles", [128, st.CT * 128], mybir.dt.float8e4)
    inp("pool_oh", [128, NB * 128], mybir.dt.float8e4)
    inp("dinvb", [128, NB])
    inp("invd_flat", [1, NB * 128], bf16)
    inp("cRep0", [128, HID], bf16); inp("cRep1", [128, HID], bf16)
    inp("w0", [INDIM, HID], bf16); inp("w1", [HID, HID])
    inp("wf1", [HID, HHID]); inp("wf2", [HHID, NCLASS])
    inp("cfb", [128, HHID])
    inp("bf2b", [128, NCLASS])
    inp("invcnt", [128, 1])
    inp("ident", [128, 128]); inp("identb", [128, 128], bf16)
    out_d = nc.dram_tensor("out", [G, NCLASS], f32, kind="ExternalOutput")

    qctr = [0]
    def next_q():
        q = qctr[0] % NQUEUES
        qctr[0] += 1
        return q

    with tile.TileContext(nc) as tc:
        import contextlib
        with contextlib.ExitStack() as ctx:
            const = ctx.enter_context(tc.tile_pool(name="const", bufs=1))
            stream = ctx.enter_context(tc.tile_pool(name="stream", bufs=1))
            xio = ctx.enter_context(tc.tile_pool(name="xio", bufs=3))
            xts = ctx.enter_context(tc.tile_pool(name="xts", bufs=3))
            hpool = ctx.enter_context(tc.tile_pool(name="hpool", bufs=NB))
            ypool = ctx.enter_context(tc.tile_pool(name="ypool", bufs=NB))
            y1pool = ctx.enter_context(tc.tile_pool(name="y1pool", bufs=NB))
            gpool = ctx.enter_context(tc.tile_pool(name="gpool", bufs=GBUFS))
            bpool = ctx.enter_context(tc.tile_pool(name="bpool", bufs=BBUFS))
            tmp = ctx.enter_context(tc.tile_pool(name="tmp", bufs=6))
            ptrans = ctx.enter_context(tc.tile_pool(name="ptrans", bufs=1, space="PSUM"))
            phw = ctx.enter_context(tc.tile_pool(name="phw", bufs=2, space="PSUM"))
            pacc = ctx.enter_context(tc.tile_pool(name="pacc", bufs=PACCB, space="PSUM"))
            ppool = ctx.enter_context(tc.tile_pool(name="ppool", bufs=1, space="PSUM"))
            dram = ctx.enter_context(tc.tile_pool(name="dram", bufs=1, space="DRAM"))

            # ---- constants into SBUF
            C = {}
            cdts = dict(w0=bf16, invd_flat=bf16, cRep0=bf16, cRep1=bf16, identb=bf16,
                        pool_oh=fp8)
            for nm in ["w0", "w1", "wf1", "wf2", "cfb", "bf2b", "invcnt",
                       "ident", "identb", "invd_flat", "cRep0", "cRep1",
                       "pool_oh"]:
                shape = list(I[nm].shape)
                tile_ = const.tile(shape, cdts.get(nm, f32), tag=nm)
                nc.sync.dma_start(out=tile_[:], in_=I[nm][:])
                C[nm] = tile_
            idx_t = stream.tile([128, st.SID], mybir.dt.int16, tag="idx")
            nc.sync.dma_start(out=idx_t[:], in_=I["idxs"][:])
            xbig = stream.tile([INDIM, NB * P], bf16, tag="xbig")
            nc.sync.dma_start(out=xbig[:], in_=I["xshT"][:])
            dinv_t = stream.tile([128, NB], f32, tag="dnv")
            nc.sync.dma_start(out=dinv_t[:], in_=I["dinvb"][:])

            shspace = "Shared" if ncores > 4 else "Local"
            bounce0 = dram.tile([NSH, HID], bf16, tag="bnc0")
            bounce1 = dram.tile([NSH, HID], bf16, tag="bnc1")
            bounce = [bounce0, bounce1]
            ar_in = dram.tile([G, HID], f32, tag="arin")

            # ---- phase A: h0' = dinv * (x @ W0), shard -> AllGather table0
            for _rep in range(reps):
              # per-rep Shared tables: a Shared DRAM tile allows only one writer
              table0 = dram.tile([N // 2, 2 * HID], bf16, tag=f"tab0_{_rep}", addr_space=shspace)
              table1 = dram.tile([N // 2, 2 * HID], bf16, tag=f"tab1_{_rep}", addr_space=shspace)
              table = [table0, table1]
              ar_out = dram.tile([G, HID], f32, tag=f"arout_{_rep}", addr_space=shspace)
              h_tiles = []
              for b in range(NB):
                  nb = P if b < NB - 1 else LB
                  hp = phw.tile([128, HID], f32, tag="hp")
                  nc.tensor.matmul(hp[:], lhsT=xbig[:, b * P:(b + 1) * P], rhs=C["w0"][:],
                                   start=True, stop=True)
                  hb16 = hpool.tile([128, HID], bf16, tag="h")
                  nc.scalar.activation(out=hb16[:], in_=hp[:], func=Act.Copy,
                                       scale=dinv_t[:, b:b + 1])
                  nc.scalar.dma_start(out=bounce[0][b * P:b * P + nb, :], in_=hb16[:nb, :])
                  h_tiles.append(hb16)

              if fake_coll:
                  nc.sync.dma_start(out=table[0][0:NSH // 2, :], in_=bounce[0][:])
              else:
                  nc.gpsimd.collective_compute(
                      "AllGather", Alu.bypass,
                      replica_groups=[list(range(ncores))],
                      ins=[bounce[0].opt()], outs=[table[0].opt()],
                  )

              # ---- GCN layers
              pp = ppool.tile([128, HID], f32, tag="pool")
              for l in range(2):
                  cRep = C["cRep0"] if l == 0 else C["cRep1"]
                  h1_tiles = []
                  t = 0
                  for sgi, sg in enumerate(st.sgs):
                      gt = {}
                      for w in range(NWIN):
                          cols = st.gcols[(sgi, w)]
                          if cols == 0:
                              continue
                          gbf = gpool.tile([128, st.GMAX, 2 * HID], bf16, tag="g")
                          ic = st.icol[(sgi, w)]
                          gt[w] = gbf
                          if no_gather:
                              continue
                          nc.gpsimd.dma_gather(
                              out_ap=gbf[:, :cols, :],
                              in_ap=table[l][(w // 2) * WS:min((w // 2 + 1) * WS, N // 2), :],
                              idxs_ap=idx_t[:, ic:ic + cols * 8],
                              num_idxs=cols * P,
                              num_idxs_reg=cols * P,
                              elem_size=2 * HID,
                              queue_num=next_q(),
                          )
                      sgch = sum(int(st.cap[b].sum()) // P for b in sg)
                      if sgch and not no_compute:
                          bsl = bpool.tile([128, st.SGMAX * 128], fp8, tag="B")
                          nc.sync.dma_start(
                              out=bsl[:, :sgch * 128],
                              in_=I["btiles"][:, t * 128:(t + sgch) * 128])
                      tsg = 0
                      for b in sg:
                          nchunks = 0 if no_compute else int(st.cap[b].sum()) // P
                          acc = pacc.tile([128, HID], f32, tag="acc")
                          # inject Cb/dinv[dst] (outer product, K=1), then the
                          # self-loop h'[dst] (identity matmul), then the edges
                          nc.tensor.matmul(acc[:],
                                           lhsT=C["invd_flat"][0:1, b * 128:(b + 1) * 128],
                                           rhs=cRep[0:1, :],
                                           start=True, stop=False)
                          nc.tensor.matmul(acc[:], lhsT=C["identb"][:],
                                           rhs=h_tiles[b][:],
                                           start=False, stop=(nchunks == 0))
                          done = 0
                          for w in range(NWIN if not no_compute else 0):
                              kk = int(st.cap[b, w]) // P
                              for j in range(kk):
                                  nc.tensor.matmul(
                                      acc[:], lhsT=bsl[:, tsg * 128:(tsg + 1) * 128],
                                      rhs=gt[w][:, st.coloff[(b, w)] + j,
                                                (w % 2) * HID:(w % 2 + 1) * HID],
                                      start=False, stop=(done == nchunks - 1))
                                  done += 1
                                  t += 1
                                  tsg += 1
                          nb = P if b < NB - 1 else LB
                          if l == 0:
                              # y0 (f32) -> transpose -> h1' = dinv*(y0 @ W1)
                              yb = ypool.tile([128, HID], f32, tag="y")
                              nc.scalar.activation(out=yb[:], in_=acc[:], func=Act.Relu,
                                                   scale=dinv_t[:, b:b + 1])
                              pt = ptrans.tile([128, 128], f32, tag="pt")
                              nc.tensor.transpose(pt[:HID, :], yb[:], C["ident"][:])
                              yTs = xts.tile([128, 128], f32, tag="xT")
                              nc.scalar.activation(out=yTs[:HID, :], in_=pt[:HID, :],
                                                   func=Act.Copy)
                              hp = phw.tile([128, HID], f32, tag="hp")
                              nc.tensor.matmul(hp[:], lhsT=yTs[:HID, :], rhs=C["w1"][:],
                                               start=True, stop=True)
                              h1b = hpool.tile([128, HID], bf16, tag="h")
                              nc.scalar.activation(out=h1b[:], in_=hp[:], func=Act.Copy,
                                                   scale=dinv_t[:, b:b + 1])
                              nc.scalar.dma_start(out=bounce[1][b * P:b * P + nb, :],
                                                  in_=h1b[:nb, :])
                              h1_tiles.append(h1b)
                          else:
                              # y1 (bf16) -> inline mean-pool accumulation
                              yb = y1pool.tile([128, HID], bf16, tag="y1")
                              nc.scalar.activation(out=yb[:], in_=acc[:], func=Act.Relu,
                                                   scale=dinv_t[:, b:b + 1])
                              nc.tensor.matmul(pp[:G, :],
                                               lhsT=C["pool_oh"][:nb, b * 128:b * 128 + G],
                                               rhs=yb[:nb, :],
                                               start=(b == 0), stop=(b == NB - 1))
                  if l == 0:
                      h_tiles = h1_tiles
                      if fake_coll:
                          nc.sync.dma_start(out=table[1][0:NSH // 2, :], in_=bounce[1][:])
                      else:
                          nc.gpsimd.collective_compute(
                              "AllGather", Alu.bypass,
                              replica_groups=[list(range(ncores))],
                              ins=[bounce[1].opt()], outs=[table[1].opt()],
                          )

              # ---- mean pool partials -> AllReduce -> head
              pooled = tmp.tile([128, HID], f32, tag="pl")
              nc.scalar.activation(out=pooled[:G, :], in_=pp[:G, :], func=Act.Copy)
              nc.sync.dma_start(out=ar_in[:], in_=pooled[:G, :])
              if fake_coll:
                  nc.sync.dma_start(out=ar_out[:], in_=ar_in[:])
              else:
                  nc.gpsimd.collective_compute(
                      "AllReduce", Alu.add,
                      replica_groups=[list(range(ncores))],
                      ins=[ar_in.opt()], outs=[ar_out.opt()],
                  )
              pooled2 = tmp.tile([128, HID], f32, tag="pl2")
              nc.sync.dma_start(out=pooled2[:G, :], in_=ar_out[:])
              nc.vector.tensor_scalar(out=pooled2[:G, :], in0=pooled2[:G, :],
                                      scalar1=C["invcnt"][:G, :], scalar2=None,
                                      op0=Alu.mult)

              # z = relu(Sf * (pooled @ Wf1) + Cf)
              pt = ptrans.tile([128, 128], f32, tag="pt")
              nc.tensor.transpose(pt[:HID, :G], pooled2[:G, :], C["ident"][:])
              pTs = xts.tile([128, 128], f32, tag="xT")
              nc.vector.tensor_copy(out=pTs[:HID, :G], in_=pt[:HID, :G])
              zp = phw.tile([128, HHID], f32, tag="hp")
              nc.tensor.matmul(zp[:G, :], lhsT=pTs[:HID, :G], rhs=C["wf1"][:],
                               start=True, stop=True)
              z = tmp.tile([128, HHID], f32, tag="z")
              nc.vector.tensor_tensor(out=z[:G, :], in0=zp[:G, :], in1=C["cfb"][:G, :], op=Alu.add)
              nc.vector.tensor_scalar(out=z[:G, :], in0=z[:G, :], scalar1=0.0,
                                      scalar2=None, op0=Alu.max)

              # logits = z @ Wf2 + bf2; out = log_softmax(logits)
              pt2 = ptrans.tile([128, 128], f32, tag="pt")
              nc.tensor.transpose(pt2[:HHID, :G], z[:G, :], C["ident"][:])
              zTs = xts.tile([128, 128], f32, tag="xT")
              nc.vector.tensor_copy(out=zTs[:HHID, :G], in_=pt2[:HHID, :G])
              lp = phw.tile([128, NCLASS], f32, tag="hp")
              nc.tensor.matmul(lp[:G, :], lhsT=zTs[:HHID, :G], rhs=C["wf2"][:],
                               start=True, stop=True)
              lg = tmp.tile([128, NCLASS], f32, tag="lg")
              nc.vector.tensor_tensor(out=lg[:G, :], in0=lp[:G, :], in1=C["bf2b"][:G, :], op=Alu.add)
              mx = tmp.tile([128, 1], f32, tag="mx")
              nc.vector.reduce_max(mx[:G, :], lg[:G, :], axis=mybir.AxisListType.X)
              nc.vector.tensor_scalar(out=lg[:G, :], in0=lg[:G, :], scalar1=mx[:G, :],
                                      scalar2=None, op0=Alu.subtract)
              ex = tmp.tile([128, NCLASS], f32, tag="ex")
              nc.scalar.activation(out=ex[:G, :], in_=lg[:G, :], func=Act.Exp)
              sm = tmp.tile([128, 1], f32, tag="sm")
              nc.vector.reduce_sum(sm[:G, :], ex[:G, :], axis=mybir.AxisListType.X)
              lsm = tmp.tile([128, 1], f32, tag="ls")
              nc.scalar.activation(out=lsm[:G, :], in_=sm[:G, :], func=Act.Ln)
              nc.vector.tensor_scalar(out=lg[:G, :], in0=lg[:G, :], scalar1=lsm[:G, :],
                                      scalar2=None, op0=Alu.subtract)
              nc.sync.dma_start(out=out_d[:], in_=lg[:G, :])

    nc.compile()
    return nc




# ------------------------------------------------------------ PJRT runner --

class SpmdRunner:
    """Run the compiled 8-core Bass module via PJRT (axon), mirroring
    concourse.bass2jax.run_bass_via_pjrt but keeping the jitted callable."""

    def __init__(self, nc, n_cores):
        import jax
        from jax.sharding import Mesh, PartitionSpec
        from jax.experimental.shard_map import shard_map
        from concourse import bass2jax, mybir as _mb
        from concourse.bass2jax import _bass_exec_p, install_neuronx_cc_hook
        install_neuronx_cc_hook()
        self.jax = jax
        self.nc = nc
        self.n_cores = n_cores
        partition_name = nc.partition_id_tensor.name if nc.partition_id_tensor else None
        in_names, out_names, out_avals, zero_outs = [], [], [], []
        for alloc in nc.m.functions[0].allocations:
            if not isinstance(alloc, _mb.MemoryLocationSet):
                continue
            name = alloc.memorylocations[0].name
            if alloc.kind == "ExternalInput":
                if name != partition_name:
                    in_names.append(name)
            elif alloc.kind == "ExternalOutput":
                shape = tuple(alloc.tensor_shape)
                dtype = _mb.dt.np(alloc.dtype)
                out_names.append(name)
                out_avals.append(jax.core.ShapedArray(shape, dtype))
                zero_outs.append(np.zeros(shape, dtype))
        self.in_names, self.out_names = in_names, out_names
        self.out_avals, self.zero_outs = out_avals, zero_outs
        n_params, n_outs = len(in_names), len(out_avals)
        self.n_params = n_params
        all_in_names = in_names + out_names + ([partition_name] if partition_name else [])

        def _body(*args):
            operands = list(args)
            if partition_name is not None:
                operands.append(bass2jax.partition_id_tensor())
            return tuple(_bass_exec_p.bind(
                *operands, out_avals=tuple(out_avals), in_names=tuple(all_in_names),
                out_names=tuple(out_names), lowering_input_output_aliases=(),
                sim_require_finite=True, sim_require_nnan=True, nc=nc))

        devices = jax.devices()[:n_cores]
        assert len(devices) == n_cores
        mesh = Mesh(np.asarray(devices), ("core",))
        self._sharding = jax.sharding.NamedSharding(mesh, PartitionSpec("core"))
        in_specs = (PartitionSpec("core"),) * (n_params + n_outs)
        out_specs = (PartitionSpec("core"),) * len(out_names)
        self._fn = jax.jit(
            shard_map(_body, mesh=mesh, in_specs=in_specs,
                      out_specs=out_specs, check_rep=False),
            keep_unused=True)

    def prepare(self, in_maps):
        per_core = [[np.asarray(m[name]) for name in self.in_names] for m in in_maps]
        concat_in = [np.concatenate([per_core[c][i] for c in range(self.n_cores)], axis=0)
                     for i in range(self.n_params)]
        concat_zeros = [np.zeros((self.n_cores * z.shape[0], *z.shape[1:]), z.dtype)
                        for z in self.zero_outs]
        return concat_in + concat_zeros

    def run(self, in_maps):
        out_arrs = self._fn(*self.prepare(in_maps))
        self.jax.block_until_ready(out_arrs)
        return self._split(out_arrs)

    def _split(self, out_arrs):
        return [{name: np.asarray(out_arrs[i]).reshape(self.n_cores, *self.out_avals[i].shape)[c]
                 for i, name in enumerate(self.out_names)}
                for c in range(self.n_cores)]

    def time(self, in_maps, iters=8):
        import time as _t
        args = self.prepare(in_maps)
        dargs = [self.jax.device_put(a, self._sharding) for a in args]
        out = self._fn(*dargs)
        self.jax.block_until_ready(out)
        results = self._split(out)
        times = []
        for _ in range(iters):
            t0 = _t.perf_counter()
            o = self._fn(*dargs)
            self.jax.block_until_ready(o)
            times.append(_t.perf_counter() - t0)
        return results, times


# ------------------------------------------------------------------- driver --

_CACHE = {}


def _get_runner(st, dims, ncores):
    nc = build_nc(st, dims, ncores)
    return SpmdRunner(nc, ncores)


def kernel(**inputs):
    x = np.asarray(inputs["x"], np.float32)
    edge_index = np.asarray(inputs["edge_index"])
    batch = np.asarray(inputs["batch"])
    edge_attr = np.asarray(inputs["edge_attr"], np.float32)
    G = 128
    params = {k: np.asarray(v) for k, v in inputs.items()
              if k not in ("x", "edge_index", "batch", "edge_attr", "pos")}
    params["cnt_G"] = G
    ncores = 8

    st, in_maps, dims = _host_prep(x, edge_index, batch, edge_attr, params, ncores)

    key = ("k", x.shape, edge_index.shape, st.SID, st.CT, st.GMAX,
           tuple(tuple(s) for s in st.sgs))
    if key not in _CACHE:
        _CACHE[key] = _get_runner(st, dims, ncores)
    runner = _CACHE[key]
    _LAST.update(st=st, dims=dims, ncores=ncores, in_maps=in_maps, runner=runner)
    results = runner.run(in_maps)
    return results[0]["out"]


_LAST = {}


def estimate_exec_ns(reps=16, iters=10):
    """Per-execution device time via wall-clock delta between a 1-rep NEFF and
    an in-NEFF `reps`-times-repeated body (cancels the axon dispatch floor).
    Median-based: the axon tunnel has heavy-tailed per-call jitter."""
    import time as _t
    import jax
    st, dims, ncores = _LAST["st"], _LAST["dims"], _LAST["ncores"]
    in_maps, r1 = _LAST["in_maps"], _LAST["runner"]
    rR = SpmdRunner(build_nc(st, dims, ncores, reps=reps), ncores)
    a1 = [jax.device_put(a, r1._sharding) for a in r1.prepare(in_maps)]
    aR = [jax.device_put(a, rR._sharding) for a in rR.prepare(in_maps)]
    jax.block_until_ready(r1._fn(*a1)); jax.block_until_ready(rR._fn(*aR))
    t1s, tRs = [], []
    for _ in range(iters):
        t0 = _t.perf_counter(); jax.block_until_ready(r1._fn(*a1)); t1s.append(_t.perf_counter() - t0)
        t0 = _t.perf_counter(); jax.block_until_ready(rR._fn(*aR)); tRs.append(_t.perf_counter() - t0)
    t1s, tRs = sorted(t1s), sorted(tRs)
    per = (tRs[len(tRs) // 2] - t1s[len(t1s) // 2]) / (reps - 1)
    return per * 1e9
